# revision 28
# baseline (speedup 1.0000x reference)
"""Trainium2 Bass kernel for nn_DecoderBlock (B=4,T=S=E=1024,H=16,D=64) on 8 cores.

Active variant ("v2", see VARIANT below): communication-free sharding,
core = (batch b, T-half h).  Each core computes its 512 query tokens for all
16 heads plus the full FFN; K/V work is duplicated across the pair so no
collective is ever issued (pairwise AllReduce on this fleet is slow and
jittery).  Self-attn keys are permuted own-half-first so the SPMD program is
identical on every core; the causal structure is handled by a diagonal-block
multiplicative mask plus zeroing V (and its denominator ones-row) for key
blocks invisible to the whole core.

The residual stream lives transposed on-chip as x^T [E(partitions), T(free)]
so every matmul is already in lhsT/rhs layout.  LayerNorm statistics are
computed with an all-ones [128x128] stationary matmul so mean/rstd are
broadcast to all partitions and the row math runs 128 lanes wide; rstd uses
sqrt + vector.reciprocal (reciprocal_approx_fast miscompiles on HW).
Attention is software-pipelined per head (QK of head h+1 issues before AV of
head h); softmax denominators ride as a 65th V row, are gathered to a [16,512]
tile with tiny SBUF->SBUF DMAs, inverted by one batched reciprocal, and
broadcast back through a constant selector-matrix matmul.

Older variants ("tp2" = DP4 x TP2 with pairwise AllReduce, "nocc" = first
comm-free version) are kept for reference and A/B timing.
"""
import sys

sys.path.insert(0, "/opt/trn_rl_repo")

import numpy as np
import ml_dtypes

import concourse.bass as bass
import concourse.bacc as bacc
import concourse.mybir as mybir
import concourse.tile as tile

BF16 = mybir.dt.bfloat16
F32 = mybir.dt.float32
AF = mybir.ActivationFunctionType
OP = mybir.AluOpType

B, T, S, E, H, D = 4, 1024, 1024, 1024, 16, 64
HL = H // 2          # heads per core (TP-2)
FF = 4 * E // 2      # ffn hidden per core
KO = E // 128        # 8 partition subtiles of E
NC_ = 512            # matmul free-dim chunk
CC = T // NC_        # 2 chunks over T
PAIRS = [[0, 1], [2, 3], [4, 5], [6, 7]]


SKIP_CC = False


def build(nbody=1):
    nc = bacc.Bacc(num_devices=8)

    def P(name, shape, dt):
        return nc.declare_dram_parameter(name, shape, dt, isOutput=False)

    xT = P("xT", [E, T], F32)
    caT = P("caT", [E, S], BF16)
    wq, wk, wv = P("wq", [E, 512], BF16), P("wk", [E, 512], BF16), P("wv", [E, 512], BF16)
    wqc, wkc, wvc = P("wqc", [E, 512], BF16), P("wkc", [E, 512], BF16), P("wvc", [E, 512], BF16)
    wo, woc = P("wo", [512, E], BF16), P("woc", [512, E], BF16)
    w1, w2 = P("w1", [E, FF], BF16), P("w2", [FF, E], BF16)
    gb = [P(f"gb{i}", [2, E], BF16) for i in (1, 2, 3)]
    gpp_d = [P(f"g{i}", [E], F32) for i in (1, 2, 3)]
    bo2, bo2c, b22 = P("bo2", [E], F32), P("bo2c", [E], F32), P("b22", [E], F32)
    b1r = P("b1r", [FF], F32)
    cmask = P("cmask", [128, 4, 512], BF16)
    out_xT = nc.declare_dram_parameter("out_xT", [E, T], F32, isOutput=True)

    with tile.TileContext(nc) as tc:
        with tc.tile_pool(name="persist", bufs=1) as pp:
            xT_sb = pp.tile([128, KO, T], F32, tag="xT")
            for ko in range(KO):
                nc.sync.dma_start(out=xT_sb[:, ko, :],
                                  in_=xT[ko * 128:(ko + 1) * 128, :])
            ca_sb = pp.tile([128, KO, S], BF16, tag="ca")
            nc.sync.dma_start(out=ca_sb[:], in_=caT.rearrange("(ko p) t -> p ko t", p=128))
            cm_sb = pp.tile([128, 4, 512], BF16, tag="cm")
            nc.sync.dma_start(out=cm_sb[:], in_=cmask[:])
            ones_bf = pp.tile([128, 512], BF16, tag="ones")
            nc.vector.memset(ones_bf[:], 1.0)
            gl_sb, bl_sb, gpp = [], [], []
            for i in range(3):
                ta = pp.tile([1, KO, 128], BF16, tag=f"gl{i}")
                nc.sync.dma_start(out=ta[:], in_=gb[i].rearrange("a (ko m) -> a ko m", m=128)[0:1])
                gl_sb.append(ta)
                tb = pp.tile([1, KO, 128], BF16, tag=f"bl{i}")
                nc.sync.dma_start(out=tb[:], in_=gb[i].rearrange("a (ko m) -> a ko m", m=128)[1:2])
                bl_sb.append(tb)
                t2 = pp.tile([128, KO], F32, tag=f"gpp{i}")
                with nc.allow_non_contiguous_dma(reason="tiny LN vector"):
                    nc.sync.dma_start(out=t2[:], in_=gpp_d[i].rearrange("(ko p) -> p ko", p=128))
                gpp.append(t2)
            bpp = []
            for nm, d in (("bo2", bo2), ("bo2c", bo2c), ("b22", b22)):
                t_ = pp.tile([128, KO], F32, tag=nm)
                with nc.allow_non_contiguous_dma(reason="tiny bias vector"):
                    nc.scalar.dma_start(out=t_[:], in_=d.rearrange("(ko p) -> p ko", p=128))
                bpp.append(t_)
            eps_t = pp.tile([1, 1], F32, tag="eps")
            nc.vector.memset(eps_t[:], 1e-5)
            b1pp = pp.tile([128, FF // 128], F32, tag="b1")
            with nc.allow_non_contiguous_dma(reason="tiny bias vector"):
                nc.sync.dma_start(out=b1pp[:], in_=b1r.rearrange("(m p) -> p m", p=128))

            for ibody in range(nbody):
                _body(nc, tc, ibody, xT_sb, ca_sb, cm_sb, ones_bf, (gl_sb, bl_sb), gpp,
                      bpp, b1pp, eps_t,
                      dict(wq=wq, wk=wk, wv=wv, wqc=wqc, wkc=wkc, wvc=wvc,
                           wo=wo, woc=woc, w1=w1, w2=w2, xT=xT),
                      out_xT)
    nc.finalize()
    return nc


def _body(nc, tc, ibody, xT_sb, ca_sb, cm_sb, ones_bf, gbl, gpp,
          bpp, b1pp, eps_t, W, out_xT):
    gl_sb, bl_sb = gbl
    bo2pp, bo2cpp, b22pp = bpp
    ar = {}
    for k in (1, 2, 3):
        ar[k] = [(nc.dram_tensor(f"ar{k}_{ibody}_{c}_in", [E, NC_], F32),
                  nc.dram_tensor(f"ar{k}_{ibody}_{c}_out", [E, NC_], F32))
                 for c in range(CC)]

    if ibody > 0:
        # re-load pristine x for the timing replica
        for ko in range(KO):
            nc.sync.dma_start(out=xT_sb[:, ko, :],
                              in_=W["xT"][ko * 128:(ko + 1) * 128, :])

    with tc.tile_pool(name=f"A{ibody}", bufs=1) as pa, \
         tc.tile_pool(name=f"ps{ibody}", bufs=8, space="PSUM") as pspool:

        def ps():
            return pspool.tile([128, NC_], F32, tag="ps", name="ps")

        def layer_norm(i):
            """LN over partitions of xT_sb -> bf16 tile [128, KO, T]."""
            ln = pa.tile([128, KO, T], BF16, tag="lnout", name="ln")
            for c in range(CC):
                cs = slice(c * NC_, (c + 1) * NC_)
                xb = pa.tile([128, KO, NC_], BF16, tag="stat", bufs=2, name="xb")
                for ko in range(KO):
                    nc.scalar.copy(out=xb[:, ko, :], in_=xT_sb[:, ko, cs])
                sq = pa.tile([128, KO, NC_], BF16, tag="stat", bufs=2, name="sq")
                nc.scalar.activation(sq[:], xb[:], AF.Square)
                ps1, ps2 = ps(), ps()
                for ko in range(KO):
                    nc.tensor.matmul(ps1[0:1, :], ones_bf[:, 0:1], xb[:, ko, :],
                                     start=(ko == 0), stop=(ko == KO - 1))
                for ko in range(KO):
                    nc.tensor.matmul(ps2[0:1, :], ones_bf[:, 0:1], sq[:, ko, :],
                                     start=(ko == 0), stop=(ko == KO - 1))
                m_ = pa.tile([1, NC_], F32, tag="row_m", bufs=1, name="m_")
                nc.vector.tensor_scalar_mul(m_[:], ps1[0:1, :], 1.0 / E)
                msq = pa.tile([1, NC_], F32, tag="row_q", bufs=1, name="msq")
                nc.vector.tensor_mul(msq[:], m_[:], m_[:])
                var = pa.tile([1, NC_], F32, tag="row_v", bufs=1, name="var")
                nc.vector.scalar_tensor_tensor(var[:], ps2[0:1, :], 1.0 / E,
                                               msq[:], OP.mult, OP.subtract)
                sqv = pa.tile([1, NC_], F32, tag="row_s", bufs=1, name="sqv")
                nc.scalar.activation(sqv[:], var[:], AF.Sqrt, bias=eps_t[:])
                rstd = pa.tile([1, NC_], F32, tag="row_r", bufs=1, name="rstd")
                nc.vector.reciprocal(rstd[:], sqv[:])
                rbf = pa.tile([1, NC_], BF16, tag="rowsb2", bufs=1, name="rbf")
                nc.vector.tensor_copy(rbf[:], rstd[:])
                nmr = pa.tile([1, NC_], BF16, tag="rowsb1", bufs=1, name="nmr")
                # nmr = -m * rstd
                nc.vector.scalar_tensor_tensor(nmr[:], m_[:], -1.0,
                                               rstd[:], OP.mult, OP.mult)
                rbc = ps()
                nc.tensor.matmul(rbc[:, :], ones_bf[0:1, 0:128], rbf[:],
                                 start=True, stop=True)
                for ko in range(KO):
                    bbc = ps()
                    nc.tensor.matmul(bbc[:, :], gl_sb[i][:, ko, :], nmr[:],
                                     start=True, stop=False)
                    nc.tensor.matmul(bbc[:, :], bl_sb[i][:, ko, :],
                                     ones_bf[0:1, 0:NC_], start=False, stop=True)
                    t0 = pa.tile([128, NC_], F32, tag="tmp", bufs=2, name="t0")
                    nc.vector.scalar_tensor_tensor(t0[:], xT_sb[:, ko, cs],
                                                   gpp[i][:, ko:ko + 1], rbc[:, :],
                                                   OP.mult, OP.mult)
                    nc.vector.tensor_tensor(ln[:, ko, cs], t0[:], bbc[:, :], OP.add)
            return ln

        def project_qk(pb_, lnsrc, w_d, tag, bufs=1):
            """-> [128, 4, T] bf16 : rows = 2 heads x 64, per pair j."""
            w_sb = pb_.tile([128, KO, 512], BF16, tag="wqkv", bufs=2, name="wsb")
            nc.sync.dma_start(out=w_sb[:], in_=w_d.rearrange("(ko p) m -> p ko m", p=128))
            qt = pb_.tile([128, 4, T], BF16, tag=tag, bufs=bufs, name="qt")
            for j in range(4):
                for c in range(CC):
                    p_ = ps()
                    for ko in range(KO):
                        nc.tensor.matmul(p_[:, :], w_sb[:, ko, j * 128:(j + 1) * 128],
                                         lnsrc[:, ko, c * NC_:(c + 1) * NC_],
                                         start=(ko == 0), stop=(ko == KO - 1))
                    nc.vector.tensor_copy(qt[:, j, c * NC_:(c + 1) * NC_], p_[:, :])
            return qt

        def project_v(pb_, src, w_d):
            """-> [128, 8, 8, 65] bf16 : [s_part, s_sub, head, d|ones]."""
            w_sb = pb_.tile([128, KO, 512], BF16, tag="wqkv", bufs=2, name="wsb")
            nc.sync.dma_start(out=w_sb[:], in_=w_d.rearrange("(ko p) m -> p ko m", p=128))
            vv = pb_.tile([128, 8, HL, 65], BF16, tag="vv", name="vv")
            for s in range(8):
                p_ = ps()
                for ko in range(KO):
                    nc.tensor.matmul(p_[:, :], src[:, ko, s * 128:(s + 1) * 128],
                                     w_sb[:, ko, :], start=(ko == 0), stop=(ko == KO - 1))
                nc.scalar.copy(out=vv[:, s, :, 0:64],
                               in_=p_[:, :].rearrange("p (h d) -> p h d", d=64))
                nc.vector.memset(vv[:, s, :, 64:65], 1.0)
            return vv

        def attention(pb_, qt, kt, vv, causal):
            onorm = pb_.tile([128, 4, T], BF16, tag="onorm", name="onorm")
            for c in range(CC):
                for h in range(HL):
                    j, half = h // 2, h % 2
                    pb = 64 * half
                    subs = list(range(4 * (c + 1))) if causal else list(range(8))
                    eb = pb_.tile([128, 8, NC_], BF16, tag="expb", bufs=2, name="eb")
                    for s_ in subs:
                        p_ = ps()
                        nc.tensor.matmul(p_[:, :],
                                         kt[pb:pb + 64, j, s_ * 128:(s_ + 1) * 128],
                                         qt[pb:pb + 64, j, c * NC_:(c + 1) * NC_],
                                         start=True, stop=True)
                        nc.scalar.activation(eb[:, s_, :], p_[:, :], AF.Exp)
                        if causal and s_ >= 4 * c:
                            nc.vector.tensor_mul(eb[:, s_, :], eb[:, s_, :],
                                                 cm_sb[:, s_ - 4 * c, :])
                    av = ps()
                    for i_, s_ in enumerate(subs):
                        nc.tensor.matmul(av[0:65, :], vv[:, s_, h, :], eb[:, s_, :],
                                         start=(i_ == 0), stop=(i_ == len(subs) - 1))
                    rr = pb_.tile([65, NC_], F32, tag="row_rr", bufs=2, name="rr")
                    nc.vector.reciprocal(rr[64:65, :], av[64:65, :])
                    rb = pb_.tile([65, NC_], BF16, tag="row_rrb", bufs=2, name="rb")
                    nc.vector.tensor_copy(rb[64:65, :], rr[64:65, :])
                    bc = ps()
                    nc.tensor.matmul(bc[0:64, :], ones_bf[64:65, 0:64], rb[64:65, :],
                                     start=True, stop=True)
                    bcs = pb_.tile([64, NC_], BF16, tag="bcs", bufs=2, name="bcs")
                    nc.vector.tensor_copy(bcs[:, :], bc[0:64, :])
                    nc.vector.tensor_tensor(onorm[pb:pb + 64, j, c * NC_:(c + 1) * NC_],
                                            av[0:64, :], bcs[:, :], OP.mult)
            return onorm

        def out_proj(pb_, onorm, wo_d, ark, bias_pp):
            wo_sb = pb_.tile([128, 4, E], BF16, tag="wo", name="wo_sb")
            nc.sync.dma_start(out=wo_sb[:], in_=wo_d.rearrange("(ks p) e -> p ks e", p=128))
            for c in range(CC):
                for m in range(KO):
                    p_ = ps()
                    for ks in range(4):
                        nc.tensor.matmul(p_[:, :], wo_sb[:, ks, m * 128:(m + 1) * 128],
                                         onorm[:, ks, c * NC_:(c + 1) * NC_],
                                         start=(ks == 0), stop=(ks == 3))
                    st = pa.tile([128, NC_], F32, tag="arst", bufs=4, name="st")
                    nc.vector.tensor_scalar_add(st[:, :], p_[:, :], bias_pp[:, m:m + 1])
                    nc.sync.dma_start(out=ark[c][0][m * 128:(m + 1) * 128, :], in_=st[:])
                allreduce_c(ark, c)

        def allreduce_c(ark, c):
            a_in, a_out = ark[c]
            if SKIP_CC:
                nc.sync.dma_start(out=a_out[:], in_=a_in[:])
            else:
                nc.gpsimd.collective_compute(
                    "AllReduce", OP.add, replica_groups=PAIRS,
                    ins=[a_in[:]], outs=[a_out[:]])
            nc.gpsimd.dma_start(
                out=xT_sb[:, :, c * NC_:(c + 1) * NC_],
                in_=a_out.rearrange("(ko p) t -> p ko t", p=128),
                accum_op=OP.add)



        with tc.tile_pool(name=f"B{ibody}", bufs=1) as pb_:
            # ---- self attention ----
            ln1 = layer_norm(0)
            qt = project_qk(pb_, ln1, W["wq"], "qt")
            kt = project_qk(pb_, ln1, W["wk"], "kt")
            vv = project_v(pb_, ln1, W["wv"])
            on1 = attention(pb_, qt, kt, vv, causal=True)
            out_proj(pb_, on1, W["wo"], ar[1], bo2pp)
            # cross K/V from raw ca — independent of AR1, fills the gap
            ktc = project_qk(pb_, ca_sb, W["wkc"], "kt")
            vvc = project_v(pb_, ca_sb, W["wvc"])

            # ---- cross attention ----
            ln2 = layer_norm(1)
            qtc = project_qk(pb_, ln2, W["wqc"], "qt")
            on2 = attention(pb_, qtc, ktc, vvc, causal=False)
            out_proj(pb_, on2, W["woc"], ar[2], bo2cpp)

        # ---- FFN ----
        ln3 = layer_norm(2)
        with tc.tile_pool(name=f"C{ibody}", bufs=1) as pc:
            ht = pc.tile([128, FF // 128, T], BF16, tag="ht", name="ht")
            for m in range(FF // 128):
                w1m = pc.tile([128, KO, 128], BF16, tag="w1m", bufs=6, name="w1m")
                nc.sync.dma_start(
                    out=w1m[:],
                    in_=W["w1"][m].rearrange("p (ko f) -> p ko f", f=128))
                for c in range(CC):
                    p_ = ps()
                    for ko in range(KO):
                        nc.tensor.matmul(p_[:, :], w1m[:, ko, :],
                                         ln3[:, ko, c * NC_:(c + 1) * NC_],
                                         start=(ko == 0), stop=(ko == KO - 1))
                    nc.scalar.activation(ht[:, m, c * NC_:(c + 1) * NC_], p_[:, :],
                                         AF.Relu, bias=b1pp[:, m:m + 1])
            w2m_t = [None] * KO
            for m in range(KO):
                w2m = pc.tile([128, FF // 128, 128], BF16, tag="w2m", bufs=8, name="w2m")
                nc.sync.dma_start(
                    out=w2m[:],
                    in_=W["w2"].rearrange("(ks p) e -> p ks e", p=128)[:, :, m * 128:(m + 1) * 128])
                w2m_t[m] = w2m
            for c in range(CC):
                for m in range(KO):
                    p_ = ps()
                    for ks in range(FF // 128):
                        nc.tensor.matmul(p_[:, :], w2m_t[m][:, ks, :],
                                         ht[:, ks, c * NC_:(c + 1) * NC_],
                                         start=(ks == 0), stop=(ks == FF // 128 - 1))
                    st = pa.tile([128, NC_], F32, tag="arst", bufs=4, name="st")
                    nc.vector.tensor_scalar_add(st[:, :], p_[:, :], b22pp[:, m:m + 1])
                    nc.sync.dma_start(out=ar[3][c][0][m * 128:(m + 1) * 128, :], in_=st[:])
                allreduce_c(ar[3], c)
                for ko in range(KO):
                    nc.sync.dma_start(
                        out=out_xT[ko * 128:(ko + 1) * 128, c * NC_:(c + 1) * NC_],
                        in_=xT_sb[:, ko, c * NC_:(c + 1) * NC_])


# ------------------------------------------------------------------ v2 (comm-free, pipelined)

def build2(nbody=1):
    """Comm-free sharding (core = (batch b, T-half h)) with:
    - LN stats broadcast across partitions (all-ones stationary matmul) so all
      row math runs 128-wide; rstd via sqrt + reciprocal_approx_fast.
    - Attention: software-pipelined QK/exp/AV across heads; per-head softmax
      denominators ride as a 65th V row, normalization deferred and batched
      (one reciprocal for all 16 heads).
    - Fully-masked key blocks handled by zeroing V (+ its ones row) per core;
      only the 4 diagonal blocks get a post-exp multiplicative mask.
    """
    nc = bacc.Bacc(num_devices=8)

    def P(name, shape, dt):
        return nc.declare_dram_parameter(name, shape, dt, isOutput=False)

    # all weight/activation layouts are host-pretransposed so every DMA is
    # contiguous per partition (descriptor-bound strided gathers killed ~180us
    # per DMA engine in the naive [E, .] layouts)
    xTb = P("xTb", [E, T], BF16)        # permuted x^T, bf16 (LN1 / self K,V)
    xTo = P("xTo", [128, KO * NC_], F32)   # own-half x^T  [p, ko*t]
    caT = P("caT", [128, KO * S], BF16)    # ca^T           [p, ko*t]
    wq, wk, wv = P("wq", [2, 128, KO * 512], BF16), P("wk", [2, 128, KO * 512], BF16), P("wv", [2, 128, KO * 512], BF16)
    wqc, wkc, wvc = P("wqc", [2, 128, KO * 512], BF16), P("wkc", [2, 128, KO * 512], BF16), P("wvc", [2, 128, KO * 512], BF16)
    wo, woc = P("wo", [KO, 128, E], BF16), P("woc", [KO, 128, E], BF16)
    w1 = P("w1", [4 * E // 128, 128, KO * 128], BF16)
    w2 = P("w2", [KO, 128, (4 * E // 128) * 128], BF16)
    gpp_d = [P(f"g{i}", [E], F32) for i in (1, 2, 3)]
    bpp_d = [P(f"be{i}", [E], F32) for i in (1, 2, 3)]
    bo_, boc_, b2_ = P("bo", [E], F32), P("boc", [E], F32), P("b2", [E], F32)
    b1r = P("b1r", [4 * E], F32)
    smask = P("smask", [128, 4, NC_], BF16)   # diagonal causal blocks
    vmsk = P("vmsk", [128, 8], F32)           # per-key-block V mask
    seld = P("seld", [16, 8, 128], BF16)      # head-pair broadcast selector
    out_xT = nc.declare_dram_parameter("out_xT", [E, NC_], F32, isOutput=True)

    with tile.TileContext(nc) as tc:
        with tc.tile_pool(name="persist", bufs=1) as pp:
            xTb_sb = pp.tile([128, KO, T], BF16, tag="xTb")
            for ko in range(KO):
                nc.sync.dma_start(out=xTb_sb[:, ko, :], in_=xTb[ko * 128:(ko + 1) * 128, :])
            xTo_sb = pp.tile([128, KO, NC_], F32, tag="xTo")
            nc.sync.dma_start(out=xTo_sb[:], in_=xTo.rearrange("p (ko t) -> p ko t", t=NC_))
            ca_sb = pp.tile([128, KO, S], BF16, tag="ca")
            nc.gpsimd.dma_start(out=ca_sb[:], in_=caT.rearrange("p (ko t) -> p ko t", t=S))
            sm_sb = pp.tile([128, 4, NC_], BF16, tag="sm")
            nc.scalar.dma_start(out=sm_sb[:], in_=smask[:])
            vm_sb = pp.tile([128, 8], F32, tag="vm")
            nc.scalar.dma_start(out=vm_sb[:], in_=vmsk[:])
            sel_sb = pp.tile([16, 8, 128], BF16, tag="sel")
            nc.scalar.dma_start(out=sel_sb[:], in_=seld[:])
            ones_bf = pp.tile([128, 512], BF16, tag="ones")
            nc.vector.memset(ones_bf[:], 1.0)
            gpp, bepp = [], []
            for i in range(3):
                t2 = pp.tile([128, KO], F32, tag=f"gpp{i}")
                with nc.allow_non_contiguous_dma(reason="tiny LN vector"):
                    nc.scalar.dma_start(out=t2[:], in_=gpp_d[i].rearrange("(ko p) -> p ko", p=128))
                gpp.append(t2)
                t3 = pp.tile([128, KO], F32, tag=f"bepp{i}")
                with nc.allow_non_contiguous_dma(reason="tiny LN vector"):
                    nc.scalar.dma_start(out=t3[:], in_=bpp_d[i].rearrange("(ko p) -> p ko", p=128))
                bepp.append(t3)
            bpp = []
            for nm, d in (("bo", bo_), ("boc", boc_), ("b2", b2_)):
                t_ = pp.tile([128, KO], F32, tag=nm)
                with nc.allow_non_contiguous_dma(reason="tiny bias vector"):
                    nc.scalar.dma_start(out=t_[:], in_=d.rearrange("(ko p) -> p ko", p=128))
                bpp.append(t_)
            eps_t = pp.tile([128, 1], F32, tag="eps")
            nc.vector.memset(eps_t[:], 1e-5)
            b1pp = pp.tile([128, 4 * E // 128], F32, tag="b1")
            with nc.allow_non_contiguous_dma(reason="tiny bias vector"):
                nc.scalar.dma_start(out=b1pp[:], in_=b1r.rearrange("(m p) -> p m", p=128))

            for ibody in range(nbody):
                _body2(nc, tc, ibody, xTb_sb, xTo_sb, ca_sb, sm_sb, vm_sb, sel_sb,
                       ones_bf, gpp, bepp, bpp, b1pp, eps_t,
                       dict(wq=wq, wk=wk, wv=wv, wqc=wqc, wkc=wkc, wvc=wvc,
                            wo=wo, woc=woc, w1=w1, w2=w2, xTo=xTo),
                       out_xT)
    nc.finalize()
    return nc


def _body2(nc, tc, ibody, xTb_sb, xTo_sb, ca_sb, sm_sb, vm_sb, sel_sb, ones_bf,
           gpp, bepp, bpp, b1pp, eps_t, W, out_xT):
    bopp, bocpp, b2pp = bpp

    if ibody > 0:
        nc.sync.dma_start(out=xTo_sb[:],
                          in_=W["xTo"].rearrange("p (ko t) -> p ko t", t=NC_))

    with tc.tile_pool(name=f"A{ibody}", bufs=1) as pa, \
         tc.tile_pool(name=f"ps{ibody}", bufs=8, space="PSUM") as pspool:

        def ps():
            return pspool.tile([128, NC_], F32, tag="ps", name="ps")

        def ln_v2(i, stat_tile, stat_off, mat_srcs, ln, nchunks):
            """stat_tile: bf16 tile [128, KO, >=stat_off+512*nchunks];
            mat_srcs[c][ko]: AP [128,512] (bf16 or f32); writes ln[:, ko, :]."""
            for c in range(nchunks):
                so = stat_off + c * NC_
                msrc = mat_srcs[c]
                sq = pa.tile([128, KO, NC_], BF16, tag="sq", bufs=1, name="sq")
                for ko in range(KO):
                    nc.vector.tensor_mul(sq[:, ko, :], stat_tile[:, ko, so:so + NC_],
                                         stat_tile[:, ko, so:so + NC_])
                ps1, ps2 = ps(), ps()
                for ko in range(KO):
                    nc.tensor.matmul(ps1[:, :], ones_bf[:, 0:128],
                                     stat_tile[:, ko, so:so + NC_],
                                     start=(ko == 0), stop=(ko == KO - 1))
                for ko in range(KO):
                    nc.tensor.matmul(ps2[:, :], ones_bf[:, 0:128], sq[:, ko, :],
                                     start=(ko == 0), stop=(ko == KO - 1))
                mbc = pa.tile([128, NC_], F32, tag="mbc", bufs=1, name="mbc")
                nc.vector.tensor_scalar_mul(mbc[:], ps1[:, :], 1.0 / E)
                msq = pa.tile([128, NC_], F32, tag="msq", bufs=1, name="msq")
                nc.vector.tensor_mul(msq[:], mbc[:], mbc[:])
                var = pa.tile([128, NC_], F32, tag="var", bufs=1, name="var")
                nc.vector.scalar_tensor_tensor(var[:], ps2[:, :], 1.0 / E,
                                               msq[:], OP.mult, OP.subtract)
                sqv = pa.tile([128, NC_], F32, tag="sqv", bufs=1, name="sqv")
                nc.scalar.activation(sqv[:], var[:], AF.Sqrt, bias=eps_t[:])
                rstd = pa.tile([128, NC_], F32, tag="rstd", bufs=1, name="rstd")
                nc.vector.reciprocal(rstd[:], sqv[:])
                nmr = pa.tile([128, NC_], F32, tag="nmr", bufs=1, name="nmr")
                nc.vector.scalar_tensor_tensor(nmr[:], mbc[:], -1.0,
                                               rstd[:], OP.mult, OP.mult)
                cs = slice(c * NC_, (c + 1) * NC_)
                for ko in range(KO):
                    t_ = pa.tile([128, NC_], F32, tag="lt", bufs=2, name="lt")
                    nc.vector.scalar_tensor_tensor(t_[:], msrc[ko],
                                                   gpp[i][:, ko:ko + 1], rstd[:],
                                                   OP.mult, OP.mult)
                    u_ = pa.tile([128, NC_], F32, tag="lu", bufs=2, name="lu")
                    nc.vector.scalar_tensor_tensor(u_[:], nmr[:],
                                                   gpp[i][:, ko:ko + 1], t_[:],
                                                   OP.mult, OP.add)
                    nc.vector.tensor_scalar_add(ln[:, ko, cs], u_[:],
                                                bepp[i][:, ko:ko + 1])

        def proj16(pb_, lnsrc, w_d, out_t, ncols):
            """16-head projection -> out_t [128, 8, ncols] bf16."""
            for jh in range(2):
                w_sb = pb_.tile([128, KO, 512], BF16, tag="wqkv", bufs=2, name="wsb")
                nc.sync.dma_start(
                    out=w_sb[:],
                    in_=w_d[jh].rearrange("p (ko m) -> p ko m", m=512))
                for jj in range(4):
                    j = jh * 4 + jj
                    for c in range(ncols // NC_):
                        p_ = ps()
                        for ko in range(KO):
                            nc.tensor.matmul(p_[:, :], w_sb[:, ko, jj * 128:(jj + 1) * 128],
                                             lnsrc[:, ko, c * NC_:(c + 1) * NC_],
                                             start=(ko == 0), stop=(ko == KO - 1))
                        nc.vector.tensor_copy(out_t[:, j, c * NC_:(c + 1) * NC_], p_[:, :])

        def proj_v(pb_, src, w_d, vv, use_vmask):
            for jh in range(2):
                w_sb = pb_.tile([128, KO, 512], BF16, tag="wqkv", bufs=2, name="wsb")
                nc.sync.dma_start(
                    out=w_sb[:],
                    in_=w_d[jh].rearrange("p (ko m) -> p ko m", m=512))
                for s in range(8):
                    p_ = ps()
                    for ko in range(KO):
                        nc.tensor.matmul(p_[:, :], src[:, ko, s * 128:(s + 1) * 128],
                                         w_sb[:, ko, :], start=(ko == 0), stop=(ko == KO - 1))
                    nc.scalar.copy(out=vv[:, s, jh * 8:(jh + 1) * 8, 0:64],
                                   in_=p_[:, :].rearrange("p (h d) -> p h d", d=64))
            for s in range(8):
                nc.vector.memset(vv[:, s, :, 64:65], 1.0)
                if use_vmask:
                    nc.vector.tensor_scalar_mul(vv[:, s, :, :],
                                                vv[:, s, :, :], vm_sb[:, s:s + 1])

        def attention2(pb_, qt, kt, vv, avo, masked):
            """Pipelined attention: per-head QK->exp->AV with the next head's
            QK emitted before this head's AV.  Softmax denominators ride as the
            65th V row, are gathered (lagged, via tiny K=1 matmuls) onto rows
            0..15 of one PSUM tile, and a single batched reciprocal serves all
            16 heads before the broadcast/normalize pass."""
            ebs, rowbufs = {}, {}
            dn16 = pb_.tile([16, NC_], F32, tag="dn16", bufs=1, name="dn16")

            def emit_qk(h):
                j, half = h // 2, h % 2
                pb = 64 * half
                eb = pb_.tile([128, 8, NC_], BF16, tag="eb", bufs=3, name="eb")
                ebs[h] = eb
                for s in range(8):
                    p_ = ps()
                    nc.tensor.matmul(p_[:, :],
                                     kt[pb:pb + 64, j, s * 128:(s + 1) * 128],
                                     qt[pb:pb + 64, j, :], start=True, stop=True)
                    nc.scalar.activation(eb[:, s, :], p_[:, :], AF.Exp)
                    if masked and s < 4:
                        nc.vector.tensor_mul(eb[:, s, :], eb[:, s, :], sm_sb[:, s, :])

            def emit_av(h):
                j, half = h // 2, h % 2
                pb = 64 * half
                eb = ebs.pop(h)
                av = ps()
                for s in range(8):
                    nc.tensor.matmul(av[0:65, :], vv[:, s, h, :], eb[:, s, :],
                                     start=(s == 0), stop=(s == 7))
                nc.vector.tensor_copy(avo[pb:pb + 64, j, :], av[0:64, :])
                rowb = pb_.tile([65, NC_], F32, tag="rowb", bufs=2, name="rowb")
                nc.vector.tensor_copy(rowb[64:65, :], av[64:65, :])
                rowbufs[h] = rowb

            def emit_gather(h):
                rowb = rowbufs.pop(h)
                nc.scalar.dma_start(out=dn16[h:h + 1, :], in_=rowb[64:65, :])

            def qkav():
                emit_qk(0)
                for h in range(H):
                    if h + 1 < H:
                        emit_qk(h + 1)
                    emit_av(h)
                    if h > 0:
                        emit_gather(h - 1)
                emit_gather(H - 1)

            def norm_tail():
                nc.vector.reciprocal(dn16[:, :], dn16[:, :])
                rb = pb_.tile([16, NC_], BF16, tag="rb", bufs=1, name="rb")
                nc.vector.tensor_copy(rb[:, :], dn16[:, :])
                for j in range(8):
                    bc = ps()
                    nc.tensor.matmul(bc[:, :], sel_sb[:, j, :], rb[:, :],
                                     start=True, stop=True)
                    nc.vector.tensor_tensor(avo[:, j, :], avo[:, j, :],
                                            bc[:, :], OP.mult)
            return qkav, norm_tail

        def out_proj2(pb_, onorm, wo_d, bias_pp, per_m=None):
            for m in range(KO):
                wom = pb_.tile([128, KO, 128], BF16, tag="wom", bufs=2, name="wom")
                nc.sync.dma_start(
                    out=wom[:],
                    in_=wo_d[m].rearrange("p (ks e) -> p ks e", e=128))
                p_ = ps()
                for ks in range(KO):
                    nc.tensor.matmul(p_[:, :], wom[:, ks, :], onorm[:, ks, :],
                                     start=(ks == 0), stop=(ks == KO - 1))
                nc.vector.scalar_tensor_tensor(xTo_sb[:, m, :], p_[:, :],
                                               bias_pp[:, m:m + 1], xTo_sb[:, m, :],
                                               OP.add, OP.add)
                if per_m is not None:
                    per_m(m)

        with tc.tile_pool(name=f"B{ibody}", bufs=1) as pb_:
            # ---- self attention ----
            ln1 = pa.tile([128, KO, T], BF16, tag="ln1", name="ln1")
            ln_v2(0, xTb_sb, 0,
                  [[xTb_sb[:, ko, 0:NC_] for ko in range(KO)],
                   [xTb_sb[:, ko, NC_:T] for ko in range(KO)]],
                  ln1, 2)
            qt = pb_.tile([128, 8, NC_], BF16, tag="qt", bufs=1, name="qt")
            proj16(pb_, ln1, W["wq"], qt, NC_)
            kt = pb_.tile([128, 8, T], BF16, tag="kt", bufs=1, name="kt")
            proj16(pb_, ln1, W["wk"], kt, T)
            vv = pb_.tile([128, 8, H, 65], BF16, tag="vv", bufs=1, name="vv")
            proj_v(pb_, ln1, W["wv"], vv, use_vmask=True)
            avo = pb_.tile([128, 8, NC_], BF16, tag="avo", bufs=1, name="avo")
            qkav, norm_tail = attention2(pb_, qt, kt, vv, avo, masked=True)
            qkav()
            # cross K projection is independent -> fills the softmax-recip tail
            ktc = pb_.tile([128, 8, T], BF16, tag="kt", bufs=1, name="ktc")
            proj16(pb_, ca_sb, W["wkc"], ktc, T)
            norm_tail()
            ln2 = pa.tile([128, KO, T], BF16, tag="ln1", name="ln2")
            xb2 = pa.tile([128, KO, NC_], BF16, tag="xb2", bufs=1, name="xb2")
            out_proj2(pb_, avo, W["wo"], bopp,
                      per_m=lambda m: nc.scalar.copy(out=xb2[:, m, :],
                                                     in_=xTo_sb[:, m, :]))

            # ---- cross attention ----
            ln_v2(1, xb2, 0, [[xTo_sb[:, ko, :] for ko in range(KO)]], ln2, 1)
            vvc = pb_.tile([128, 8, H, 65], BF16, tag="vv", bufs=1, name="vvc")
            proj_v(pb_, ca_sb, W["wvc"], vvc, use_vmask=False)
            qtc = pb_.tile([128, 8, NC_], BF16, tag="qt", bufs=1, name="qtc")
            proj16(pb_, ln2, W["wqc"], qtc, NC_)
            avoc = pb_.tile([128, 8, NC_], BF16, tag="avo", bufs=1, name="avoc")
            qkavc, norm_tailc = attention2(pb_, qtc, ktc, vvc, avoc, masked=False)
            qkavc()
            norm_tailc()
            xb3 = pa.tile([128, KO, NC_], BF16, tag="xb2", bufs=1, name="xb3")
            out_proj2(pb_, avoc, W["woc"], bocpp,
                      per_m=lambda m: nc.scalar.copy(out=xb3[:, m, :],
                                                     in_=xTo_sb[:, m, :]))

        # ---- FFN ----
        with tc.tile_pool(name=f"C{ibody}", bufs=1) as pc:
            ln3 = pa.tile([128, KO, T], BF16, tag="ln1", name="ln3")
            ln_v2(2, xb3, 0, [[xTo_sb[:, ko, :] for ko in range(KO)]], ln3, 1)
            FH = 4 * E // 128
            ht = pc.tile([128, FH, NC_], BF16, tag="ht", name="ht")
            for m in range(FH):
                w1m = pc.tile([128, KO, 128], BF16, tag="w1m", bufs=6, name="w1m")
                nc.sync.dma_start(
                    out=w1m[:],
                    in_=W["w1"][m].rearrange("p (ko f) -> p ko f", f=128))
                p_ = ps()
                for ko in range(KO):
                    nc.tensor.matmul(p_[:, :], w1m[:, ko, :], ln3[:, ko, 0:NC_],
                                     start=(ko == 0), stop=(ko == KO - 1))
                nc.scalar.activation(ht[:, m, :], p_[:, :], AF.Relu,
                                     bias=b1pp[:, m:m + 1])
            for m in range(KO):
                w2m = pc.tile([128, FH, 128], BF16, tag="w2m", bufs=2, name="w2m")
                nc.sync.dma_start(
                    out=w2m[:],
                    in_=W["w2"][m].rearrange("p (ks e) -> p ks e", e=128))
                p_ = ps()
                for ks in range(FH):
                    nc.tensor.matmul(p_[:, :], w2m[:, ks, :], ht[:, ks, :],
                                     start=(ks == 0), stop=(ks == FH - 1))
                nc.vector.scalar_tensor_tensor(xTo_sb[:, m, :], p_[:, :],
                                               b2pp[:, m:m + 1], xTo_sb[:, m, :],
                                               OP.add, OP.add)
        for ko in range(KO):
            nc.sync.dma_start(out=out_xT[ko * 128:(ko + 1) * 128, :],
                              in_=xTo_sb[:, ko, :])


def _make_core_inputs2(c, inp):
    bf = ml_dtypes.bfloat16
    b, h = divmod(c, 2)
    sc = float(E) ** -0.5
    own = slice(512 * h, 512 * h + 512)
    oth = slice(512 * (1 - h), 512 * (1 - h) + 512)

    def stack_heads(w):  # [16, E, D] -> [E, 1024]
        return np.ascontiguousarray(np.transpose(w, (1, 0, 2)).reshape(E, E))

    def qkv_layout(w):  # [E, 1024] -> [2(half), 128, KO*512], contiguous DMA
        a = w.reshape(8, 128, 1024).transpose(1, 0, 2)      # [p, ko, m]
        return np.ascontiguousarray(np.stack(
            [a[:, :, :512].reshape(128, 8 * 512),
             a[:, :, 512:].reshape(128, 8 * 512)])).astype(bf)

    def mtile_layout(w, km, cm):  # [km*128, cm*128] -> [cm, 128, km*128]
        a = w.reshape(km, 128, cm, 128).transpose(2, 1, 0, 3)
        return np.ascontiguousarray(a.reshape(cm, 128, km * 128)).astype(bf)

    def pkt_layout(xT):  # [E, ncols] -> [128, KO*ncols]
        n = xT.shape[1]
        return np.ascontiguousarray(
            xT.reshape(8, 128, n).transpose(1, 0, 2).reshape(128, 8 * n))

    xt = np.asarray(inp["x"][b], np.float32)           # [T, E]
    xperm = np.concatenate([xt[own], xt[oth]], axis=0)  # keys permuted: own first

    # diagonal causal mask blocks: key local pos (128j + p) <= query local pos f
    p, f = np.arange(128)[:, None, None], np.arange(512)[None, None, :]
    jj = np.arange(4)[None, :, None]
    smask = (128 * jj + p <= f).astype(bf)

    # V-block mask: subs 0..3 = own half (visible, diag-masked); 4..7 = other
    # half: visible iff this core owns the second half (h == 1)
    vmsk = np.ones((128, 8), np.float32)
    vmsk[:, 4:] = 1.0 if h == 1 else 0.0

    # selector: sel[k, j, m] = 1 iff head (2j + m//64) == k  (softmax recip
    # broadcast: one K=16 matmul expands rows of rb to a [128,512] tile)
    kk = np.arange(16)[:, None, None]
    jj2 = np.arange(8)[None, :, None]
    mm = np.arange(128)[None, None, :]
    sel = (kk == 2 * jj2 + mm // 64).astype(bf)

    return {
        "xTb": np.ascontiguousarray(xperm.T).astype(bf),
        "xTo": pkt_layout(xt[own].T).astype(np.float32),
        "caT": pkt_layout(np.asarray(inp["ca"][b]).T).astype(bf),
        "wq": qkv_layout(stack_heads(inp["Wq_s"]) * sc),
        "wk": qkv_layout(stack_heads(inp["Wk_s"])),
        "wv": qkv_layout(stack_heads(inp["Wv_s"])),
        "wqc": qkv_layout(stack_heads(inp["Wq_c"]) * sc),
        "wkc": qkv_layout(stack_heads(inp["Wk_c"])),
        "wvc": qkv_layout(stack_heads(inp["Wv_c"])),
        "wo": mtile_layout(np.asarray(inp["Wo_s"], np.float32), 8, 8),
        "woc": mtile_layout(np.asarray(inp["Wo_c"], np.float32), 8, 8),
        "w1": mtile_layout(np.asarray(inp["W1"], np.float32), 8, 32),
        "w2": mtile_layout(np.asarray(inp["W2"], np.float32), 32, 8),
        "g1": np.asarray(inp["ln1_g"], np.float32),
        "g2": np.asarray(inp["ln2_g"], np.float32),
        "g3": np.asarray(inp["ln3_g"], np.float32),
        "be1": np.asarray(inp["ln1_b"], np.float32),
        "be2": np.asarray(inp["ln2_b"], np.float32),
        "be3": np.asarray(inp["ln3_b"], np.float32),
        "bo": np.asarray(inp["bo_s"], np.float32),
        "boc": np.asarray(inp["bo_c"], np.float32),
        "b2": np.asarray(inp["b2"], np.float32),
        "b1r": np.asarray(inp["b1"], np.float32),
        "smask": smask,
        "vmsk": vmsk,
        "seld": sel,
    }


# ------------------------------------------------------------------ host side

_CACHE = {}


COMM_FREE = True
VARIANT = "v2"   # "tp2" | "nocc" | "v2"


def _variant():
    builders = {"tp2": (build, _make_core_inputs),
                "nocc": (build_nocc, _make_core_inputs_nocc),
                "v2": (build2, _make_core_inputs2)}
    return builders[VARIANT]


def _get_runner(nbody=1):
    key = (nbody, VARIANT)
    if key in _CACHE:
        return _CACHE[key]
    import jax
    from jax.sharding import Mesh, PartitionSpec
    from jax.experimental.shard_map import shard_map
    from concourse.bass2jax import (_bass_exec_p, install_neuronx_cc_hook,
                                    partition_id_tensor)

    nc = _variant()[0](nbody)
    install_neuronx_cc_hook()
    pn = nc.partition_id_tensor.name if nc.partition_id_tensor else None
    in_names, out_names, out_avals = [], [], []
    for alloc in nc.m.functions[0].allocations:
        if not isinstance(alloc, mybir.MemoryLocationSet):
            continue
        name = alloc.memorylocations[0].name
        if alloc.kind == "ExternalInput":
            if name != pn:
                in_names.append(name)
        elif alloc.kind == "ExternalOutput":
            out_names.append(name)
            out_avals.append(jax.core.ShapedArray(
                tuple(alloc.tensor_shape), mybir.dt.np(alloc.dtype)))
    n_params = len(in_names)
    all_in = in_names + out_names + ([pn] if pn else [])

    def _jbody(*args):
        ops = list(args)
        if pn:
            ops.append(partition_id_tensor())
        return tuple(_bass_exec_p.bind(
            *ops, out_avals=tuple(out_avals), in_names=tuple(all_in),
            out_names=tuple(out_names), lowering_input_output_aliases=(),
            sim_require_finite=True, sim_require_nnan=True, nc=nc))

    devices = jax.devices()[:8]
    mesh = Mesh(np.asarray(devices), ("core",))
    spec = (PartitionSpec("core"),)
    fn = jax.jit(shard_map(_jbody, mesh=mesh,
                           in_specs=spec * (n_params + len(out_names)),
                           out_specs=spec * len(out_names), check_rep=False),
                 keep_unused=True)
    _CACHE[key] = (fn, in_names, out_names, out_avals)
    return _CACHE[key]


def _make_core_inputs(c, inp):
    bf = ml_dtypes.bfloat16
    b, r = divmod(c, 2)
    hs = slice(8 * r, 8 * r + 8)
    sc = float(E) ** -0.5

    def stack_heads(w):  # [8, E, D] -> [E, 512]
        return np.ascontiguousarray(np.transpose(w, (1, 0, 2)).reshape(E, 512))

    p, f = np.arange(128)[:, None, None], np.arange(512)[None, None, :]
    jj = np.arange(4)[None, :, None]
    cmask = (f >= 128 * jj + p).astype(bf)

    return {
        "xT": np.ascontiguousarray(inp["x"][b].T).astype(np.float32),
        "caT": np.ascontiguousarray(inp["ca"][b].T).astype(bf),
        "wq": (stack_heads(inp["Wq_s"][hs]) * sc).astype(bf),
        "wk": stack_heads(inp["Wk_s"][hs]).astype(bf),
        "wv": stack_heads(inp["Wv_s"][hs]).astype(bf),
        "wqc": (stack_heads(inp["Wq_c"][hs]) * sc).astype(bf),
        "wkc": stack_heads(inp["Wk_c"][hs]).astype(bf),
        "wvc": stack_heads(inp["Wv_c"][hs]).astype(bf),
        "wo": np.ascontiguousarray(inp["Wo_s"][512 * r:512 * (r + 1), :]).astype(bf),
        "woc": np.ascontiguousarray(inp["Wo_c"][512 * r:512 * (r + 1), :]).astype(bf),
        "w1": np.ascontiguousarray(inp["W1"][:, FF * r:FF * (r + 1)]).astype(bf),
        "w2": np.ascontiguousarray(inp["W2"][FF * r:FF * (r + 1), :]).astype(bf),
        "gb1": np.stack([inp["ln1_g"], inp["ln1_b"]]).astype(bf),
        "gb2": np.stack([inp["ln2_g"], inp["ln2_b"]]).astype(bf),
        "gb3": np.stack([inp["ln3_g"], inp["ln3_b"]]).astype(bf),
        "g1": np.asarray(inp["ln1_g"], np.float32),
        "g2": np.asarray(inp["ln2_g"], np.float32),
        "g3": np.asarray(inp["ln3_g"], np.float32),
        "bo2": np.asarray(inp["bo_s"], np.float32) * 0.5,
        "bo2c": np.asarray(inp["bo_c"], np.float32) * 0.5,
        "b22": np.asarray(inp["b2"], np.float32) * 0.5,
        "b1r": np.asarray(inp["b1"][FF * r:FF * (r + 1)], np.float32),
        "cmask": cmask,
    }


def _run(nbody, in_maps, dev_inputs=None, dev_zeros=None, download=True):
    import jax
    fn, in_names, out_names, out_avals = _get_runner(nbody)
    if dev_inputs is None:
        concat = [np.concatenate([np.asarray(in_maps[c][n]) for c in range(8)], axis=0)
                  for n in in_names]
        dev_inputs = [jax.device_put(a) for a in concat]
    if dev_zeros is None:
        dev_zeros = [jax.device_put(np.zeros((8 * a.shape[0], *a.shape[1:]), a.dtype))
                     for a in out_avals]
    outs = fn(*dev_inputs, *dev_zeros)
    for o in outs:
        o.block_until_ready()
    if not download:
        return None, (dev_inputs, dev_zeros)
    res = []
    for c in range(8):
        res.append({n: np.asarray(outs[i]).reshape(8, *out_avals[i].shape)[c]
                    for i, n in enumerate(out_names)})
    return res, (dev_inputs, dev_zeros)


def kernel(**inputs):
    inp = {k: np.asarray(v) for k, v in inputs.items()}
    mk = _variant()[1]
    in_maps = [mk(c, inp) for c in range(8)]
    res, _ = _run(1, in_maps)
    if VARIANT in ("nocc", "v2"):
        out = np.stack([
            np.concatenate([res[2 * b]["out_xT"], res[2 * b + 1]["out_xT"]],
                           axis=1).T
            for b in range(B)]).astype(np.float32)
    else:
        out = np.stack([res[2 * b]["out_xT"].T for b in range(B)]).astype(np.float32)
    return out


# ---------------------------------------------------------------- comm-free

def build_nocc(nbody=1):
    """Communication-free sharding: core = (batch b, T-half h).  Each core
    computes its 512 query tokens for ALL 16 heads and the full FFN, with
    K/V duplicated across the pair.  Self-attn keys are permuted so the own
    half always sits at key positions 0..511 (the per-core causal mask input
    encodes the permutation) — keeps the SPMD program identical on all cores.
    """
    nc = bacc.Bacc(num_devices=8)

    def P(name, shape, dt):
        return nc.declare_dram_parameter(name, shape, dt, isOutput=False)

    # all weight/activation layouts are host-pretransposed so every DMA is
    # contiguous per partition (descriptor-bound strided gathers killed ~180us
    # per DMA engine in the naive [E, .] layouts)
    xTb = P("xTb", [E, T], BF16)        # permuted x^T, bf16 (LN1 / self K,V)
    xTo = P("xTo", [128, KO * NC_], F32)   # own-half x^T  [p, ko*t]
    caT = P("caT", [128, KO * S], BF16)    # ca^T           [p, ko*t]
    wq, wk, wv = P("wq", [2, 128, KO * 512], BF16), P("wk", [2, 128, KO * 512], BF16), P("wv", [2, 128, KO * 512], BF16)
    wqc, wkc, wvc = P("wqc", [2, 128, KO * 512], BF16), P("wkc", [2, 128, KO * 512], BF16), P("wvc", [2, 128, KO * 512], BF16)
    wo, woc = P("wo", [KO, 128, E], BF16), P("woc", [KO, 128, E], BF16)
    w1 = P("w1", [4 * E // 128, 128, KO * 128], BF16)
    w2 = P("w2", [KO, 128, (4 * E // 128) * 128], BF16)
    gb = [P(f"gb{i}", [2, E], BF16) for i in (1, 2, 3)]
    gpp_d = [P(f"g{i}", [E], F32) for i in (1, 2, 3)]
    bo_, boc_, b2_ = P("bo", [E], F32), P("boc", [E], F32), P("b2", [E], F32)
    b1r = P("b1r", [4 * E], F32)
    smask = P("smask", [128, 8, NC_], BF16)
    out_xT = nc.declare_dram_parameter("out_xT", [E, NC_], F32, isOutput=True)

    with tile.TileContext(nc) as tc:
        with tc.tile_pool(name="persist", bufs=1) as pp:
            xTb_sb = pp.tile([128, KO, T], BF16, tag="xTb")
            for ko in range(KO):
                nc.sync.dma_start(out=xTb_sb[:, ko, :], in_=xTb[ko * 128:(ko + 1) * 128, :])
            xTo_sb = pp.tile([128, KO, NC_], F32, tag="xTo")
            nc.sync.dma_start(out=xTo_sb[:], in_=xTo.rearrange("(ko p) t -> p ko t", p=128))
            ca_sb = pp.tile([128, KO, S], BF16, tag="ca")
            nc.sync.dma_start(out=ca_sb[:], in_=caT.rearrange("(ko p) t -> p ko t", p=128))
            sm_sb = pp.tile([128, 8, NC_], BF16, tag="sm")
            nc.sync.dma_start(out=sm_sb[:], in_=smask[:])
            ones_bf = pp.tile([128, 512], BF16, tag="ones")
            nc.vector.memset(ones_bf[:], 1.0)
            gl_sb, bl_sb, gpp = [], [], []
            for i in range(3):
                ta = pp.tile([1, KO, 128], BF16, tag=f"gl{i}")
                nc.sync.dma_start(out=ta[:], in_=gb[i].rearrange("a (ko m) -> a ko m", m=128)[0:1])
                gl_sb.append(ta)
                tb = pp.tile([1, KO, 128], BF16, tag=f"bl{i}")
                nc.sync.dma_start(out=tb[:], in_=gb[i].rearrange("a (ko m) -> a ko m", m=128)[1:2])
                bl_sb.append(tb)
                t2 = pp.tile([128, KO], F32, tag=f"gpp{i}")
                with nc.allow_non_contiguous_dma(reason="tiny LN vector"):
                    nc.sync.dma_start(out=t2[:], in_=gpp_d[i].rearrange("(ko p) -> p ko", p=128))
                gpp.append(t2)
            bpp = []
            for nm, d in (("bo", bo_), ("boc", boc_), ("b2", b2_)):
                t_ = pp.tile([128, KO], F32, tag=nm)
                with nc.allow_non_contiguous_dma(reason="tiny bias vector"):
                    nc.scalar.dma_start(out=t_[:], in_=d.rearrange("(ko p) -> p ko", p=128))
                bpp.append(t_)
            eps_t = pp.tile([1, 1], F32, tag="eps")
            nc.vector.memset(eps_t[:], 1e-5)
            b1pp = pp.tile([128, 4 * E // 128], F32, tag="b1")
            with nc.allow_non_contiguous_dma(reason="tiny bias vector"):
                nc.scalar.dma_start(out=b1pp[:], in_=b1r.rearrange("(m p) -> p m", p=128))

            for ibody in range(nbody):
                _body_nocc(nc, tc, ibody, xTb_sb, xTo_sb, ca_sb, sm_sb, ones_bf,
                           (gl_sb, bl_sb), gpp, bpp, b1pp, eps_t,
                           dict(wq=wq, wk=wk, wv=wv, wqc=wqc, wkc=wkc, wvc=wvc,
                                wo=wo, woc=woc, w1=w1, w2=w2, xTo=xTo),
                           out_xT)
    nc.finalize()
    return nc


def _body_nocc(nc, tc, ibody, xTb_sb, xTo_sb, ca_sb, sm_sb, ones_bf, gbl, gpp,
               bpp, b1pp, eps_t, W, out_xT):
    gl_sb, bl_sb = gbl
    bopp, bocpp, b2pp = bpp

    if ibody > 0:
        nc.sync.dma_start(out=xTo_sb[:],
                          in_=W["xTo"].rearrange("p (ko t) -> p ko t", t=NC_))

    with tc.tile_pool(name=f"A{ibody}", bufs=1) as pa, \
         tc.tile_pool(name=f"ps{ibody}", bufs=8, space="PSUM") as pspool:

        pb2_ref = [None]

        def ps():
            return pspool.tile([128, NC_], F32, tag="ps", name="ps")

        def ln_rows(i, ps1, ps2, cs_out, ln, src, src_is_bf, gsl, ncols):
            m_ = pa.tile([1, NC_], F32, tag="row_m", bufs=1, name="m_")
            nc.vector.tensor_scalar_mul(m_[:, :ncols], ps1[0:1, :ncols], 1.0 / E)
            msq = pa.tile([1, NC_], F32, tag="row_q", bufs=1, name="msq")
            nc.vector.tensor_mul(msq[:, :ncols], m_[:, :ncols], m_[:, :ncols])
            var = pa.tile([1, NC_], F32, tag="row_v", bufs=1, name="var")
            nc.vector.scalar_tensor_tensor(var[:, :ncols], ps2[0:1, :ncols], 1.0 / E,
                                           msq[:, :ncols], OP.mult, OP.subtract)
            sqv = pa.tile([1, NC_], F32, tag="row_s", bufs=1, name="sqv")
            nc.scalar.activation(sqv[:, :ncols], var[:, :ncols], AF.Sqrt, bias=eps_t[:])
            rstd = pa.tile([1, NC_], F32, tag="row_r", bufs=1, name="rstd")
            nc.vector.reciprocal(rstd[:, :ncols], sqv[:, :ncols])
            rbf = pa.tile([1, NC_], BF16, tag="rowsb2", bufs=1, name="rbf")
            nc.vector.tensor_copy(rbf[:, :ncols], rstd[:, :ncols])
            nmr = pa.tile([1, NC_], BF16, tag="rowsb1", bufs=1, name="nmr")
            nc.vector.scalar_tensor_tensor(nmr[:, :ncols], m_[:, :ncols], -1.0,
                                           rstd[:, :ncols], OP.mult, OP.mult)
            rbc = ps()
            nc.tensor.matmul(rbc[:, :ncols], ones_bf[0:1, 0:128], rbf[:, :ncols],
                             start=True, stop=True)
            for ko in range(KO):
                bbc = ps()
                nc.tensor.matmul(bbc[:, :ncols], gl_sb[i][:, ko, :], nmr[:, :ncols],
                                 start=True, stop=False)
                nc.tensor.matmul(bbc[:, :ncols], bl_sb[i][:, ko, :],
                                 ones_bf[0:1, :ncols], start=False, stop=True)
                t0 = pa.tile([128, NC_], F32, tag="tmp", bufs=2, name="t0")
                nc.vector.scalar_tensor_tensor(t0[:, :ncols], src[ko],
                                               gpp[i][:, ko:ko + 1], rbc[:, :ncols],
                                               OP.mult, OP.mult)
                nc.vector.tensor_tensor(ln[:, ko, cs_out], t0[:, :ncols],
                                        bbc[:, :ncols], OP.add)

        def layer_norm1():
            """full-T LN over xTb (bf16 source)."""
            ln = pa.tile([128, KO, T], BF16, tag="lnf", name="lnf")
            for c in range(CC):
                cs = slice(c * NC_, (c + 1) * NC_)
                sq = pa.tile([128, KO, NC_], BF16, tag="stat", bufs=2, name="sq")
                nc.scalar.activation(sq[:], xTb_sb[:, :, cs], AF.Square)
                ps1, ps2 = ps(), ps()
                for ko in range(KO):
                    nc.tensor.matmul(ps1[0:1, :], ones_bf[:, 0:1], xTb_sb[:, ko, cs],
                                     start=(ko == 0), stop=(ko == KO - 1))
                for ko in range(KO):
                    nc.tensor.matmul(ps2[0:1, :], ones_bf[:, 0:1], sq[:, ko, :],
                                     start=(ko == 0), stop=(ko == KO - 1))
                ln_rows(0, ps1, ps2, cs, ln,
                        [xTb_sb[:, ko, cs] for ko in range(KO)], True, None, NC_)
            return ln

        def layer_norm_h(i):
            """own-half LN over xTo (f32 residual)."""
            ln = pa.tile([128, KO, NC_], BF16, tag="lnh", bufs=1, name="lnh")
            xb = pa.tile([128, KO, NC_], BF16, tag="stat", bufs=2, name="xb")
            for ko in range(KO):
                nc.scalar.copy(out=xb[:, ko, :], in_=xTo_sb[:, ko, :])
            sq = pa.tile([128, KO, NC_], BF16, tag="stat", bufs=2, name="sq")
            nc.scalar.activation(sq[:], xb[:], AF.Square)
            ps1, ps2 = ps(), ps()
            for ko in range(KO):
                nc.tensor.matmul(ps1[0:1, :], ones_bf[:, 0:1], xb[:, ko, :],
                                 start=(ko == 0), stop=(ko == KO - 1))
            for ko in range(KO):
                nc.tensor.matmul(ps2[0:1, :], ones_bf[:, 0:1], sq[:, ko, :],
                                 start=(ko == 0), stop=(ko == KO - 1))
            ln_rows(i, ps1, ps2, slice(0, NC_), ln,
                    [xTo_sb[:, ko, :] for ko in range(KO)], False, None, NC_)
            return ln

        def project_qt(lnsrc, w_d, cols):
            """Q^T for 16 heads over `cols` own tokens -> [128, 8, 512]."""
            qt = pb2_ref[0].tile([128, 8, NC_], BF16, tag="qon", bufs=2, name="qt")
            for jh in range(2):          # stream wq in halves of 512 cols
                w_sb = pb2_ref[0].tile([128, KO, 512], BF16, tag="wqkv", bufs=1, name="wsb")
                nc.sync.dma_start(
                    out=w_sb[:],
                    in_=w_d.rearrange("(ko p) m -> p ko m", p=128)[:, :, jh * 512:(jh + 1) * 512])
                for jj in range(4):
                    j = jh * 4 + jj
                    p_ = ps()
                    for ko in range(KO):
                        nc.tensor.matmul(p_[:, :], w_sb[:, ko, jj * 128:(jj + 1) * 128],
                                         lnsrc[ko], start=(ko == 0), stop=(ko == KO - 1))
                    nc.vector.tensor_copy(qt[:, j, :], p_[:, :])
            return qt

        def project_kt(src, w_d):
            """K^T for 16 heads over full S -> [128, 8, 1024]."""
            kt = pb2_ref[0].tile([128, 8, T], BF16, tag="kt", name="kt")
            for jh in range(2):
                w_sb = pb2_ref[0].tile([128, KO, 512], BF16, tag="wqkv", bufs=1, name="wsb")
                nc.sync.dma_start(
                    out=w_sb[:],
                    in_=w_d.rearrange("(ko p) m -> p ko m", p=128)[:, :, jh * 512:(jh + 1) * 512])
                for jj in range(4):
                    j = jh * 4 + jj
                    for c in range(CC):
                        p_ = ps()
                        for ko in range(KO):
                            nc.tensor.matmul(p_[:, :], w_sb[:, ko, jj * 128:(jj + 1) * 128],
                                             src[:, ko, c * NC_:(c + 1) * NC_],
                                             start=(ko == 0), stop=(ko == KO - 1))
                        nc.vector.tensor_copy(kt[:, j, c * NC_:(c + 1) * NC_], p_[:, :])
            return kt

        def project_v(src, w_d):
            """V for 16 heads -> [128, 8, 16, 65]."""
            vv = pb2_ref[0].tile([128, 8, H, 65], BF16, tag="vv", name="vv")
            for jh in range(2):
                w_sb = pb2_ref[0].tile([128, KO, 512], BF16, tag="wqkv", bufs=1, name="wsb")
                nc.sync.dma_start(
                    out=w_sb[:],
                    in_=w_d.rearrange("(ko p) m -> p ko m", p=128)[:, :, jh * 512:(jh + 1) * 512])
                for s in range(8):
                    p_ = ps()
                    for ko in range(KO):
                        nc.tensor.matmul(p_[:, :], src[:, ko, s * 128:(s + 1) * 128],
                                         w_sb[:, ko, :], start=(ko == 0), stop=(ko == KO - 1))
                    nc.scalar.copy(out=vv[:, s, jh * 8:(jh + 1) * 8, 0:64],
                                   in_=p_[:, :].rearrange("p (h d) -> p h d", d=64))
                    nc.vector.memset(vv[:, s, jh * 8:(jh + 1) * 8, 64:65], 1.0)
            return vv

        def attention(qt, kt, vv, onorm, masked):
            for h_ in range(H):
                j, half = h_ // 2, h_ % 2
                pb = 64 * half
                eb = pb2_ref[0].tile([128, 8, NC_], BF16, tag="expb", bufs=1, name="eb")
                for s_ in range(8):
                    p_ = ps()
                    nc.tensor.matmul(p_[:, :],
                                     kt[pb:pb + 64, j, s_ * 128:(s_ + 1) * 128],
                                     qt[pb:pb + 64, j, :], start=True, stop=True)
                    nc.scalar.activation(eb[:, s_, :], p_[:, :], AF.Exp)
                    if masked:
                        nc.vector.tensor_mul(eb[:, s_, :], eb[:, s_, :],
                                             sm_sb[:, s_, :])
                av = ps()
                for s_ in range(8):
                    nc.tensor.matmul(av[0:65, :], vv[:, s_, h_, :], eb[:, s_, :],
                                     start=(s_ == 0), stop=(s_ == 7))
                rr = pb2_ref[0].tile([65, NC_], F32, tag="row_rr", bufs=1, name="rr")
                nc.vector.reciprocal(rr[64:65, :], av[64:65, :])
                rb = pb2_ref[0].tile([65, NC_], BF16, tag="row_rrb", bufs=1, name="rb")
                nc.vector.tensor_copy(rb[64:65, :], rr[64:65, :])
                bc = ps()
                nc.tensor.matmul(bc[0:64, :], ones_bf[64:65, 0:64], rb[64:65, :],
                                 start=True, stop=True)
                bcs = pb2_ref[0].tile([64, NC_], BF16, tag="bcs", bufs=2, name="bcs")
                nc.vector.tensor_copy(bcs[:, :], bc[0:64, :])
                nc.vector.tensor_tensor(onorm[pb:pb + 64, j, :],
                                        av[0:64, :], bcs[:, :], OP.mult)

        def out_proj(onorm, wo_d, bias_pp):
            for m in range(KO):
                wom = pb2_ref[0].tile([128, KO, 128], BF16, tag="wom", bufs=4, name="wom")
                nc.sync.dma_start(
                    out=wom[:],
                    in_=wo_d.rearrange("(ks p) e -> p ks e", p=128)[:, :, m * 128:(m + 1) * 128])
                p_ = ps()
                for ks in range(KO):
                    nc.tensor.matmul(p_[:, :], wom[:, ks, :], onorm[:, ks, :],
                                     start=(ks == 0), stop=(ks == KO - 1))
                nc.vector.scalar_tensor_tensor(xTo_sb[:, m, :], p_[:, :],
                                               bias_pp[:, m:m + 1], xTo_sb[:, m, :],
                                               OP.add, OP.add)

        with tc.tile_pool(name=f"B{ibody}", bufs=1) as _pb2:
            pb2_ref[0] = _pb2
            # ---- self attention ----
            ln1 = layer_norm1()
            qt = project_qt([ln1[:, ko, 0:NC_] for ko in range(KO)], W["wq"], NC_)
            kt = project_kt(ln1, W["wk"])
            vv = project_v(ln1, W["wv"])
            on1 = _pb2.tile([128, 8, NC_], BF16, tag="qon", bufs=2, name="on1")
            attention(qt, kt, vv, on1, masked=True)
            out_proj(on1, W["wo"], bopp)

            # ---- cross attention ----
            ln2 = layer_norm_h(1)
            qtc = project_qt([ln2[:, ko, :] for ko in range(KO)], W["wqc"], NC_)
            ktc = project_kt(ca_sb, W["wkc"])
            vvc = project_v(ca_sb, W["wvc"])
            on2 = _pb2.tile([128, 8, NC_], BF16, tag="qon", bufs=2, name="on2")
            attention(qtc, ktc, vvc, on2, masked=False)
            out_proj(on2, W["woc"], bocpp)

        # ---- FFN ----
        ln3 = layer_norm_h(2)
        with tc.tile_pool(name=f"C{ibody}", bufs=1) as pc:
            FH = 4 * E // 128
            ht = pc.tile([128, FH, NC_], BF16, tag="ht", name="ht")
            for m in range(FH):
                w1m = pc.tile([128, KO, 128], BF16, tag="w1m", bufs=6, name="w1m")
                nc.sync.dma_start(
                    out=w1m[:],
                    in_=W["w1"][m].rearrange("p (ko f) -> p ko f", f=128))
                p_ = ps()
                for ko in range(KO):
                    nc.tensor.matmul(p_[:, :], w1m[:, ko, :], ln3[:, ko, 0:NC_],
                                     start=(ko == 0), stop=(ko == KO - 1))
                nc.scalar.activation(ht[:, m, :], p_[:, :], AF.Relu,
                                     bias=b1pp[:, m:m + 1])
            for m in range(KO):
                w2m = pc.tile([128, FH, 128], BF16, tag="w2m", bufs=2, name="w2m")
                nc.sync.dma_start(
                    out=w2m[:],
                    in_=W["w2"][m].rearrange("p (ks e) -> p ks e", e=128))
                p_ = ps()
                for ks in range(FH):
                    nc.tensor.matmul(p_[:, :], w2m[:, ks, :], ht[:, ks, :],
                                     start=(ks == 0), stop=(ks == FH - 1))
                nc.vector.scalar_tensor_tensor(xTo_sb[:, m, :], p_[:, :],
                                               b2pp[:, m:m + 1], xTo_sb[:, m, :],
                                               OP.add, OP.add)
        for ko in range(KO):
            nc.sync.dma_start(out=out_xT[ko * 128:(ko + 1) * 128, :],
                              in_=xTo_sb[:, ko, :])


def _make_core_inputs_nocc(c, inp):
    bf = ml_dtypes.bfloat16
    b, h = divmod(c, 2)
    sc = float(E) ** -0.5
    own = slice(512 * h, 512 * h + 512)
    oth = slice(512 * (1 - h), 512 * (1 - h) + 512)

    def stack_heads(w):  # [16, E, D] -> [E, 1024]
        return np.ascontiguousarray(np.transpose(w, (1, 0, 2)).reshape(E, E))

    def qkv_layout(w):  # [E, 1024] -> [2(half), 128, KO*512], contiguous DMA
        a = w.reshape(8, 128, 1024).transpose(1, 0, 2)      # [p, ko, m]
        return np.ascontiguousarray(np.stack(
            [a[:, :, :512].reshape(128, 8 * 512),
             a[:, :, 512:].reshape(128, 8 * 512)])).astype(bf)

    def mtile_layout(w, km, cm):  # [km*128, cm*128] -> [cm, 128, km*128]
        a = w.reshape(km, 128, cm, 128).transpose(2, 1, 0, 3)
        return np.ascontiguousarray(a.reshape(cm, 128, km * 128)).astype(bf)

    def pkt_layout(xT):  # [E, ncols] -> [128, KO*ncols]
        n = xT.shape[1]
        return np.ascontiguousarray(
            xT.reshape(8, 128, n).transpose(1, 0, 2).reshape(128, 8 * n))

    xt = np.asarray(inp["x"][b], np.float32)           # [T, E]
    xperm = np.concatenate([xt[own], xt[oth]], axis=0)  # keys permuted: own first
    # causal mask in permuted key order, own queries t = 512h + f
    s_perm = np.arange(1024)
    s_glob = np.where(s_perm < 512, s_perm + 512 * h, s_perm - 512 * h)
    f = np.arange(512)
    mask = (s_glob[:, None] <= (512 * h + f)[None, :])  # [1024, 512]
    smask = mask.reshape(8, 128, 512).transpose(1, 0, 2).astype(bf)

    return {
        "xTb": np.ascontiguousarray(xperm.T).astype(bf),
        "xTo": pkt_layout(xt[own].T).astype(np.float32),
        "caT": pkt_layout(np.asarray(inp["ca"][b]).T).astype(bf),
        "wq": qkv_layout(stack_heads(inp["Wq_s"]) * sc),
        "wk": qkv_layout(stack_heads(inp["Wk_s"])),
        "wv": qkv_layout(stack_heads(inp["Wv_s"])),
        "wqc": qkv_layout(stack_heads(inp["Wq_c"]) * sc),
        "wkc": qkv_layout(stack_heads(inp["Wk_c"])),
        "wvc": qkv_layout(stack_heads(inp["Wv_c"])),
        "wo": mtile_layout(np.asarray(inp["Wo_s"], np.float32), 8, 8),
        "woc": mtile_layout(np.asarray(inp["Wo_c"], np.float32), 8, 8),
        "w1": mtile_layout(np.asarray(inp["W1"], np.float32), 8, 32),
        "w2": mtile_layout(np.asarray(inp["W2"], np.float32), 32, 8),
        "gb1": np.stack([inp["ln1_g"], inp["ln1_b"]]).astype(bf),
        "gb2": np.stack([inp["ln2_g"], inp["ln2_b"]]).astype(bf),
        "gb3": np.stack([inp["ln3_g"], inp["ln3_b"]]).astype(bf),
        "g1": np.asarray(inp["ln1_g"], np.float32),
        "g2": np.asarray(inp["ln2_g"], np.float32),
        "g3": np.asarray(inp["ln3_g"], np.float32),
        "bo": np.asarray(inp["bo_s"], np.float32),
        "boc": np.asarray(inp["bo_c"], np.float32),
        "b2": np.asarray(inp["b2"], np.float32),
        "b1r": np.asarray(inp["b1"], np.float32),
        "smask": smask,
    }



# revision 29
# speedup vs baseline: 1.0186x; 1.0186x over previous
"""Trainium2 Bass kernel for nn_DecoderBlock (B=4,T=S=E=1024,H=16,D=64) on 8 cores.

Active variant ("v2", see VARIANT below): communication-free sharding,
core = (batch b, T-half h).  Each core computes its 512 query tokens for all
16 heads plus the full FFN; K/V work is duplicated across the pair so no
collective is ever issued (pairwise AllReduce on this fleet is slow and
jittery).  Self-attn keys are permuted own-half-first so the SPMD program is
identical on every core; the causal structure is handled by a diagonal-block
multiplicative mask plus zeroing V (and its denominator ones-row) for key
blocks invisible to the whole core.

The residual stream lives transposed on-chip as x^T [E(partitions), T(free)]
so every matmul is already in lhsT/rhs layout.  LayerNorm statistics are
computed with an all-ones [128x128] stationary matmul so mean/rstd are
broadcast to all partitions and the row math runs 128 lanes wide; rstd uses
sqrt + vector.reciprocal (reciprocal_approx_fast miscompiles on HW).
Attention is software-pipelined per head (QK of head h+1 issues before AV of
head h); softmax denominators ride as a 65th V row, are gathered to a [16,512]
tile with tiny SBUF->SBUF DMAs, inverted by one batched reciprocal, and
broadcast back through a constant selector-matrix matmul.

Older variants ("tp2" = DP4 x TP2 with pairwise AllReduce, "nocc" = first
comm-free version) are kept for reference and A/B timing.
"""
import sys

sys.path.insert(0, "/opt/trn_rl_repo")

import numpy as np
import ml_dtypes

import concourse.bass as bass
import concourse.bacc as bacc
import concourse.mybir as mybir
import concourse.tile as tile

BF16 = mybir.dt.bfloat16
F32 = mybir.dt.float32
AF = mybir.ActivationFunctionType
OP = mybir.AluOpType

B, T, S, E, H, D = 4, 1024, 1024, 1024, 16, 64
HL = H // 2          # heads per core (TP-2)
FF = 4 * E // 2      # ffn hidden per core
KO = E // 128        # 8 partition subtiles of E
NC_ = 512            # matmul free-dim chunk
CC = T // NC_        # 2 chunks over T
PAIRS = [[0, 1], [2, 3], [4, 5], [6, 7]]


SKIP_CC = False


def build(nbody=1):
    nc = bacc.Bacc(num_devices=8)

    def P(name, shape, dt):
        return nc.declare_dram_parameter(name, shape, dt, isOutput=False)

    xT = P("xT", [E, T], F32)
    caT = P("caT", [E, S], BF16)
    wq, wk, wv = P("wq", [E, 512], BF16), P("wk", [E, 512], BF16), P("wv", [E, 512], BF16)
    wqc, wkc, wvc = P("wqc", [E, 512], BF16), P("wkc", [E, 512], BF16), P("wvc", [E, 512], BF16)
    wo, woc = P("wo", [512, E], BF16), P("woc", [512, E], BF16)
    w1, w2 = P("w1", [E, FF], BF16), P("w2", [FF, E], BF16)
    gb = [P(f"gb{i}", [2, E], BF16) for i in (1, 2, 3)]
    gpp_d = [P(f"g{i}", [E], F32) for i in (1, 2, 3)]
    bo2, bo2c, b22 = P("bo2", [E], F32), P("bo2c", [E], F32), P("b22", [E], F32)
    b1r = P("b1r", [FF], F32)
    cmask = P("cmask", [128, 4, 512], BF16)
    out_xT = nc.declare_dram_parameter("out_xT", [E, T], F32, isOutput=True)

    with tile.TileContext(nc) as tc:
        with tc.tile_pool(name="persist", bufs=1) as pp:
            xT_sb = pp.tile([128, KO, T], F32, tag="xT")
            for ko in range(KO):
                nc.sync.dma_start(out=xT_sb[:, ko, :],
                                  in_=xT[ko * 128:(ko + 1) * 128, :])
            ca_sb = pp.tile([128, KO, S], BF16, tag="ca")
            nc.sync.dma_start(out=ca_sb[:], in_=caT.rearrange("(ko p) t -> p ko t", p=128))
            cm_sb = pp.tile([128, 4, 512], BF16, tag="cm")
            nc.sync.dma_start(out=cm_sb[:], in_=cmask[:])
            ones_bf = pp.tile([128, 512], BF16, tag="ones")
            nc.vector.memset(ones_bf[:], 1.0)
            gl_sb, bl_sb, gpp = [], [], []
            for i in range(3):
                ta = pp.tile([1, KO, 128], BF16, tag=f"gl{i}")
                nc.sync.dma_start(out=ta[:], in_=gb[i].rearrange("a (ko m) -> a ko m", m=128)[0:1])
                gl_sb.append(ta)
                tb = pp.tile([1, KO, 128], BF16, tag=f"bl{i}")
                nc.sync.dma_start(out=tb[:], in_=gb[i].rearrange("a (ko m) -> a ko m", m=128)[1:2])
                bl_sb.append(tb)
                t2 = pp.tile([128, KO], F32, tag=f"gpp{i}")
                with nc.allow_non_contiguous_dma(reason="tiny LN vector"):
                    nc.sync.dma_start(out=t2[:], in_=gpp_d[i].rearrange("(ko p) -> p ko", p=128))
                gpp.append(t2)
            bpp = []
            for nm, d in (("bo2", bo2), ("bo2c", bo2c), ("b22", b22)):
                t_ = pp.tile([128, KO], F32, tag=nm)
                with nc.allow_non_contiguous_dma(reason="tiny bias vector"):
                    nc.scalar.dma_start(out=t_[:], in_=d.rearrange("(ko p) -> p ko", p=128))
                bpp.append(t_)
            eps_t = pp.tile([1, 1], F32, tag="eps")
            nc.vector.memset(eps_t[:], 1e-5)
            b1pp = pp.tile([128, FF // 128], F32, tag="b1")
            with nc.allow_non_contiguous_dma(reason="tiny bias vector"):
                nc.sync.dma_start(out=b1pp[:], in_=b1r.rearrange("(m p) -> p m", p=128))

            for ibody in range(nbody):
                _body(nc, tc, ibody, xT_sb, ca_sb, cm_sb, ones_bf, (gl_sb, bl_sb), gpp,
                      bpp, b1pp, eps_t,
                      dict(wq=wq, wk=wk, wv=wv, wqc=wqc, wkc=wkc, wvc=wvc,
                           wo=wo, woc=woc, w1=w1, w2=w2, xT=xT),
                      out_xT)
    nc.finalize()
    return nc


def _body(nc, tc, ibody, xT_sb, ca_sb, cm_sb, ones_bf, gbl, gpp,
          bpp, b1pp, eps_t, W, out_xT):
    gl_sb, bl_sb = gbl
    bo2pp, bo2cpp, b22pp = bpp
    ar = {}
    for k in (1, 2, 3):
        ar[k] = [(nc.dram_tensor(f"ar{k}_{ibody}_{c}_in", [E, NC_], F32),
                  nc.dram_tensor(f"ar{k}_{ibody}_{c}_out", [E, NC_], F32))
                 for c in range(CC)]

    if ibody > 0:
        # re-load pristine x for the timing replica
        for ko in range(KO):
            nc.sync.dma_start(out=xT_sb[:, ko, :],
                              in_=W["xT"][ko * 128:(ko + 1) * 128, :])

    with tc.tile_pool(name=f"A{ibody}", bufs=1) as pa, \
         tc.tile_pool(name=f"ps{ibody}", bufs=8, space="PSUM") as pspool:

        def ps():
            return pspool.tile([128, NC_], F32, tag="ps", name="ps")

        def layer_norm(i):
            """LN over partitions of xT_sb -> bf16 tile [128, KO, T]."""
            ln = pa.tile([128, KO, T], BF16, tag="lnout", name="ln")
            for c in range(CC):
                cs = slice(c * NC_, (c + 1) * NC_)
                xb = pa.tile([128, KO, NC_], BF16, tag="stat", bufs=2, name="xb")
                for ko in range(KO):
                    nc.scalar.copy(out=xb[:, ko, :], in_=xT_sb[:, ko, cs])
                sq = pa.tile([128, KO, NC_], BF16, tag="stat", bufs=2, name="sq")
                nc.scalar.activation(sq[:], xb[:], AF.Square)
                ps1, ps2 = ps(), ps()
                for ko in range(KO):
                    nc.tensor.matmul(ps1[0:1, :], ones_bf[:, 0:1], xb[:, ko, :],
                                     start=(ko == 0), stop=(ko == KO - 1))
                for ko in range(KO):
                    nc.tensor.matmul(ps2[0:1, :], ones_bf[:, 0:1], sq[:, ko, :],
                                     start=(ko == 0), stop=(ko == KO - 1))
                m_ = pa.tile([1, NC_], F32, tag="row_m", bufs=1, name="m_")
                nc.vector.tensor_scalar_mul(m_[:], ps1[0:1, :], 1.0 / E)
                msq = pa.tile([1, NC_], F32, tag="row_q", bufs=1, name="msq")
                nc.vector.tensor_mul(msq[:], m_[:], m_[:])
                var = pa.tile([1, NC_], F32, tag="row_v", bufs=1, name="var")
                nc.vector.scalar_tensor_tensor(var[:], ps2[0:1, :], 1.0 / E,
                                               msq[:], OP.mult, OP.subtract)
                sqv = pa.tile([1, NC_], F32, tag="row_s", bufs=1, name="sqv")
                nc.scalar.activation(sqv[:], var[:], AF.Sqrt, bias=eps_t[:])
                rstd = pa.tile([1, NC_], F32, tag="row_r", bufs=1, name="rstd")
                nc.vector.reciprocal(rstd[:], sqv[:])
                rbf = pa.tile([1, NC_], BF16, tag="rowsb2", bufs=1, name="rbf")
                nc.vector.tensor_copy(rbf[:], rstd[:])
                nmr = pa.tile([1, NC_], BF16, tag="rowsb1", bufs=1, name="nmr")
                # nmr = -m * rstd
                nc.vector.scalar_tensor_tensor(nmr[:], m_[:], -1.0,
                                               rstd[:], OP.mult, OP.mult)
                rbc = ps()
                nc.tensor.matmul(rbc[:, :], ones_bf[0:1, 0:128], rbf[:],
                                 start=True, stop=True)
                for ko in range(KO):
                    bbc = ps()
                    nc.tensor.matmul(bbc[:, :], gl_sb[i][:, ko, :], nmr[:],
                                     start=True, stop=False)
                    nc.tensor.matmul(bbc[:, :], bl_sb[i][:, ko, :],
                                     ones_bf[0:1, 0:NC_], start=False, stop=True)
                    t0 = pa.tile([128, NC_], F32, tag="tmp", bufs=2, name="t0")
                    nc.vector.scalar_tensor_tensor(t0[:], xT_sb[:, ko, cs],
                                                   gpp[i][:, ko:ko + 1], rbc[:, :],
                                                   OP.mult, OP.mult)
                    nc.vector.tensor_tensor(ln[:, ko, cs], t0[:], bbc[:, :], OP.add)
            return ln

        def project_qk(pb_, lnsrc, w_d, tag, bufs=1):
            """-> [128, 4, T] bf16 : rows = 2 heads x 64, per pair j."""
            w_sb = pb_.tile([128, KO, 512], BF16, tag="wqkv", bufs=2, name="wsb")
            nc.sync.dma_start(out=w_sb[:], in_=w_d.rearrange("(ko p) m -> p ko m", p=128))
            qt = pb_.tile([128, 4, T], BF16, tag=tag, bufs=bufs, name="qt")
            for j in range(4):
                for c in range(CC):
                    p_ = ps()
                    for ko in range(KO):
                        nc.tensor.matmul(p_[:, :], w_sb[:, ko, j * 128:(j + 1) * 128],
                                         lnsrc[:, ko, c * NC_:(c + 1) * NC_],
                                         start=(ko == 0), stop=(ko == KO - 1))
                    nc.vector.tensor_copy(qt[:, j, c * NC_:(c + 1) * NC_], p_[:, :])
            return qt

        def project_v(pb_, src, w_d):
            """-> [128, 8, 8, 65] bf16 : [s_part, s_sub, head, d|ones]."""
            w_sb = pb_.tile([128, KO, 512], BF16, tag="wqkv", bufs=2, name="wsb")
            nc.sync.dma_start(out=w_sb[:], in_=w_d.rearrange("(ko p) m -> p ko m", p=128))
            vv = pb_.tile([128, 8, HL, 65], BF16, tag="vv", name="vv")
            for s in range(8):
                p_ = ps()
                for ko in range(KO):
                    nc.tensor.matmul(p_[:, :], src[:, ko, s * 128:(s + 1) * 128],
                                     w_sb[:, ko, :], start=(ko == 0), stop=(ko == KO - 1))
                nc.scalar.copy(out=vv[:, s, :, 0:64],
                               in_=p_[:, :].rearrange("p (h d) -> p h d", d=64))
                nc.vector.memset(vv[:, s, :, 64:65], 1.0)
            return vv

        def attention(pb_, qt, kt, vv, causal):
            onorm = pb_.tile([128, 4, T], BF16, tag="onorm", name="onorm")
            for c in range(CC):
                for h in range(HL):
                    j, half = h // 2, h % 2
                    pb = 64 * half
                    subs = list(range(4 * (c + 1))) if causal else list(range(8))
                    eb = pb_.tile([128, 8, NC_], BF16, tag="expb", bufs=2, name="eb")
                    for s_ in subs:
                        p_ = ps()
                        nc.tensor.matmul(p_[:, :],
                                         kt[pb:pb + 64, j, s_ * 128:(s_ + 1) * 128],
                                         qt[pb:pb + 64, j, c * NC_:(c + 1) * NC_],
                                         start=True, stop=True)
                        nc.scalar.activation(eb[:, s_, :], p_[:, :], AF.Exp)
                        if causal and s_ >= 4 * c:
                            nc.vector.tensor_mul(eb[:, s_, :], eb[:, s_, :],
                                                 cm_sb[:, s_ - 4 * c, :])
                    av = ps()
                    for i_, s_ in enumerate(subs):
                        nc.tensor.matmul(av[0:65, :], vv[:, s_, h, :], eb[:, s_, :],
                                         start=(i_ == 0), stop=(i_ == len(subs) - 1))
                    rr = pb_.tile([65, NC_], F32, tag="row_rr", bufs=2, name="rr")
                    nc.vector.reciprocal(rr[64:65, :], av[64:65, :])
                    rb = pb_.tile([65, NC_], BF16, tag="row_rrb", bufs=2, name="rb")
                    nc.vector.tensor_copy(rb[64:65, :], rr[64:65, :])
                    bc = ps()
                    nc.tensor.matmul(bc[0:64, :], ones_bf[64:65, 0:64], rb[64:65, :],
                                     start=True, stop=True)
                    bcs = pb_.tile([64, NC_], BF16, tag="bcs", bufs=2, name="bcs")
                    nc.vector.tensor_copy(bcs[:, :], bc[0:64, :])
                    nc.vector.tensor_tensor(onorm[pb:pb + 64, j, c * NC_:(c + 1) * NC_],
                                            av[0:64, :], bcs[:, :], OP.mult)
            return onorm

        def out_proj(pb_, onorm, wo_d, ark, bias_pp):
            wo_sb = pb_.tile([128, 4, E], BF16, tag="wo", name="wo_sb")
            nc.sync.dma_start(out=wo_sb[:], in_=wo_d.rearrange("(ks p) e -> p ks e", p=128))
            for c in range(CC):
                for m in range(KO):
                    p_ = ps()
                    for ks in range(4):
                        nc.tensor.matmul(p_[:, :], wo_sb[:, ks, m * 128:(m + 1) * 128],
                                         onorm[:, ks, c * NC_:(c + 1) * NC_],
                                         start=(ks == 0), stop=(ks == 3))
                    st = pa.tile([128, NC_], F32, tag="arst", bufs=4, name="st")
                    nc.vector.tensor_scalar_add(st[:, :], p_[:, :], bias_pp[:, m:m + 1])
                    nc.sync.dma_start(out=ark[c][0][m * 128:(m + 1) * 128, :], in_=st[:])
                allreduce_c(ark, c)

        def allreduce_c(ark, c):
            a_in, a_out = ark[c]
            if SKIP_CC:
                nc.sync.dma_start(out=a_out[:], in_=a_in[:])
            else:
                nc.gpsimd.collective_compute(
                    "AllReduce", OP.add, replica_groups=PAIRS,
                    ins=[a_in[:]], outs=[a_out[:]])
            nc.gpsimd.dma_start(
                out=xT_sb[:, :, c * NC_:(c + 1) * NC_],
                in_=a_out.rearrange("(ko p) t -> p ko t", p=128),
                accum_op=OP.add)



        with tc.tile_pool(name=f"B{ibody}", bufs=1) as pb_:
            # ---- self attention ----
            ln1 = layer_norm(0)
            qt = project_qk(pb_, ln1, W["wq"], "qt")
            kt = project_qk(pb_, ln1, W["wk"], "kt")
            vv = project_v(pb_, ln1, W["wv"])
            on1 = attention(pb_, qt, kt, vv, causal=True)
            out_proj(pb_, on1, W["wo"], ar[1], bo2pp)
            # cross K/V from raw ca — independent of AR1, fills the gap
            ktc = project_qk(pb_, ca_sb, W["wkc"], "kt")
            vvc = project_v(pb_, ca_sb, W["wvc"])

            # ---- cross attention ----
            ln2 = layer_norm(1)
            qtc = project_qk(pb_, ln2, W["wqc"], "qt")
            on2 = attention(pb_, qtc, ktc, vvc, causal=False)
            out_proj(pb_, on2, W["woc"], ar[2], bo2cpp)

        # ---- FFN ----
        ln3 = layer_norm(2)
        with tc.tile_pool(name=f"C{ibody}", bufs=1) as pc:
            ht = pc.tile([128, FF // 128, T], BF16, tag="ht", name="ht")
            for m in range(FF // 128):
                w1m = pc.tile([128, KO, 128], BF16, tag="w1m", bufs=6, name="w1m")
                nc.sync.dma_start(
                    out=w1m[:],
                    in_=W["w1"][m].rearrange("p (ko f) -> p ko f", f=128))
                for c in range(CC):
                    p_ = ps()
                    for ko in range(KO):
                        nc.tensor.matmul(p_[:, :], w1m[:, ko, :],
                                         ln3[:, ko, c * NC_:(c + 1) * NC_],
                                         start=(ko == 0), stop=(ko == KO - 1))
                    nc.scalar.activation(ht[:, m, c * NC_:(c + 1) * NC_], p_[:, :],
                                         AF.Relu, bias=b1pp[:, m:m + 1])
            w2m_t = [None] * KO
            for m in range(KO):
                w2m = pc.tile([128, FF // 128, 128], BF16, tag="w2m", bufs=8, name="w2m")
                nc.sync.dma_start(
                    out=w2m[:],
                    in_=W["w2"].rearrange("(ks p) e -> p ks e", p=128)[:, :, m * 128:(m + 1) * 128])
                w2m_t[m] = w2m
            for c in range(CC):
                for m in range(KO):
                    p_ = ps()
                    for ks in range(FF // 128):
                        nc.tensor.matmul(p_[:, :], w2m_t[m][:, ks, :],
                                         ht[:, ks, c * NC_:(c + 1) * NC_],
                                         start=(ks == 0), stop=(ks == FF // 128 - 1))
                    st = pa.tile([128, NC_], F32, tag="arst", bufs=4, name="st")
                    nc.vector.tensor_scalar_add(st[:, :], p_[:, :], b22pp[:, m:m + 1])
                    nc.sync.dma_start(out=ar[3][c][0][m * 128:(m + 1) * 128, :], in_=st[:])
                allreduce_c(ar[3], c)
                for ko in range(KO):
                    nc.sync.dma_start(
                        out=out_xT[ko * 128:(ko + 1) * 128, c * NC_:(c + 1) * NC_],
                        in_=xT_sb[:, ko, c * NC_:(c + 1) * NC_])


# ------------------------------------------------------------------ v2 (comm-free, pipelined)

def build2(nbody=1):
    """Comm-free sharding (core = (batch b, T-half h)) with:
    - LN stats broadcast across partitions (all-ones stationary matmul) so all
      row math runs 128-wide; rstd via sqrt + reciprocal_approx_fast.
    - Attention: software-pipelined QK/exp/AV across heads; per-head softmax
      denominators ride as a 65th V row, normalization deferred and batched
      (one reciprocal for all 16 heads).
    - Fully-masked key blocks handled by zeroing V (+ its ones row) per core;
      only the 4 diagonal blocks get a post-exp multiplicative mask.
    """
    nc = bacc.Bacc(num_devices=8)

    def P(name, shape, dt):
        return nc.declare_dram_parameter(name, shape, dt, isOutput=False)

    # all weight/activation layouts are host-pretransposed so every DMA is
    # contiguous per partition (descriptor-bound strided gathers killed ~180us
    # per DMA engine in the naive [E, .] layouts)
    xTb = P("xTb", [E, T], BF16)        # permuted x^T, bf16 (LN1 / self K,V)
    xTo = P("xTo", [128, KO * NC_], F32)   # own-half x^T  [p, ko*t]
    caT = P("caT", [128, KO * S], BF16)    # ca^T           [p, ko*t]
    wq, wk, wv = P("wq", [2, 128, KO * 512], BF16), P("wk", [2, 128, KO * 512], BF16), P("wv", [2, 128, KO * 512], BF16)
    wqc, wkc, wvc = P("wqc", [2, 128, KO * 512], BF16), P("wkc", [2, 128, KO * 512], BF16), P("wvc", [2, 128, KO * 512], BF16)
    wo, woc = P("wo", [KO, 128, E], BF16), P("woc", [KO, 128, E], BF16)
    w1 = P("w1", [4 * E // 128, 128, KO * 128], BF16)
    w2 = P("w2", [KO, 128, (4 * E // 128) * 128], BF16)
    gpp_d = [P(f"g{i}", [E], F32) for i in (1, 2, 3)]
    bpp_d = [P(f"be{i}", [E], F32) for i in (1, 2, 3)]
    bo_, boc_, b2_ = P("bo", [E], F32), P("boc", [E], F32), P("b2", [E], F32)
    b1r = P("b1r", [4 * E], F32)
    smask = P("smask", [128, 4, NC_], BF16)   # diagonal causal blocks
    vmsk = P("vmsk", [128, 8], F32)           # per-key-block V mask
    seld = P("seld", [16, 8, 128], BF16)      # head-pair broadcast selector
    out_xT = nc.declare_dram_parameter("out_xT", [E, NC_], F32, isOutput=True)

    with tile.TileContext(nc) as tc:
        with tc.tile_pool(name="persist", bufs=1) as pp:
            xTb_sb = pp.tile([128, KO, T], BF16, tag="xTb")
            for ko in range(KO):
                nc.sync.dma_start(out=xTb_sb[:, ko, :], in_=xTb[ko * 128:(ko + 1) * 128, :])
            xTo_sb = pp.tile([128, KO, NC_], F32, tag="xTo")
            nc.sync.dma_start(out=xTo_sb[:], in_=xTo.rearrange("p (ko t) -> p ko t", t=NC_))
            ca_sb = pp.tile([128, KO, S], BF16, tag="ca")
            nc.gpsimd.dma_start(out=ca_sb[:], in_=caT.rearrange("p (ko t) -> p ko t", t=S))
            sm_sb = pp.tile([128, 4, NC_], BF16, tag="sm")
            nc.scalar.dma_start(out=sm_sb[:], in_=smask[:])
            vm_sb = pp.tile([128, 8], F32, tag="vm")
            nc.scalar.dma_start(out=vm_sb[:], in_=vmsk[:])
            sel_sb = pp.tile([16, 8, 128], BF16, tag="sel")
            nc.scalar.dma_start(out=sel_sb[:], in_=seld[:])
            ones_bf = pp.tile([128, 512], BF16, tag="ones")
            nc.vector.memset(ones_bf[:], 1.0)
            gpp, bepp = [], []
            for i in range(3):
                t2 = pp.tile([128, KO], F32, tag=f"gpp{i}")
                with nc.allow_non_contiguous_dma(reason="tiny LN vector"):
                    nc.scalar.dma_start(out=t2[:], in_=gpp_d[i].rearrange("(ko p) -> p ko", p=128))
                gpp.append(t2)
                t3 = pp.tile([128, KO], F32, tag=f"bepp{i}")
                with nc.allow_non_contiguous_dma(reason="tiny LN vector"):
                    nc.scalar.dma_start(out=t3[:], in_=bpp_d[i].rearrange("(ko p) -> p ko", p=128))
                bepp.append(t3)
            bpp = []
            for nm, d in (("bo", bo_), ("boc", boc_), ("b2", b2_)):
                t_ = pp.tile([128, KO], F32, tag=nm)
                with nc.allow_non_contiguous_dma(reason="tiny bias vector"):
                    nc.scalar.dma_start(out=t_[:], in_=d.rearrange("(ko p) -> p ko", p=128))
                bpp.append(t_)
            eps_t = pp.tile([128, 1], F32, tag="eps")
            nc.vector.memset(eps_t[:], 1e-5)
            b1pp = pp.tile([128, 4 * E // 128], F32, tag="b1")
            with nc.allow_non_contiguous_dma(reason="tiny bias vector"):
                nc.scalar.dma_start(out=b1pp[:], in_=b1r.rearrange("(m p) -> p m", p=128))

            for ibody in range(nbody):
                _body2(nc, tc, ibody, xTb_sb, xTo_sb, ca_sb, sm_sb, vm_sb, sel_sb,
                       ones_bf, gpp, bepp, bpp, b1pp, eps_t,
                       dict(wq=wq, wk=wk, wv=wv, wqc=wqc, wkc=wkc, wvc=wvc,
                            wo=wo, woc=woc, w1=w1, w2=w2, xTo=xTo),
                       out_xT)
    nc.finalize()
    return nc


def _body2(nc, tc, ibody, xTb_sb, xTo_sb, ca_sb, sm_sb, vm_sb, sel_sb, ones_bf,
           gpp, bepp, bpp, b1pp, eps_t, W, out_xT):
    bopp, bocpp, b2pp = bpp

    if ibody > 0:
        nc.sync.dma_start(out=xTo_sb[:],
                          in_=W["xTo"].rearrange("p (ko t) -> p ko t", t=NC_))

    with tc.tile_pool(name=f"A{ibody}", bufs=1) as pa, \
         tc.tile_pool(name=f"ps{ibody}", bufs=8, space="PSUM") as pspool:

        def ps():
            return pspool.tile([128, NC_], F32, tag="ps", name="ps")

        def ln_v2(i, stat_tile, stat_off, mat_srcs, ln, nchunks):
            """stat_tile: bf16 tile [128, KO, >=stat_off+512*nchunks];
            mat_srcs[c][ko]: AP [128,512] (bf16 or f32); writes ln[:, ko, :]."""
            for c in range(nchunks):
                so = stat_off + c * NC_
                msrc = mat_srcs[c]
                sq = pa.tile([128, KO, NC_], BF16, tag="sq", bufs=1, name="sq")
                for ko in range(KO):
                    nc.vector.tensor_mul(sq[:, ko, :], stat_tile[:, ko, so:so + NC_],
                                         stat_tile[:, ko, so:so + NC_])
                ps1, ps2 = ps(), ps()
                for ko in range(KO):
                    nc.tensor.matmul(ps1[:, :], ones_bf[:, 0:128],
                                     stat_tile[:, ko, so:so + NC_],
                                     start=(ko == 0), stop=(ko == KO - 1))
                for ko in range(KO):
                    nc.tensor.matmul(ps2[:, :], ones_bf[:, 0:128], sq[:, ko, :],
                                     start=(ko == 0), stop=(ko == KO - 1))
                mbc = pa.tile([128, NC_], F32, tag="mbc", bufs=1, name="mbc")
                nc.vector.tensor_scalar_mul(mbc[:], ps1[:, :], 1.0 / E)
                msq = pa.tile([128, NC_], F32, tag="msq", bufs=1, name="msq")
                nc.vector.tensor_mul(msq[:], mbc[:], mbc[:])
                var = pa.tile([128, NC_], F32, tag="var", bufs=1, name="var")
                nc.vector.scalar_tensor_tensor(var[:], ps2[:, :], 1.0 / E,
                                               msq[:], OP.mult, OP.subtract)
                sqv = pa.tile([128, NC_], F32, tag="sqv", bufs=1, name="sqv")
                nc.scalar.activation(sqv[:], var[:], AF.Sqrt, bias=eps_t[:])
                rstd = pa.tile([128, NC_], F32, tag="rstd", bufs=1, name="rstd")
                nc.vector.reciprocal(rstd[:], sqv[:])
                nmr = pa.tile([128, NC_], F32, tag="nmr", bufs=1, name="nmr")
                nc.vector.scalar_tensor_tensor(nmr[:], mbc[:], -1.0,
                                               rstd[:], OP.mult, OP.mult)
                cs = slice(c * NC_, (c + 1) * NC_)
                for ko in range(KO):
                    t_ = pa.tile([128, NC_], F32, tag="lt", bufs=2, name="lt")
                    nc.vector.scalar_tensor_tensor(t_[:], msrc[ko],
                                                   gpp[i][:, ko:ko + 1], rstd[:],
                                                   OP.mult, OP.mult)
                    u_ = pa.tile([128, NC_], F32, tag="lu", bufs=2, name="lu")
                    nc.vector.scalar_tensor_tensor(u_[:], nmr[:],
                                                   gpp[i][:, ko:ko + 1], t_[:],
                                                   OP.mult, OP.add)
                    nc.vector.tensor_scalar_add(ln[:, ko, cs], u_[:],
                                                bepp[i][:, ko:ko + 1])

        def proj16(pb_, lnsrc, w_d, out_t, ncols):
            """16-head projection -> out_t [128, 8, ncols] bf16."""
            for jh in range(2):
                w_sb = pb_.tile([128, KO, 512], BF16, tag="wqkv", bufs=2, name="wsb")
                nc.sync.dma_start(
                    out=w_sb[:],
                    in_=w_d[jh].rearrange("p (ko m) -> p ko m", m=512))
                for jj in range(4):
                    j = jh * 4 + jj
                    for c in range(ncols // NC_):
                        p_ = ps()
                        for ko in range(KO):
                            nc.tensor.matmul(p_[:, :], w_sb[:, ko, jj * 128:(jj + 1) * 128],
                                             lnsrc[:, ko, c * NC_:(c + 1) * NC_],
                                             start=(ko == 0), stop=(ko == KO - 1))
                        nc.vector.tensor_copy(out_t[:, j, c * NC_:(c + 1) * NC_], p_[:, :])

        def proj_v(pb_, src, w_d, vv, use_vmask):
            for jh in range(2):
                w_sb = pb_.tile([128, KO, 512], BF16, tag="wqkv", bufs=2, name="wsb")
                nc.sync.dma_start(
                    out=w_sb[:],
                    in_=w_d[jh].rearrange("p (ko m) -> p ko m", m=512))
                for s in range(8):
                    p_ = ps()
                    for ko in range(KO):
                        nc.tensor.matmul(p_[:, :], src[:, ko, s * 128:(s + 1) * 128],
                                         w_sb[:, ko, :], start=(ko == 0), stop=(ko == KO - 1))
                    nc.scalar.copy(out=vv[:, s, jh * 8:(jh + 1) * 8, 0:64],
                                   in_=p_[:, :].rearrange("p (h d) -> p h d", d=64))
            for s in range(8):
                nc.vector.memset(vv[:, s, :, 64:65], 1.0)
                if use_vmask:
                    nc.vector.tensor_scalar_mul(vv[:, s, :, :],
                                                vv[:, s, :, :], vm_sb[:, s:s + 1])

        def attention2(pb_, qt, kt, vv, avo, masked):
            """Pipelined attention: per-head QK->exp->AV with the next head's
            QK emitted before this head's AV.  Softmax denominators ride as the
            65th V row, are gathered (lagged, via tiny K=1 matmuls) onto rows
            0..15 of one PSUM tile, and a single batched reciprocal serves all
            16 heads before the broadcast/normalize pass."""
            ebs, rowbufs = {}, {}
            dn16 = pb_.tile([16, NC_], F32, tag="dn16", bufs=1, name="dn16")

            def emit_qk(h):
                j, half = h // 2, h % 2
                pb = 64 * half
                eb = pb_.tile([128, 8, NC_], BF16, tag="eb", bufs=2, name="eb")
                ebs[h] = eb
                for s in range(8):
                    p_ = ps()
                    nc.tensor.matmul(p_[:, :],
                                     kt[pb:pb + 64, j, s * 128:(s + 1) * 128],
                                     qt[pb:pb + 64, j, :], start=True, stop=True)
                    nc.scalar.activation(eb[:, s, :], p_[:, :], AF.Exp)
                    if masked and s < 4:
                        nc.vector.tensor_mul(eb[:, s, :], eb[:, s, :], sm_sb[:, s, :])

            def emit_av(h):
                j, half = h // 2, h % 2
                pb = 64 * half
                eb = ebs.pop(h)
                av = ps()
                for s in range(8):
                    nc.tensor.matmul(av[0:65, :], vv[:, s, h, :], eb[:, s, :],
                                     start=(s == 0), stop=(s == 7))
                nc.vector.tensor_copy(avo[pb:pb + 64, j, :], av[0:64, :])
                rowb = pb_.tile([65, NC_], F32, tag="rowb", bufs=2, name="rowb")
                nc.vector.tensor_copy(rowb[64:65, :], av[64:65, :])
                rowbufs[h] = rowb

            def emit_gather(h):
                rowb = rowbufs.pop(h)
                nc.sync.dma_start(out=dn16[h:h + 1, :], in_=rowb[64:65, :])

            def qkav():
                emit_qk(0)
                for h in range(H):
                    if h + 1 < H:
                        emit_qk(h + 1)
                    emit_av(h)
                    if h > 0:
                        emit_gather(h - 1)
                emit_gather(H - 1)

            def norm_tail():
                nc.vector.reciprocal(dn16[:, :], dn16[:, :])
                rb = pb_.tile([16, NC_], BF16, tag="rb", bufs=1, name="rb")
                nc.vector.tensor_copy(rb[:, :], dn16[:, :])
                for j in range(8):
                    bc = ps()
                    nc.tensor.matmul(bc[:, :], sel_sb[:, j, :], rb[:, :],
                                     start=True, stop=True)
                    nc.vector.tensor_tensor(avo[:, j, :], avo[:, j, :],
                                            bc[:, :], OP.mult)
            return qkav, norm_tail

        def out_proj2(pb_, onorm, wo_d, bias_pp, per_m=None):
            for m in range(KO):
                wom = pb_.tile([128, KO, 128], BF16, tag="wom", bufs=2, name="wom")
                nc.sync.dma_start(
                    out=wom[:],
                    in_=wo_d[m].rearrange("p (ks e) -> p ks e", e=128))
                p_ = ps()
                for ks in range(KO):
                    nc.tensor.matmul(p_[:, :], wom[:, ks, :], onorm[:, ks, :],
                                     start=(ks == 0), stop=(ks == KO - 1))
                nc.vector.scalar_tensor_tensor(xTo_sb[:, m, :], p_[:, :],
                                               bias_pp[:, m:m + 1], xTo_sb[:, m, :],
                                               OP.add, OP.add)
                if per_m is not None:
                    per_m(m)

        with tc.tile_pool(name=f"B{ibody}", bufs=1) as pb_:
            # ---- self attention ----
            ln1 = pa.tile([128, KO, T], BF16, tag="ln1", name="ln1")
            ln_v2(0, xTb_sb, 0,
                  [[xTb_sb[:, ko, 0:NC_] for ko in range(KO)],
                   [xTb_sb[:, ko, NC_:T] for ko in range(KO)]],
                  ln1, 2)
            qt = pb_.tile([128, 8, NC_], BF16, tag="qt", bufs=1, name="qt")
            proj16(pb_, ln1, W["wq"], qt, NC_)
            kt = pb_.tile([128, 8, T], BF16, tag="kt", bufs=1, name="kt")
            proj16(pb_, ln1, W["wk"], kt, T)
            vv = pb_.tile([128, 8, H, 65], BF16, tag="vv", bufs=1, name="vv")
            proj_v(pb_, ln1, W["wv"], vv, use_vmask=True)
            avo = pb_.tile([128, 8, NC_], BF16, tag="avo", bufs=1, name="avo")
            qkav, norm_tail = attention2(pb_, qt, kt, vv, avo, masked=True)
            qkav()
            # cross K projection is independent -> fills the softmax-recip tail
            ktc = pb_.tile([128, 8, T], BF16, tag="kt", bufs=1, name="ktc")
            proj16(pb_, ca_sb, W["wkc"], ktc, T)
            norm_tail()
            ln2 = pa.tile([128, KO, NC_], BF16, tag="ln2", name="ln2")
            xb2 = pa.tile([128, KO, NC_], BF16, tag="xb2", bufs=1, name="xb2")
            out_proj2(pb_, avo, W["wo"], bopp,
                      per_m=lambda m: nc.scalar.copy(out=xb2[:, m, :],
                                                     in_=xTo_sb[:, m, :]))

            # ---- cross attention ----
            ln_v2(1, xb2, 0, [[xTo_sb[:, ko, :] for ko in range(KO)]], ln2, 1)
            vvc = pb_.tile([128, 8, H, 65], BF16, tag="vv", bufs=1, name="vvc")
            proj_v(pb_, ca_sb, W["wvc"], vvc, use_vmask=False)
            qtc = pb_.tile([128, 8, NC_], BF16, tag="qt", bufs=1, name="qtc")
            proj16(pb_, ln2, W["wqc"], qtc, NC_)
            avoc = pb_.tile([128, 8, NC_], BF16, tag="avo", bufs=1, name="avoc")
            qkavc, norm_tailc = attention2(pb_, qtc, ktc, vvc, avoc, masked=False)
            qkavc()
            norm_tailc()
            xb3 = pa.tile([128, KO, NC_], BF16, tag="xb2", bufs=1, name="xb3")
            out_proj2(pb_, avoc, W["woc"], bocpp,
                      per_m=lambda m: nc.scalar.copy(out=xb3[:, m, :],
                                                     in_=xTo_sb[:, m, :]))

        # ---- FFN ----
        with tc.tile_pool(name=f"C{ibody}", bufs=1) as pc:
            ln3 = pa.tile([128, KO, NC_], BF16, tag="ln2", name="ln3")
            ln_v2(2, xb3, 0, [[xTo_sb[:, ko, :] for ko in range(KO)]], ln3, 1)
            FH = 4 * E // 128
            ht = pc.tile([128, FH, NC_], BF16, tag="ht", name="ht")
            for m in range(FH):
                w1m = pc.tile([128, KO, 128], BF16, tag="w1m", bufs=6, name="w1m")
                nc.sync.dma_start(
                    out=w1m[:],
                    in_=W["w1"][m].rearrange("p (ko f) -> p ko f", f=128))
                p_ = ps()
                for ko in range(KO):
                    nc.tensor.matmul(p_[:, :], w1m[:, ko, :], ln3[:, ko, :],
                                     start=(ko == 0), stop=(ko == KO - 1))
                nc.scalar.activation(ht[:, m, :], p_[:, :], AF.Relu,
                                     bias=b1pp[:, m:m + 1])
            for m in range(KO):
                w2m = pc.tile([128, FH, 128], BF16, tag="w2m", bufs=2, name="w2m")
                nc.sync.dma_start(
                    out=w2m[:],
                    in_=W["w2"][m].rearrange("p (ks e) -> p ks e", e=128))
                p_ = ps()
                for ks in range(FH):
                    nc.tensor.matmul(p_[:, :], w2m[:, ks, :], ht[:, ks, :],
                                     start=(ks == 0), stop=(ks == FH - 1))
                nc.vector.scalar_tensor_tensor(xTo_sb[:, m, :], p_[:, :],
                                               b2pp[:, m:m + 1], xTo_sb[:, m, :],
                                               OP.add, OP.add)
        for ko in range(KO):
            nc.sync.dma_start(out=out_xT[ko * 128:(ko + 1) * 128, :],
                              in_=xTo_sb[:, ko, :])


def _make_core_inputs2(c, inp):
    bf = ml_dtypes.bfloat16
    b, h = divmod(c, 2)
    sc = float(E) ** -0.5
    own = slice(512 * h, 512 * h + 512)
    oth = slice(512 * (1 - h), 512 * (1 - h) + 512)

    def stack_heads(w):  # [16, E, D] -> [E, 1024]
        return np.ascontiguousarray(np.transpose(w, (1, 0, 2)).reshape(E, E))

    def qkv_layout(w):  # [E, 1024] -> [2(half), 128, KO*512], contiguous DMA
        a = w.reshape(8, 128, 1024).transpose(1, 0, 2)      # [p, ko, m]
        return np.ascontiguousarray(np.stack(
            [a[:, :, :512].reshape(128, 8 * 512),
             a[:, :, 512:].reshape(128, 8 * 512)])).astype(bf)

    def mtile_layout(w, km, cm):  # [km*128, cm*128] -> [cm, 128, km*128]
        a = w.reshape(km, 128, cm, 128).transpose(2, 1, 0, 3)
        return np.ascontiguousarray(a.reshape(cm, 128, km * 128)).astype(bf)

    def pkt_layout(xT):  # [E, ncols] -> [128, KO*ncols]
        n = xT.shape[1]
        return np.ascontiguousarray(
            xT.reshape(8, 128, n).transpose(1, 0, 2).reshape(128, 8 * n))

    xt = np.asarray(inp["x"][b], np.float32)           # [T, E]
    xperm = np.concatenate([xt[own], xt[oth]], axis=0)  # keys permuted: own first

    # diagonal causal mask blocks: key local pos (128j + p) <= query local pos f
    p, f = np.arange(128)[:, None, None], np.arange(512)[None, None, :]
    jj = np.arange(4)[None, :, None]
    smask = (128 * jj + p <= f).astype(bf)

    # V-block mask: subs 0..3 = own half (visible, diag-masked); 4..7 = other
    # half: visible iff this core owns the second half (h == 1)
    vmsk = np.ones((128, 8), np.float32)
    vmsk[:, 4:] = 1.0 if h == 1 else 0.0

    # selector: sel[k, j, m] = 1 iff head (2j + m//64) == k  (softmax recip
    # broadcast: one K=16 matmul expands rows of rb to a [128,512] tile)
    kk = np.arange(16)[:, None, None]
    jj2 = np.arange(8)[None, :, None]
    mm = np.arange(128)[None, None, :]
    sel = (kk == 2 * jj2 + mm // 64).astype(bf)

    return {
        "xTb": np.ascontiguousarray(xperm.T).astype(bf),
        "xTo": pkt_layout(xt[own].T).astype(np.float32),
        "caT": pkt_layout(np.asarray(inp["ca"][b]).T).astype(bf),
        "wq": qkv_layout(stack_heads(inp["Wq_s"]) * sc),
        "wk": qkv_layout(stack_heads(inp["Wk_s"])),
        "wv": qkv_layout(stack_heads(inp["Wv_s"])),
        "wqc": qkv_layout(stack_heads(inp["Wq_c"]) * sc),
        "wkc": qkv_layout(stack_heads(inp["Wk_c"])),
        "wvc": qkv_layout(stack_heads(inp["Wv_c"])),
        "wo": mtile_layout(np.asarray(inp["Wo_s"], np.float32), 8, 8),
        "woc": mtile_layout(np.asarray(inp["Wo_c"], np.float32), 8, 8),
        "w1": mtile_layout(np.asarray(inp["W1"], np.float32), 8, 32),
        "w2": mtile_layout(np.asarray(inp["W2"], np.float32), 32, 8),
        "g1": np.asarray(inp["ln1_g"], np.float32),
        "g2": np.asarray(inp["ln2_g"], np.float32),
        "g3": np.asarray(inp["ln3_g"], np.float32),
        "be1": np.asarray(inp["ln1_b"], np.float32),
        "be2": np.asarray(inp["ln2_b"], np.float32),
        "be3": np.asarray(inp["ln3_b"], np.float32),
        "bo": np.asarray(inp["bo_s"], np.float32),
        "boc": np.asarray(inp["bo_c"], np.float32),
        "b2": np.asarray(inp["b2"], np.float32),
        "b1r": np.asarray(inp["b1"], np.float32),
        "smask": smask,
        "vmsk": vmsk,
        "seld": sel,
    }


# ------------------------------------------------------------------ host side

_CACHE = {}


COMM_FREE = True
VARIANT = "v2"   # "tp2" | "nocc" | "v2"


def _variant():
    builders = {"tp2": (build, _make_core_inputs),
                "nocc": (build_nocc, _make_core_inputs_nocc),
                "v2": (build2, _make_core_inputs2)}
    return builders[VARIANT]


def _get_runner(nbody=1):
    key = (nbody, VARIANT)
    if key in _CACHE:
        return _CACHE[key]
    import jax
    from jax.sharding import Mesh, PartitionSpec
    from jax.experimental.shard_map import shard_map
    from concourse.bass2jax import (_bass_exec_p, install_neuronx_cc_hook,
                                    partition_id_tensor)

    nc = _variant()[0](nbody)
    install_neuronx_cc_hook()
    pn = nc.partition_id_tensor.name if nc.partition_id_tensor else None
    in_names, out_names, out_avals = [], [], []
    for alloc in nc.m.functions[0].allocations:
        if not isinstance(alloc, mybir.MemoryLocationSet):
            continue
        name = alloc.memorylocations[0].name
        if alloc.kind == "ExternalInput":
            if name != pn:
                in_names.append(name)
        elif alloc.kind == "ExternalOutput":
            out_names.append(name)
            out_avals.append(jax.core.ShapedArray(
                tuple(alloc.tensor_shape), mybir.dt.np(alloc.dtype)))
    n_params = len(in_names)
    all_in = in_names + out_names + ([pn] if pn else [])

    def _jbody(*args):
        ops = list(args)
        if pn:
            ops.append(partition_id_tensor())
        return tuple(_bass_exec_p.bind(
            *ops, out_avals=tuple(out_avals), in_names=tuple(all_in),
            out_names=tuple(out_names), lowering_input_output_aliases=(),
            sim_require_finite=True, sim_require_nnan=True, nc=nc))

    devices = jax.devices()[:8]
    mesh = Mesh(np.asarray(devices), ("core",))
    spec = (PartitionSpec("core"),)
    fn = jax.jit(shard_map(_jbody, mesh=mesh,
                           in_specs=spec * (n_params + len(out_names)),
                           out_specs=spec * len(out_names), check_rep=False),
                 keep_unused=True)
    _CACHE[key] = (fn, in_names, out_names, out_avals)
    return _CACHE[key]


def _make_core_inputs(c, inp):
    bf = ml_dtypes.bfloat16
    b, r = divmod(c, 2)
    hs = slice(8 * r, 8 * r + 8)
    sc = float(E) ** -0.5

    def stack_heads(w):  # [8, E, D] -> [E, 512]
        return np.ascontiguousarray(np.transpose(w, (1, 0, 2)).reshape(E, 512))

    p, f = np.arange(128)[:, None, None], np.arange(512)[None, None, :]
    jj = np.arange(4)[None, :, None]
    cmask = (f >= 128 * jj + p).astype(bf)

    return {
        "xT": np.ascontiguousarray(inp["x"][b].T).astype(np.float32),
        "caT": np.ascontiguousarray(inp["ca"][b].T).astype(bf),
        "wq": (stack_heads(inp["Wq_s"][hs]) * sc).astype(bf),
        "wk": stack_heads(inp["Wk_s"][hs]).astype(bf),
        "wv": stack_heads(inp["Wv_s"][hs]).astype(bf),
        "wqc": (stack_heads(inp["Wq_c"][hs]) * sc).astype(bf),
        "wkc": stack_heads(inp["Wk_c"][hs]).astype(bf),
        "wvc": stack_heads(inp["Wv_c"][hs]).astype(bf),
        "wo": np.ascontiguousarray(inp["Wo_s"][512 * r:512 * (r + 1), :]).astype(bf),
        "woc": np.ascontiguousarray(inp["Wo_c"][512 * r:512 * (r + 1), :]).astype(bf),
        "w1": np.ascontiguousarray(inp["W1"][:, FF * r:FF * (r + 1)]).astype(bf),
        "w2": np.ascontiguousarray(inp["W2"][FF * r:FF * (r + 1), :]).astype(bf),
        "gb1": np.stack([inp["ln1_g"], inp["ln1_b"]]).astype(bf),
        "gb2": np.stack([inp["ln2_g"], inp["ln2_b"]]).astype(bf),
        "gb3": np.stack([inp["ln3_g"], inp["ln3_b"]]).astype(bf),
        "g1": np.asarray(inp["ln1_g"], np.float32),
        "g2": np.asarray(inp["ln2_g"], np.float32),
        "g3": np.asarray(inp["ln3_g"], np.float32),
        "bo2": np.asarray(inp["bo_s"], np.float32) * 0.5,
        "bo2c": np.asarray(inp["bo_c"], np.float32) * 0.5,
        "b22": np.asarray(inp["b2"], np.float32) * 0.5,
        "b1r": np.asarray(inp["b1"][FF * r:FF * (r + 1)], np.float32),
        "cmask": cmask,
    }


def _run(nbody, in_maps, dev_inputs=None, dev_zeros=None, download=True):
    import jax
    fn, in_names, out_names, out_avals = _get_runner(nbody)
    if dev_inputs is None:
        concat = [np.concatenate([np.asarray(in_maps[c][n]) for c in range(8)], axis=0)
                  for n in in_names]
        dev_inputs = [jax.device_put(a) for a in concat]
    if dev_zeros is None:
        dev_zeros = [jax.device_put(np.zeros((8 * a.shape[0], *a.shape[1:]), a.dtype))
                     for a in out_avals]
    outs = fn(*dev_inputs, *dev_zeros)
    for o in outs:
        o.block_until_ready()
    if not download:
        return None, (dev_inputs, dev_zeros)
    res = []
    for c in range(8):
        res.append({n: np.asarray(outs[i]).reshape(8, *out_avals[i].shape)[c]
                    for i, n in enumerate(out_names)})
    return res, (dev_inputs, dev_zeros)


def kernel(**inputs):
    inp = {k: np.asarray(v) for k, v in inputs.items()}
    mk = _variant()[1]
    in_maps = [mk(c, inp) for c in range(8)]
    res, _ = _run(1, in_maps)
    if VARIANT in ("nocc", "v2"):
        out = np.stack([
            np.concatenate([res[2 * b]["out_xT"], res[2 * b + 1]["out_xT"]],
                           axis=1).T
            for b in range(B)]).astype(np.float32)
    else:
        out = np.stack([res[2 * b]["out_xT"].T for b in range(B)]).astype(np.float32)
    return out


# ---------------------------------------------------------------- comm-free

def build_nocc(nbody=1):
    """Communication-free sharding: core = (batch b, T-half h).  Each core
    computes its 512 query tokens for ALL 16 heads and the full FFN, with
    K/V duplicated across the pair.  Self-attn keys are permuted so the own
    half always sits at key positions 0..511 (the per-core causal mask input
    encodes the permutation) — keeps the SPMD program identical on all cores.
    """
    nc = bacc.Bacc(num_devices=8)

    def P(name, shape, dt):
        return nc.declare_dram_parameter(name, shape, dt, isOutput=False)

    # all weight/activation layouts are host-pretransposed so every DMA is
    # contiguous per partition (descriptor-bound strided gathers killed ~180us
    # per DMA engine in the naive [E, .] layouts)
    xTb = P("xTb", [E, T], BF16)        # permuted x^T, bf16 (LN1 / self K,V)
    xTo = P("xTo", [128, KO * NC_], F32)   # own-half x^T  [p, ko*t]
    caT = P("caT", [128, KO * S], BF16)    # ca^T           [p, ko*t]
    wq, wk, wv = P("wq", [2, 128, KO * 512], BF16), P("wk", [2, 128, KO * 512], BF16), P("wv", [2, 128, KO * 512], BF16)
    wqc, wkc, wvc = P("wqc", [2, 128, KO * 512], BF16), P("wkc", [2, 128, KO * 512], BF16), P("wvc", [2, 128, KO * 512], BF16)
    wo, woc = P("wo", [KO, 128, E], BF16), P("woc", [KO, 128, E], BF16)
    w1 = P("w1", [4 * E // 128, 128, KO * 128], BF16)
    w2 = P("w2", [KO, 128, (4 * E // 128) * 128], BF16)
    gb = [P(f"gb{i}", [2, E], BF16) for i in (1, 2, 3)]
    gpp_d = [P(f"g{i}", [E], F32) for i in (1, 2, 3)]
    bo_, boc_, b2_ = P("bo", [E], F32), P("boc", [E], F32), P("b2", [E], F32)
    b1r = P("b1r", [4 * E], F32)
    smask = P("smask", [128, 8, NC_], BF16)
    out_xT = nc.declare_dram_parameter("out_xT", [E, NC_], F32, isOutput=True)

    with tile.TileContext(nc) as tc:
        with tc.tile_pool(name="persist", bufs=1) as pp:
            xTb_sb = pp.tile([128, KO, T], BF16, tag="xTb")
            for ko in range(KO):
                nc.sync.dma_start(out=xTb_sb[:, ko, :], in_=xTb[ko * 128:(ko + 1) * 128, :])
            xTo_sb = pp.tile([128, KO, NC_], F32, tag="xTo")
            nc.sync.dma_start(out=xTo_sb[:], in_=xTo.rearrange("(ko p) t -> p ko t", p=128))
            ca_sb = pp.tile([128, KO, S], BF16, tag="ca")
            nc.sync.dma_start(out=ca_sb[:], in_=caT.rearrange("(ko p) t -> p ko t", p=128))
            sm_sb = pp.tile([128, 8, NC_], BF16, tag="sm")
            nc.sync.dma_start(out=sm_sb[:], in_=smask[:])
            ones_bf = pp.tile([128, 512], BF16, tag="ones")
            nc.vector.memset(ones_bf[:], 1.0)
            gl_sb, bl_sb, gpp = [], [], []
            for i in range(3):
                ta = pp.tile([1, KO, 128], BF16, tag=f"gl{i}")
                nc.sync.dma_start(out=ta[:], in_=gb[i].rearrange("a (ko m) -> a ko m", m=128)[0:1])
                gl_sb.append(ta)
                tb = pp.tile([1, KO, 128], BF16, tag=f"bl{i}")
                nc.sync.dma_start(out=tb[:], in_=gb[i].rearrange("a (ko m) -> a ko m", m=128)[1:2])
                bl_sb.append(tb)
                t2 = pp.tile([128, KO], F32, tag=f"gpp{i}")
                with nc.allow_non_contiguous_dma(reason="tiny LN vector"):
                    nc.sync.dma_start(out=t2[:], in_=gpp_d[i].rearrange("(ko p) -> p ko", p=128))
                gpp.append(t2)
            bpp = []
            for nm, d in (("bo", bo_), ("boc", boc_), ("b2", b2_)):
                t_ = pp.tile([128, KO], F32, tag=nm)
                with nc.allow_non_contiguous_dma(reason="tiny bias vector"):
                    nc.scalar.dma_start(out=t_[:], in_=d.rearrange("(ko p) -> p ko", p=128))
                bpp.append(t_)
            eps_t = pp.tile([1, 1], F32, tag="eps")
            nc.vector.memset(eps_t[:], 1e-5)
            b1pp = pp.tile([128, 4 * E // 128], F32, tag="b1")
            with nc.allow_non_contiguous_dma(reason="tiny bias vector"):
                nc.scalar.dma_start(out=b1pp[:], in_=b1r.rearrange("(m p) -> p m", p=128))

            for ibody in range(nbody):
                _body_nocc(nc, tc, ibody, xTb_sb, xTo_sb, ca_sb, sm_sb, ones_bf,
                           (gl_sb, bl_sb), gpp, bpp, b1pp, eps_t,
                           dict(wq=wq, wk=wk, wv=wv, wqc=wqc, wkc=wkc, wvc=wvc,
                                wo=wo, woc=woc, w1=w1, w2=w2, xTo=xTo),
                           out_xT)
    nc.finalize()
    return nc


def _body_nocc(nc, tc, ibody, xTb_sb, xTo_sb, ca_sb, sm_sb, ones_bf, gbl, gpp,
               bpp, b1pp, eps_t, W, out_xT):
    gl_sb, bl_sb = gbl
    bopp, bocpp, b2pp = bpp

    if ibody > 0:
        nc.sync.dma_start(out=xTo_sb[:],
                          in_=W["xTo"].rearrange("p (ko t) -> p ko t", t=NC_))

    with tc.tile_pool(name=f"A{ibody}", bufs=1) as pa, \
         tc.tile_pool(name=f"ps{ibody}", bufs=8, space="PSUM") as pspool:

        pb2_ref = [None]

        def ps():
            return pspool.tile([128, NC_], F32, tag="ps", name="ps")

        def ln_rows(i, ps1, ps2, cs_out, ln, src, src_is_bf, gsl, ncols):
            m_ = pa.tile([1, NC_], F32, tag="row_m", bufs=1, name="m_")
            nc.vector.tensor_scalar_mul(m_[:, :ncols], ps1[0:1, :ncols], 1.0 / E)
            msq = pa.tile([1, NC_], F32, tag="row_q", bufs=1, name="msq")
            nc.vector.tensor_mul(msq[:, :ncols], m_[:, :ncols], m_[:, :ncols])
            var = pa.tile([1, NC_], F32, tag="row_v", bufs=1, name="var")
            nc.vector.scalar_tensor_tensor(var[:, :ncols], ps2[0:1, :ncols], 1.0 / E,
                                           msq[:, :ncols], OP.mult, OP.subtract)
            sqv = pa.tile([1, NC_], F32, tag="row_s", bufs=1, name="sqv")
            nc.scalar.activation(sqv[:, :ncols], var[:, :ncols], AF.Sqrt, bias=eps_t[:])
            rstd = pa.tile([1, NC_], F32, tag="row_r", bufs=1, name="rstd")
            nc.vector.reciprocal(rstd[:, :ncols], sqv[:, :ncols])
            rbf = pa.tile([1, NC_], BF16, tag="rowsb2", bufs=1, name="rbf")
            nc.vector.tensor_copy(rbf[:, :ncols], rstd[:, :ncols])
            nmr = pa.tile([1, NC_], BF16, tag="rowsb1", bufs=1, name="nmr")
            nc.vector.scalar_tensor_tensor(nmr[:, :ncols], m_[:, :ncols], -1.0,
                                           rstd[:, :ncols], OP.mult, OP.mult)
            rbc = ps()
            nc.tensor.matmul(rbc[:, :ncols], ones_bf[0:1, 0:128], rbf[:, :ncols],
                             start=True, stop=True)
            for ko in range(KO):
                bbc = ps()
                nc.tensor.matmul(bbc[:, :ncols], gl_sb[i][:, ko, :], nmr[:, :ncols],
                                 start=True, stop=False)
                nc.tensor.matmul(bbc[:, :ncols], bl_sb[i][:, ko, :],
                                 ones_bf[0:1, :ncols], start=False, stop=True)
                t0 = pa.tile([128, NC_], F32, tag="tmp", bufs=2, name="t0")
                nc.vector.scalar_tensor_tensor(t0[:, :ncols], src[ko],
                                               gpp[i][:, ko:ko + 1], rbc[:, :ncols],
                                               OP.mult, OP.mult)
                nc.vector.tensor_tensor(ln[:, ko, cs_out], t0[:, :ncols],
                                        bbc[:, :ncols], OP.add)

        def layer_norm1():
            """full-T LN over xTb (bf16 source)."""
            ln = pa.tile([128, KO, T], BF16, tag="lnf", name="lnf")
            for c in range(CC):
                cs = slice(c * NC_, (c + 1) * NC_)
                sq = pa.tile([128, KO, NC_], BF16, tag="stat", bufs=2, name="sq")
                nc.scalar.activation(sq[:], xTb_sb[:, :, cs], AF.Square)
                ps1, ps2 = ps(), ps()
                for ko in range(KO):
                    nc.tensor.matmul(ps1[0:1, :], ones_bf[:, 0:1], xTb_sb[:, ko, cs],
                                     start=(ko == 0), stop=(ko == KO - 1))
                for ko in range(KO):
                    nc.tensor.matmul(ps2[0:1, :], ones_bf[:, 0:1], sq[:, ko, :],
                                     start=(ko == 0), stop=(ko == KO - 1))
                ln_rows(0, ps1, ps2, cs, ln,
                        [xTb_sb[:, ko, cs] for ko in range(KO)], True, None, NC_)
            return ln

        def layer_norm_h(i):
            """own-half LN over xTo (f32 residual)."""
            ln = pa.tile([128, KO, NC_], BF16, tag="lnh", bufs=1, name="lnh")
            xb = pa.tile([128, KO, NC_], BF16, tag="stat", bufs=2, name="xb")
            for ko in range(KO):
                nc.scalar.copy(out=xb[:, ko, :], in_=xTo_sb[:, ko, :])
            sq = pa.tile([128, KO, NC_], BF16, tag="stat", bufs=2, name="sq")
            nc.scalar.activation(sq[:], xb[:], AF.Square)
            ps1, ps2 = ps(), ps()
            for ko in range(KO):
                nc.tensor.matmul(ps1[0:1, :], ones_bf[:, 0:1], xb[:, ko, :],
                                 start=(ko == 0), stop=(ko == KO - 1))
            for ko in range(KO):
                nc.tensor.matmul(ps2[0:1, :], ones_bf[:, 0:1], sq[:, ko, :],
                                 start=(ko == 0), stop=(ko == KO - 1))
            ln_rows(i, ps1, ps2, slice(0, NC_), ln,
                    [xTo_sb[:, ko, :] for ko in range(KO)], False, None, NC_)
            return ln

        def project_qt(lnsrc, w_d, cols):
            """Q^T for 16 heads over `cols` own tokens -> [128, 8, 512]."""
            qt = pb2_ref[0].tile([128, 8, NC_], BF16, tag="qon", bufs=2, name="qt")
            for jh in range(2):          # stream wq in halves of 512 cols
                w_sb = pb2_ref[0].tile([128, KO, 512], BF16, tag="wqkv", bufs=1, name="wsb")
                nc.sync.dma_start(
                    out=w_sb[:],
                    in_=w_d.rearrange("(ko p) m -> p ko m", p=128)[:, :, jh * 512:(jh + 1) * 512])
                for jj in range(4):
                    j = jh * 4 + jj
                    p_ = ps()
                    for ko in range(KO):
                        nc.tensor.matmul(p_[:, :], w_sb[:, ko, jj * 128:(jj + 1) * 128],
                                         lnsrc[ko], start=(ko == 0), stop=(ko == KO - 1))
                    nc.vector.tensor_copy(qt[:, j, :], p_[:, :])
            return qt

        def project_kt(src, w_d):
            """K^T for 16 heads over full S -> [128, 8, 1024]."""
            kt = pb2_ref[0].tile([128, 8, T], BF16, tag="kt", name="kt")
            for jh in range(2):
                w_sb = pb2_ref[0].tile([128, KO, 512], BF16, tag="wqkv", bufs=1, name="wsb")
                nc.sync.dma_start(
                    out=w_sb[:],
                    in_=w_d.rearrange("(ko p) m -> p ko m", p=128)[:, :, jh * 512:(jh + 1) * 512])
                for jj in range(4):
                    j = jh * 4 + jj
                    for c in range(CC):
                        p_ = ps()
                        for ko in range(KO):
                            nc.tensor.matmul(p_[:, :], w_sb[:, ko, jj * 128:(jj + 1) * 128],
                                             src[:, ko, c * NC_:(c + 1) * NC_],
                                             start=(ko == 0), stop=(ko == KO - 1))
                        nc.vector.tensor_copy(kt[:, j, c * NC_:(c + 1) * NC_], p_[:, :])
            return kt

        def project_v(src, w_d):
            """V for 16 heads -> [128, 8, 16, 65]."""
            vv = pb2_ref[0].tile([128, 8, H, 65], BF16, tag="vv", name="vv")
            for jh in range(2):
                w_sb = pb2_ref[0].tile([128, KO, 512], BF16, tag="wqkv", bufs=1, name="wsb")
                nc.sync.dma_start(
                    out=w_sb[:],
                    in_=w_d.rearrange("(ko p) m -> p ko m", p=128)[:, :, jh * 512:(jh + 1) * 512])
                for s in range(8):
                    p_ = ps()
                    for ko in range(KO):
                        nc.tensor.matmul(p_[:, :], src[:, ko, s * 128:(s + 1) * 128],
                                         w_sb[:, ko, :], start=(ko == 0), stop=(ko == KO - 1))
                    nc.scalar.copy(out=vv[:, s, jh * 8:(jh + 1) * 8, 0:64],
                                   in_=p_[:, :].rearrange("p (h d) -> p h d", d=64))
                    nc.vector.memset(vv[:, s, jh * 8:(jh + 1) * 8, 64:65], 1.0)
            return vv

        def attention(qt, kt, vv, onorm, masked):
            for h_ in range(H):
                j, half = h_ // 2, h_ % 2
                pb = 64 * half
                eb = pb2_ref[0].tile([128, 8, NC_], BF16, tag="expb", bufs=1, name="eb")
                for s_ in range(8):
                    p_ = ps()
                    nc.tensor.matmul(p_[:, :],
                                     kt[pb:pb + 64, j, s_ * 128:(s_ + 1) * 128],
                                     qt[pb:pb + 64, j, :], start=True, stop=True)
                    nc.scalar.activation(eb[:, s_, :], p_[:, :], AF.Exp)
                    if masked:
                        nc.vector.tensor_mul(eb[:, s_, :], eb[:, s_, :],
                                             sm_sb[:, s_, :])
                av = ps()
                for s_ in range(8):
                    nc.tensor.matmul(av[0:65, :], vv[:, s_, h_, :], eb[:, s_, :],
                                     start=(s_ == 0), stop=(s_ == 7))
                rr = pb2_ref[0].tile([65, NC_], F32, tag="row_rr", bufs=1, name="rr")
                nc.vector.reciprocal(rr[64:65, :], av[64:65, :])
                rb = pb2_ref[0].tile([65, NC_], BF16, tag="row_rrb", bufs=1, name="rb")
                nc.vector.tensor_copy(rb[64:65, :], rr[64:65, :])
                bc = ps()
                nc.tensor.matmul(bc[0:64, :], ones_bf[64:65, 0:64], rb[64:65, :],
                                 start=True, stop=True)
                bcs = pb2_ref[0].tile([64, NC_], BF16, tag="bcs", bufs=2, name="bcs")
                nc.vector.tensor_copy(bcs[:, :], bc[0:64, :])
                nc.vector.tensor_tensor(onorm[pb:pb + 64, j, :],
                                        av[0:64, :], bcs[:, :], OP.mult)

        def out_proj(onorm, wo_d, bias_pp):
            for m in range(KO):
                wom = pb2_ref[0].tile([128, KO, 128], BF16, tag="wom", bufs=4, name="wom")
                nc.sync.dma_start(
                    out=wom[:],
                    in_=wo_d.rearrange("(ks p) e -> p ks e", p=128)[:, :, m * 128:(m + 1) * 128])
                p_ = ps()
                for ks in range(KO):
                    nc.tensor.matmul(p_[:, :], wom[:, ks, :], onorm[:, ks, :],
                                     start=(ks == 0), stop=(ks == KO - 1))
                nc.vector.scalar_tensor_tensor(xTo_sb[:, m, :], p_[:, :],
                                               bias_pp[:, m:m + 1], xTo_sb[:, m, :],
                                               OP.add, OP.add)

        with tc.tile_pool(name=f"B{ibody}", bufs=1) as _pb2:
            pb2_ref[0] = _pb2
            # ---- self attention ----
            ln1 = layer_norm1()
            qt = project_qt([ln1[:, ko, 0:NC_] for ko in range(KO)], W["wq"], NC_)
            kt = project_kt(ln1, W["wk"])
            vv = project_v(ln1, W["wv"])
            on1 = _pb2.tile([128, 8, NC_], BF16, tag="qon", bufs=2, name="on1")
            attention(qt, kt, vv, on1, masked=True)
            out_proj(on1, W["wo"], bopp)

            # ---- cross attention ----
            ln2 = layer_norm_h(1)
            qtc = project_qt([ln2[:, ko, :] for ko in range(KO)], W["wqc"], NC_)
            ktc = project_kt(ca_sb, W["wkc"])
            vvc = project_v(ca_sb, W["wvc"])
            on2 = _pb2.tile([128, 8, NC_], BF16, tag="qon", bufs=2, name="on2")
            attention(qtc, ktc, vvc, on2, masked=False)
            out_proj(on2, W["woc"], bocpp)

        # ---- FFN ----
        ln3 = layer_norm_h(2)
        with tc.tile_pool(name=f"C{ibody}", bufs=1) as pc:
            FH = 4 * E // 128
            ht = pc.tile([128, FH, NC_], BF16, tag="ht", name="ht")
            for m in range(FH):
                w1m = pc.tile([128, KO, 128], BF16, tag="w1m", bufs=6, name="w1m")
                nc.sync.dma_start(
                    out=w1m[:],
                    in_=W["w1"][m].rearrange("p (ko f) -> p ko f", f=128))
                p_ = ps()
                for ko in range(KO):
                    nc.tensor.matmul(p_[:, :], w1m[:, ko, :], ln3[:, ko, :],
                                     start=(ko == 0), stop=(ko == KO - 1))
                nc.scalar.activation(ht[:, m, :], p_[:, :], AF.Relu,
                                     bias=b1pp[:, m:m + 1])
            for m in range(KO):
                w2m = pc.tile([128, FH, 128], BF16, tag="w2m", bufs=2, name="w2m")
                nc.sync.dma_start(
                    out=w2m[:],
                    in_=W["w2"][m].rearrange("p (ks e) -> p ks e", e=128))
                p_ = ps()
                for ks in range(FH):
                    nc.tensor.matmul(p_[:, :], w2m[:, ks, :], ht[:, ks, :],
                                     start=(ks == 0), stop=(ks == FH - 1))
                nc.vector.scalar_tensor_tensor(xTo_sb[:, m, :], p_[:, :],
                                               b2pp[:, m:m + 1], xTo_sb[:, m, :],
                                               OP.add, OP.add)
        for ko in range(KO):
            nc.sync.dma_start(out=out_xT[ko * 128:(ko + 1) * 128, :],
                              in_=xTo_sb[:, ko, :])


def _make_core_inputs_nocc(c, inp):
    bf = ml_dtypes.bfloat16
    b, h = divmod(c, 2)
    sc = float(E) ** -0.5
    own = slice(512 * h, 512 * h + 512)
    oth = slice(512 * (1 - h), 512 * (1 - h) + 512)

    def stack_heads(w):  # [16, E, D] -> [E, 1024]
        return np.ascontiguousarray(np.transpose(w, (1, 0, 2)).reshape(E, E))

    def qkv_layout(w):  # [E, 1024] -> [2(half), 128, KO*512], contiguous DMA
        a = w.reshape(8, 128, 1024).transpose(1, 0, 2)      # [p, ko, m]
        return np.ascontiguousarray(np.stack(
            [a[:, :, :512].reshape(128, 8 * 512),
             a[:, :, 512:].reshape(128, 8 * 512)])).astype(bf)

    def mtile_layout(w, km, cm):  # [km*128, cm*128] -> [cm, 128, km*128]
        a = w.reshape(km, 128, cm, 128).transpose(2, 1, 0, 3)
        return np.ascontiguousarray(a.reshape(cm, 128, km * 128)).astype(bf)

    def pkt_layout(xT):  # [E, ncols] -> [128, KO*ncols]
        n = xT.shape[1]
        return np.ascontiguousarray(
            xT.reshape(8, 128, n).transpose(1, 0, 2).reshape(128, 8 * n))

    xt = np.asarray(inp["x"][b], np.float32)           # [T, E]
    xperm = np.concatenate([xt[own], xt[oth]], axis=0)  # keys permuted: own first
    # causal mask in permuted key order, own queries t = 512h + f
    s_perm = np.arange(1024)
    s_glob = np.where(s_perm < 512, s_perm + 512 * h, s_perm - 512 * h)
    f = np.arange(512)
    mask = (s_glob[:, None] <= (512 * h + f)[None, :])  # [1024, 512]
    smask = mask.reshape(8, 128, 512).transpose(1, 0, 2).astype(bf)

    return {
        "xTb": np.ascontiguousarray(xperm.T).astype(bf),
        "xTo": pkt_layout(xt[own].T).astype(np.float32),
        "caT": pkt_layout(np.asarray(inp["ca"][b]).T).astype(bf),
        "wq": qkv_layout(stack_heads(inp["Wq_s"]) * sc),
        "wk": qkv_layout(stack_heads(inp["Wk_s"])),
        "wv": qkv_layout(stack_heads(inp["Wv_s"])),
        "wqc": qkv_layout(stack_heads(inp["Wq_c"]) * sc),
        "wkc": qkv_layout(stack_heads(inp["Wk_c"])),
        "wvc": qkv_layout(stack_heads(inp["Wv_c"])),
        "wo": mtile_layout(np.asarray(inp["Wo_s"], np.float32), 8, 8),
        "woc": mtile_layout(np.asarray(inp["Wo_c"], np.float32), 8, 8),
        "w1": mtile_layout(np.asarray(inp["W1"], np.float32), 8, 32),
        "w2": mtile_layout(np.asarray(inp["W2"], np.float32), 32, 8),
        "gb1": np.stack([inp["ln1_g"], inp["ln1_b"]]).astype(bf),
        "gb2": np.stack([inp["ln2_g"], inp["ln2_b"]]).astype(bf),
        "gb3": np.stack([inp["ln3_g"], inp["ln3_b"]]).astype(bf),
        "g1": np.asarray(inp["ln1_g"], np.float32),
        "g2": np.asarray(inp["ln2_g"], np.float32),
        "g3": np.asarray(inp["ln3_g"], np.float32),
        "bo": np.asarray(inp["bo_s"], np.float32),
        "boc": np.asarray(inp["bo_c"], np.float32),
        "b2": np.asarray(inp["b2"], np.float32),
        "b1r": np.asarray(inp["b1"], np.float32),
        "smask": smask,
    }



# revision 31
# speedup vs baseline: 1.0400x; 1.0210x over previous
"""Trainium2 Bass kernel for nn_DecoderBlock (B=4,T=S=E=1024,H=16,D=64) on 8 cores.

Active variant ("v2", see VARIANT below): communication-free sharding,
core = (batch b, T-half h).  Each core computes its 512 query tokens for all
16 heads plus the full FFN; K/V work is duplicated across the pair so no
collective is ever issued (pairwise AllReduce on this fleet is slow and
jittery).  Self-attn keys are permuted own-half-first so the SPMD program is
identical on every core; the causal structure is handled by a diagonal-block
multiplicative mask plus zeroing V (and its denominator ones-row) for key
blocks invisible to the whole core.

The residual stream lives transposed on-chip as x^T [E(partitions), T(free)]
so every matmul is already in lhsT/rhs layout.  LayerNorm statistics are
computed with an all-ones [128x128] stationary matmul so mean/rstd are
broadcast to all partitions and the row math runs 128 lanes wide; rstd uses
sqrt + vector.reciprocal (reciprocal_approx_fast miscompiles on HW).
Attention is software-pipelined per head (QK of head h+1 issues before AV of
head h); softmax denominators ride as a 65th V row, are gathered to a [16,512]
tile with tiny SBUF->SBUF DMAs, inverted by one batched reciprocal, and
broadcast back through a constant selector-matrix matmul.

Older variants ("tp2" = DP4 x TP2 with pairwise AllReduce, "nocc" = first
comm-free version) are kept for reference and A/B timing.
"""
import sys

sys.path.insert(0, "/opt/trn_rl_repo")

import numpy as np
import ml_dtypes

import concourse.bass as bass
import concourse.bacc as bacc
import concourse.mybir as mybir
import concourse.tile as tile

BF16 = mybir.dt.bfloat16
F32 = mybir.dt.float32
AF = mybir.ActivationFunctionType
OP = mybir.AluOpType

B, T, S, E, H, D = 4, 1024, 1024, 1024, 16, 64
HL = H // 2          # heads per core (TP-2)
FF = 4 * E // 2      # ffn hidden per core
KO = E // 128        # 8 partition subtiles of E
NC_ = 512            # matmul free-dim chunk
CC = T // NC_        # 2 chunks over T
PAIRS = [[0, 1], [2, 3], [4, 5], [6, 7]]


SKIP_CC = False


def build(nbody=1):
    nc = bacc.Bacc(num_devices=8)

    def P(name, shape, dt):
        return nc.declare_dram_parameter(name, shape, dt, isOutput=False)

    xT = P("xT", [E, T], F32)
    caT = P("caT", [E, S], BF16)
    wq, wk, wv = P("wq", [E, 512], BF16), P("wk", [E, 512], BF16), P("wv", [E, 512], BF16)
    wqc, wkc, wvc = P("wqc", [E, 512], BF16), P("wkc", [E, 512], BF16), P("wvc", [E, 512], BF16)
    wo, woc = P("wo", [512, E], BF16), P("woc", [512, E], BF16)
    w1, w2 = P("w1", [E, FF], BF16), P("w2", [FF, E], BF16)
    gb = [P(f"gb{i}", [2, E], BF16) for i in (1, 2, 3)]
    gpp_d = [P(f"g{i}", [E], F32) for i in (1, 2, 3)]
    bo2, bo2c, b22 = P("bo2", [E], F32), P("bo2c", [E], F32), P("b22", [E], F32)
    b1r = P("b1r", [FF], F32)
    cmask = P("cmask", [128, 4, 512], BF16)
    out_xT = nc.declare_dram_parameter("out_xT", [E, T], F32, isOutput=True)

    with tile.TileContext(nc) as tc:
        with tc.tile_pool(name="persist", bufs=1) as pp:
            xT_sb = pp.tile([128, KO, T], F32, tag="xT")
            for ko in range(KO):
                nc.sync.dma_start(out=xT_sb[:, ko, :],
                                  in_=xT[ko * 128:(ko + 1) * 128, :])
            ca_sb = pp.tile([128, KO, S], BF16, tag="ca")
            nc.sync.dma_start(out=ca_sb[:], in_=caT.rearrange("(ko p) t -> p ko t", p=128))
            cm_sb = pp.tile([128, 4, 512], BF16, tag="cm")
            nc.sync.dma_start(out=cm_sb[:], in_=cmask[:])
            ones_bf = pp.tile([128, 512], BF16, tag="ones")
            nc.vector.memset(ones_bf[:], 1.0)
            gl_sb, bl_sb, gpp = [], [], []
            for i in range(3):
                ta = pp.tile([1, KO, 128], BF16, tag=f"gl{i}")
                nc.sync.dma_start(out=ta[:], in_=gb[i].rearrange("a (ko m) -> a ko m", m=128)[0:1])
                gl_sb.append(ta)
                tb = pp.tile([1, KO, 128], BF16, tag=f"bl{i}")
                nc.sync.dma_start(out=tb[:], in_=gb[i].rearrange("a (ko m) -> a ko m", m=128)[1:2])
                bl_sb.append(tb)
                t2 = pp.tile([128, KO], F32, tag=f"gpp{i}")
                with nc.allow_non_contiguous_dma(reason="tiny LN vector"):
                    nc.sync.dma_start(out=t2[:], in_=gpp_d[i].rearrange("(ko p) -> p ko", p=128))
                gpp.append(t2)
            bpp = []
            for nm, d in (("bo2", bo2), ("bo2c", bo2c), ("b22", b22)):
                t_ = pp.tile([128, KO], F32, tag=nm)
                with nc.allow_non_contiguous_dma(reason="tiny bias vector"):
                    nc.scalar.dma_start(out=t_[:], in_=d.rearrange("(ko p) -> p ko", p=128))
                bpp.append(t_)
            eps_t = pp.tile([1, 1], F32, tag="eps")
            nc.vector.memset(eps_t[:], 1e-5)
            b1pp = pp.tile([128, FF // 128], F32, tag="b1")
            with nc.allow_non_contiguous_dma(reason="tiny bias vector"):
                nc.sync.dma_start(out=b1pp[:], in_=b1r.rearrange("(m p) -> p m", p=128))

            for ibody in range(nbody):
                _body(nc, tc, ibody, xT_sb, ca_sb, cm_sb, ones_bf, (gl_sb, bl_sb), gpp,
                      bpp, b1pp, eps_t,
                      dict(wq=wq, wk=wk, wv=wv, wqc=wqc, wkc=wkc, wvc=wvc,
                           wo=wo, woc=woc, w1=w1, w2=w2, xT=xT),
                      out_xT)
    nc.finalize()
    return nc


def _body(nc, tc, ibody, xT_sb, ca_sb, cm_sb, ones_bf, gbl, gpp,
          bpp, b1pp, eps_t, W, out_xT):
    gl_sb, bl_sb = gbl
    bo2pp, bo2cpp, b22pp = bpp
    ar = {}
    for k in (1, 2, 3):
        ar[k] = [(nc.dram_tensor(f"ar{k}_{ibody}_{c}_in", [E, NC_], F32),
                  nc.dram_tensor(f"ar{k}_{ibody}_{c}_out", [E, NC_], F32))
                 for c in range(CC)]

    if ibody > 0:
        # re-load pristine x for the timing replica
        for ko in range(KO):
            nc.sync.dma_start(out=xT_sb[:, ko, :],
                              in_=W["xT"][ko * 128:(ko + 1) * 128, :])

    with tc.tile_pool(name=f"A{ibody}", bufs=1) as pa, \
         tc.tile_pool(name=f"ps{ibody}", bufs=8, space="PSUM") as pspool:

        def ps():
            return pspool.tile([128, NC_], F32, tag="ps", name="ps")

        def layer_norm(i):
            """LN over partitions of xT_sb -> bf16 tile [128, KO, T]."""
            ln = pa.tile([128, KO, T], BF16, tag="lnout", name="ln")
            for c in range(CC):
                cs = slice(c * NC_, (c + 1) * NC_)
                xb = pa.tile([128, KO, NC_], BF16, tag="stat", bufs=2, name="xb")
                for ko in range(KO):
                    nc.scalar.copy(out=xb[:, ko, :], in_=xT_sb[:, ko, cs])
                sq = pa.tile([128, KO, NC_], BF16, tag="stat", bufs=2, name="sq")
                nc.scalar.activation(sq[:], xb[:], AF.Square)
                ps1, ps2 = ps(), ps()
                for ko in range(KO):
                    nc.tensor.matmul(ps1[0:1, :], ones_bf[:, 0:1], xb[:, ko, :],
                                     start=(ko == 0), stop=(ko == KO - 1))
                for ko in range(KO):
                    nc.tensor.matmul(ps2[0:1, :], ones_bf[:, 0:1], sq[:, ko, :],
                                     start=(ko == 0), stop=(ko == KO - 1))
                m_ = pa.tile([1, NC_], F32, tag="row_m", bufs=1, name="m_")
                nc.vector.tensor_scalar_mul(m_[:], ps1[0:1, :], 1.0 / E)
                msq = pa.tile([1, NC_], F32, tag="row_q", bufs=1, name="msq")
                nc.vector.tensor_mul(msq[:], m_[:], m_[:])
                var = pa.tile([1, NC_], F32, tag="row_v", bufs=1, name="var")
                nc.vector.scalar_tensor_tensor(var[:], ps2[0:1, :], 1.0 / E,
                                               msq[:], OP.mult, OP.subtract)
                sqv = pa.tile([1, NC_], F32, tag="row_s", bufs=1, name="sqv")
                nc.scalar.activation(sqv[:], var[:], AF.Sqrt, bias=eps_t[:])
                rstd = pa.tile([1, NC_], F32, tag="row_r", bufs=1, name="rstd")
                nc.vector.reciprocal(rstd[:], sqv[:])
                rbf = pa.tile([1, NC_], BF16, tag="rowsb2", bufs=1, name="rbf")
                nc.vector.tensor_copy(rbf[:], rstd[:])
                nmr = pa.tile([1, NC_], BF16, tag="rowsb1", bufs=1, name="nmr")
                # nmr = -m * rstd
                nc.vector.scalar_tensor_tensor(nmr[:], m_[:], -1.0,
                                               rstd[:], OP.mult, OP.mult)
                rbc = ps()
                nc.tensor.matmul(rbc[:, :], ones_bf[0:1, 0:128], rbf[:],
                                 start=True, stop=True)
                for ko in range(KO):
                    bbc = ps()
                    nc.tensor.matmul(bbc[:, :], gl_sb[i][:, ko, :], nmr[:],
                                     start=True, stop=False)
                    nc.tensor.matmul(bbc[:, :], bl_sb[i][:, ko, :],
                                     ones_bf[0:1, 0:NC_], start=False, stop=True)
                    t0 = pa.tile([128, NC_], F32, tag="tmp", bufs=2, name="t0")
                    nc.vector.scalar_tensor_tensor(t0[:], xT_sb[:, ko, cs],
                                                   gpp[i][:, ko:ko + 1], rbc[:, :],
                                                   OP.mult, OP.mult)
                    nc.vector.tensor_tensor(ln[:, ko, cs], t0[:], bbc[:, :], OP.add)
            return ln

        def project_qk(pb_, lnsrc, w_d, tag, bufs=1):
            """-> [128, 4, T] bf16 : rows = 2 heads x 64, per pair j."""
            w_sb = pb_.tile([128, KO, 512], BF16, tag="wqkv", bufs=2, name="wsb")
            nc.sync.dma_start(out=w_sb[:], in_=w_d.rearrange("(ko p) m -> p ko m", p=128))
            qt = pb_.tile([128, 4, T], BF16, tag=tag, bufs=bufs, name="qt")
            for j in range(4):
                for c in range(CC):
                    p_ = ps()
                    for ko in range(KO):
                        nc.tensor.matmul(p_[:, :], w_sb[:, ko, j * 128:(j + 1) * 128],
                                         lnsrc[:, ko, c * NC_:(c + 1) * NC_],
                                         start=(ko == 0), stop=(ko == KO - 1))
                    nc.vector.tensor_copy(qt[:, j, c * NC_:(c + 1) * NC_], p_[:, :])
            return qt

        def project_v(pb_, src, w_d):
            """-> [128, 8, 8, 65] bf16 : [s_part, s_sub, head, d|ones]."""
            w_sb = pb_.tile([128, KO, 512], BF16, tag="wqkv", bufs=2, name="wsb")
            nc.sync.dma_start(out=w_sb[:], in_=w_d.rearrange("(ko p) m -> p ko m", p=128))
            vv = pb_.tile([128, 8, HL, 65], BF16, tag="vv", name="vv")
            for s in range(8):
                p_ = ps()
                for ko in range(KO):
                    nc.tensor.matmul(p_[:, :], src[:, ko, s * 128:(s + 1) * 128],
                                     w_sb[:, ko, :], start=(ko == 0), stop=(ko == KO - 1))
                nc.scalar.copy(out=vv[:, s, :, 0:64],
                               in_=p_[:, :].rearrange("p (h d) -> p h d", d=64))
                nc.vector.memset(vv[:, s, :, 64:65], 1.0)
            return vv

        def attention(pb_, qt, kt, vv, causal):
            onorm = pb_.tile([128, 4, T], BF16, tag="onorm", name="onorm")
            for c in range(CC):
                for h in range(HL):
                    j, half = h // 2, h % 2
                    pb = 64 * half
                    subs = list(range(4 * (c + 1))) if causal else list(range(8))
                    eb = pb_.tile([128, 8, NC_], BF16, tag="expb", bufs=2, name="eb")
                    for s_ in subs:
                        p_ = ps()
                        nc.tensor.matmul(p_[:, :],
                                         kt[pb:pb + 64, j, s_ * 128:(s_ + 1) * 128],
                                         qt[pb:pb + 64, j, c * NC_:(c + 1) * NC_],
                                         start=True, stop=True)
                        nc.scalar.activation(eb[:, s_, :], p_[:, :], AF.Exp)
                        if causal and s_ >= 4 * c:
                            nc.vector.tensor_mul(eb[:, s_, :], eb[:, s_, :],
                                                 cm_sb[:, s_ - 4 * c, :])
                    av = ps()
                    for i_, s_ in enumerate(subs):
                        nc.tensor.matmul(av[0:65, :], vv[:, s_, h, :], eb[:, s_, :],
                                         start=(i_ == 0), stop=(i_ == len(subs) - 1))
                    rr = pb_.tile([65, NC_], F32, tag="row_rr", bufs=2, name="rr")
                    nc.vector.reciprocal(rr[64:65, :], av[64:65, :])
                    rb = pb_.tile([65, NC_], BF16, tag="row_rrb", bufs=2, name="rb")
                    nc.vector.tensor_copy(rb[64:65, :], rr[64:65, :])
                    bc = ps()
                    nc.tensor.matmul(bc[0:64, :], ones_bf[64:65, 0:64], rb[64:65, :],
                                     start=True, stop=True)
                    bcs = pb_.tile([64, NC_], BF16, tag="bcs", bufs=2, name="bcs")
                    nc.vector.tensor_copy(bcs[:, :], bc[0:64, :])
                    nc.vector.tensor_tensor(onorm[pb:pb + 64, j, c * NC_:(c + 1) * NC_],
                                            av[0:64, :], bcs[:, :], OP.mult)
            return onorm

        def out_proj(pb_, onorm, wo_d, ark, bias_pp):
            wo_sb = pb_.tile([128, 4, E], BF16, tag="wo", name="wo_sb")
            nc.sync.dma_start(out=wo_sb[:], in_=wo_d.rearrange("(ks p) e -> p ks e", p=128))
            for c in range(CC):
                for m in range(KO):
                    p_ = ps()
                    for ks in range(4):
                        nc.tensor.matmul(p_[:, :], wo_sb[:, ks, m * 128:(m + 1) * 128],
                                         onorm[:, ks, c * NC_:(c + 1) * NC_],
                                         start=(ks == 0), stop=(ks == 3))
                    st = pa.tile([128, NC_], F32, tag="arst", bufs=4, name="st")
                    nc.vector.tensor_scalar_add(st[:, :], p_[:, :], bias_pp[:, m:m + 1])
                    nc.sync.dma_start(out=ark[c][0][m * 128:(m + 1) * 128, :], in_=st[:])
                allreduce_c(ark, c)

        def allreduce_c(ark, c):
            a_in, a_out = ark[c]
            if SKIP_CC:
                nc.sync.dma_start(out=a_out[:], in_=a_in[:])
            else:
                nc.gpsimd.collective_compute(
                    "AllReduce", OP.add, replica_groups=PAIRS,
                    ins=[a_in[:]], outs=[a_out[:]])
            nc.gpsimd.dma_start(
                out=xT_sb[:, :, c * NC_:(c + 1) * NC_],
                in_=a_out.rearrange("(ko p) t -> p ko t", p=128),
                accum_op=OP.add)



        with tc.tile_pool(name=f"B{ibody}", bufs=1) as pb_:
            # ---- self attention ----
            ln1 = layer_norm(0)
            qt = project_qk(pb_, ln1, W["wq"], "qt")
            kt = project_qk(pb_, ln1, W["wk"], "kt")
            vv = project_v(pb_, ln1, W["wv"])
            on1 = attention(pb_, qt, kt, vv, causal=True)
            out_proj(pb_, on1, W["wo"], ar[1], bo2pp)
            # cross K/V from raw ca — independent of AR1, fills the gap
            ktc = project_qk(pb_, ca_sb, W["wkc"], "kt")
            vvc = project_v(pb_, ca_sb, W["wvc"])

            # ---- cross attention ----
            ln2 = layer_norm(1)
            qtc = project_qk(pb_, ln2, W["wqc"], "qt")
            on2 = attention(pb_, qtc, ktc, vvc, causal=False)
            out_proj(pb_, on2, W["woc"], ar[2], bo2cpp)

        # ---- FFN ----
        ln3 = layer_norm(2)
        with tc.tile_pool(name=f"C{ibody}", bufs=1) as pc:
            ht = pc.tile([128, FF // 128, T], BF16, tag="ht", name="ht")
            for m in range(FF // 128):
                w1m = pc.tile([128, KO, 128], BF16, tag="w1m", bufs=6, name="w1m")
                nc.sync.dma_start(
                    out=w1m[:],
                    in_=W["w1"][m].rearrange("p (ko f) -> p ko f", f=128))
                for c in range(CC):
                    p_ = ps()
                    for ko in range(KO):
                        nc.tensor.matmul(p_[:, :], w1m[:, ko, :],
                                         ln3[:, ko, c * NC_:(c + 1) * NC_],
                                         start=(ko == 0), stop=(ko == KO - 1))
                    nc.scalar.activation(ht[:, m, c * NC_:(c + 1) * NC_], p_[:, :],
                                         AF.Relu, bias=b1pp[:, m:m + 1])
            w2m_t = [None] * KO
            for m in range(KO):
                w2m = pc.tile([128, FF // 128, 128], BF16, tag="w2m", bufs=8, name="w2m")
                nc.sync.dma_start(
                    out=w2m[:],
                    in_=W["w2"].rearrange("(ks p) e -> p ks e", p=128)[:, :, m * 128:(m + 1) * 128])
                w2m_t[m] = w2m
            for c in range(CC):
                for m in range(KO):
                    p_ = ps()
                    for ks in range(FF // 128):
                        nc.tensor.matmul(p_[:, :], w2m_t[m][:, ks, :],
                                         ht[:, ks, c * NC_:(c + 1) * NC_],
                                         start=(ks == 0), stop=(ks == FF // 128 - 1))
                    st = pa.tile([128, NC_], F32, tag="arst", bufs=4, name="st")
                    nc.vector.tensor_scalar_add(st[:, :], p_[:, :], b22pp[:, m:m + 1])
                    nc.sync.dma_start(out=ar[3][c][0][m * 128:(m + 1) * 128, :], in_=st[:])
                allreduce_c(ar[3], c)
                for ko in range(KO):
                    nc.sync.dma_start(
                        out=out_xT[ko * 128:(ko + 1) * 128, c * NC_:(c + 1) * NC_],
                        in_=xT_sb[:, ko, c * NC_:(c + 1) * NC_])


# ------------------------------------------------------------------ v2 (comm-free, pipelined)

def build2(nbody=1):
    """Comm-free sharding (core = (batch b, T-half h)) with:
    - LN stats broadcast across partitions (all-ones stationary matmul) so all
      row math runs 128-wide; rstd via sqrt + reciprocal_approx_fast.
    - Attention: software-pipelined QK/exp/AV across heads; per-head softmax
      denominators ride as a 65th V row, normalization deferred and batched
      (one reciprocal for all 16 heads).
    - Fully-masked key blocks handled by zeroing V (+ its ones row) per core;
      only the 4 diagonal blocks get a post-exp multiplicative mask.
    """
    nc = bacc.Bacc(num_devices=8)

    def P(name, shape, dt):
        return nc.declare_dram_parameter(name, shape, dt, isOutput=False)

    # all weight/activation layouts are host-pretransposed so every DMA is
    # contiguous per partition (descriptor-bound strided gathers killed ~180us
    # per DMA engine in the naive [E, .] layouts)
    xTb = P("xTb", [E, T], BF16)        # permuted x^T, bf16 (LN1 / self K,V)
    xTo = P("xTo", [128, KO * NC_], F32)   # own-half x^T  [p, ko*t]
    caT = P("caT", [128, KO * S], BF16)    # ca^T           [p, ko*t]
    wq, wk, wv = P("wq", [2, 128, KO * 512], BF16), P("wk", [2, 128, KO * 512], BF16), P("wv", [2, 128, KO * 512], BF16)
    wqc, wkc, wvc = P("wqc", [2, 128, KO * 512], BF16), P("wkc", [2, 128, KO * 512], BF16), P("wvc", [2, 128, KO * 512], BF16)
    wo, woc = P("wo", [KO, 128, E], BF16), P("woc", [KO, 128, E], BF16)
    w1 = P("w1", [4 * E // 128, 128, KO * 128], BF16)
    w2 = P("w2", [KO, 128, (4 * E // 128) * 128], BF16)
    gpp_d = [P(f"g{i}", [E], F32) for i in (1, 2, 3)]
    bpp_d = [P(f"be{i}", [E], F32) for i in (1, 2, 3)]
    bo_, boc_, b2_ = P("bo", [E], F32), P("boc", [E], F32), P("b2", [E], F32)
    b1r = P("b1r", [4 * E], F32)
    smask = P("smask", [128, 4, NC_], BF16)   # diagonal causal blocks
    vmsk = P("vmsk", [128, 8], F32)           # per-key-block V mask
    seld = P("seld", [16, 8, 128], BF16)      # head-pair broadcast selector
    out_xT = nc.declare_dram_parameter("out_xT", [E, NC_], F32, isOutput=True)

    with tile.TileContext(nc) as tc:
        with tc.tile_pool(name="persist", bufs=1) as pp:
            xTb_sb = pp.tile([128, KO, T], BF16, tag="xTb")
            for ko in range(KO):
                nc.sync.dma_start(out=xTb_sb[:, ko, 0:NC_],
                                  in_=xTb[ko * 128:(ko + 1) * 128, 0:NC_])
            for ko in range(KO):
                nc.sync.dma_start(out=xTb_sb[:, ko, NC_:T],
                                  in_=xTb[ko * 128:(ko + 1) * 128, NC_:T])
            xTo_sb = pp.tile([128, KO, NC_], F32, tag="xTo")
            nc.sync.dma_start(out=xTo_sb[:], in_=xTo.rearrange("p (ko t) -> p ko t", t=NC_))
            ca_sb = pp.tile([128, KO, S], BF16, tag="ca")
            nc.gpsimd.dma_start(out=ca_sb[:], in_=caT.rearrange("p (ko t) -> p ko t", t=S))
            sm_sb = pp.tile([128, 4, NC_], BF16, tag="sm")
            nc.scalar.dma_start(out=sm_sb[:], in_=smask[:])
            vm_sb = pp.tile([128, 8], F32, tag="vm")
            nc.scalar.dma_start(out=vm_sb[:], in_=vmsk[:])
            sel_sb = pp.tile([16, 8, 128], BF16, tag="sel")
            nc.scalar.dma_start(out=sel_sb[:], in_=seld[:])
            ones_bf = pp.tile([128, 512], BF16, tag="ones")
            nc.vector.memset(ones_bf[:], 1.0)
            gpp, bepp = [], []
            for i in range(3):
                t2 = pp.tile([128, KO], F32, tag=f"gpp{i}")
                with nc.allow_non_contiguous_dma(reason="tiny LN vector"):
                    nc.scalar.dma_start(out=t2[:], in_=gpp_d[i].rearrange("(ko p) -> p ko", p=128))
                gpp.append(t2)
                t3 = pp.tile([128, KO], F32, tag=f"bepp{i}")
                with nc.allow_non_contiguous_dma(reason="tiny LN vector"):
                    nc.scalar.dma_start(out=t3[:], in_=bpp_d[i].rearrange("(ko p) -> p ko", p=128))
                bepp.append(t3)
            bpp = []
            for nm, d in (("bo", bo_), ("boc", boc_), ("b2", b2_)):
                t_ = pp.tile([128, KO], F32, tag=nm)
                with nc.allow_non_contiguous_dma(reason="tiny bias vector"):
                    nc.scalar.dma_start(out=t_[:], in_=d.rearrange("(ko p) -> p ko", p=128))
                bpp.append(t_)
            eps_t = pp.tile([128, 1], F32, tag="eps")
            nc.vector.memset(eps_t[:], 1e-5)
            b1pp = pp.tile([128, 4 * E // 128], F32, tag="b1")
            with nc.allow_non_contiguous_dma(reason="tiny bias vector"):
                nc.scalar.dma_start(out=b1pp[:], in_=b1r.rearrange("(m p) -> p m", p=128))

            for ibody in range(nbody):
                _body2(nc, tc, ibody, xTb_sb, xTo_sb, ca_sb, sm_sb, vm_sb, sel_sb,
                       ones_bf, gpp, bepp, bpp, b1pp, eps_t,
                       dict(wq=wq, wk=wk, wv=wv, wqc=wqc, wkc=wkc, wvc=wvc,
                            wo=wo, woc=woc, w1=w1, w2=w2, xTo=xTo),
                       out_xT)
    nc.finalize()
    return nc


def _body2(nc, tc, ibody, xTb_sb, xTo_sb, ca_sb, sm_sb, vm_sb, sel_sb, ones_bf,
           gpp, bepp, bpp, b1pp, eps_t, W, out_xT):
    bopp, bocpp, b2pp = bpp

    if ibody > 0:
        nc.sync.dma_start(out=xTo_sb[:],
                          in_=W["xTo"].rearrange("p (ko t) -> p ko t", t=NC_))

    with tc.tile_pool(name=f"A{ibody}", bufs=1) as pa, \
         tc.tile_pool(name=f"ps{ibody}", bufs=8, space="PSUM") as pspool:

        def ps():
            return pspool.tile([128, NC_], F32, tag="ps", name="ps")

        def ln_v2(i, stat_tile, stat_off, mat_srcs, ln, nchunks):
            """stat_tile: bf16 tile [128, KO, >=stat_off+512*nchunks];
            mat_srcs[c][ko]: AP [128,512] (bf16 or f32); writes ln[:, ko, :]."""
            for c in range(nchunks):
                so = stat_off + c * NC_
                msrc = mat_srcs[c]
                sq = pa.tile([128, KO, NC_], BF16, tag="sq", bufs=1, name="sq")
                for ko in range(KO):
                    nc.vector.tensor_mul(sq[:, ko, :], stat_tile[:, ko, so:so + NC_],
                                         stat_tile[:, ko, so:so + NC_])
                ps1, ps2 = ps(), ps()
                for ko in range(KO):
                    nc.tensor.matmul(ps1[:, :], ones_bf[:, 0:128],
                                     stat_tile[:, ko, so:so + NC_],
                                     start=(ko == 0), stop=(ko == KO - 1))
                for ko in range(KO):
                    nc.tensor.matmul(ps2[:, :], ones_bf[:, 0:128], sq[:, ko, :],
                                     start=(ko == 0), stop=(ko == KO - 1))
                mbc = pa.tile([128, NC_], F32, tag="mbc", bufs=1, name="mbc")
                nc.vector.tensor_scalar_mul(mbc[:], ps1[:, :], 1.0 / E)
                msq = pa.tile([128, NC_], F32, tag="msq", bufs=1, name="msq")
                nc.vector.tensor_mul(msq[:], mbc[:], mbc[:])
                var = pa.tile([128, NC_], F32, tag="var", bufs=1, name="var")
                nc.vector.scalar_tensor_tensor(var[:], ps2[:, :], 1.0 / E,
                                               msq[:], OP.mult, OP.subtract)
                sqv = pa.tile([128, NC_], F32, tag="sqv", bufs=1, name="sqv")
                nc.scalar.activation(sqv[:], var[:], AF.Sqrt, bias=eps_t[:])
                rstd = pa.tile([128, NC_], F32, tag="rstd", bufs=1, name="rstd")
                nc.vector.reciprocal(rstd[:], sqv[:])
                nmr = pa.tile([128, NC_], F32, tag="nmr", bufs=1, name="nmr")
                nc.vector.scalar_tensor_tensor(nmr[:], mbc[:], -1.0,
                                               rstd[:], OP.mult, OP.mult)
                cs = slice(c * NC_, (c + 1) * NC_)
                for ko in range(KO):
                    t_ = pa.tile([128, NC_], F32, tag="lt", bufs=2, name="lt")
                    nc.vector.scalar_tensor_tensor(t_[:], msrc[ko],
                                                   gpp[i][:, ko:ko + 1], rstd[:],
                                                   OP.mult, OP.mult)
                    u_ = pa.tile([128, NC_], F32, tag="lu", bufs=2, name="lu")
                    nc.vector.scalar_tensor_tensor(u_[:], nmr[:],
                                                   gpp[i][:, ko:ko + 1], t_[:],
                                                   OP.mult, OP.add)
                    nc.vector.tensor_scalar_add(ln[:, ko, cs], u_[:],
                                                bepp[i][:, ko:ko + 1])

        def proj16(pb_, lnsrc, w_d, out_t, ncols):
            """16-head projection -> out_t [128, 8, ncols] bf16."""
            for jh in range(2):
                w_sb = pb_.tile([128, KO, 512], BF16, tag="wqkv", bufs=2, name="wsb")
                nc.sync.dma_start(
                    out=w_sb[:],
                    in_=w_d[jh].rearrange("p (ko m) -> p ko m", m=512))
                for jj in range(4):
                    j = jh * 4 + jj
                    for c in range(ncols // NC_):
                        p_ = ps()
                        for ko in range(KO):
                            nc.tensor.matmul(p_[:, :], w_sb[:, ko, jj * 128:(jj + 1) * 128],
                                             lnsrc[:, ko, c * NC_:(c + 1) * NC_],
                                             start=(ko == 0), stop=(ko == KO - 1))
                        nc.vector.tensor_copy(out_t[:, j, c * NC_:(c + 1) * NC_], p_[:, :])

        def proj_v(pb_, src, w_d, vv, use_vmask):
            for jh in range(2):
                w_sb = pb_.tile([128, KO, 512], BF16, tag="wqkv", bufs=2, name="wsb")
                nc.sync.dma_start(
                    out=w_sb[:],
                    in_=w_d[jh].rearrange("p (ko m) -> p ko m", m=512))
                for s in range(8):
                    p_ = ps()
                    for ko in range(KO):
                        nc.tensor.matmul(p_[:, :], src[:, ko, s * 128:(s + 1) * 128],
                                         w_sb[:, ko, :], start=(ko == 0), stop=(ko == KO - 1))
                    nc.scalar.copy(out=vv[:, s, jh * 8:(jh + 1) * 8, 0:64],
                                   in_=p_[:, :].rearrange("p (h d) -> p h d", d=64))
            for s in range(8):
                nc.vector.memset(vv[:, s, :, 64:65], 1.0)
                if use_vmask:
                    nc.vector.tensor_scalar_mul(vv[:, s, :, :],
                                                vv[:, s, :, :], vm_sb[:, s:s + 1])

        def attention2(pb_, qt, kt, vv, avo, masked):
            """Pipelined attention: per-head QK->exp->AV with the next head's
            QK emitted before this head's AV.  Softmax denominators ride as the
            65th V row, are gathered (lagged, via tiny K=1 matmuls) onto rows
            0..15 of one PSUM tile, and a single batched reciprocal serves all
            16 heads before the broadcast/normalize pass."""
            ebs, rowbufs = {}, {}
            dn16 = pb_.tile([16, NC_], F32, tag="dn16", bufs=1, name="dn16")

            def emit_qk(h):
                j, half = h // 2, h % 2
                pb = 64 * half
                eb = pb_.tile([128, 8, NC_], BF16, tag="eb", bufs=2, name="eb")
                ebs[h] = eb
                for s in range(8):
                    p_ = ps()
                    nc.tensor.matmul(p_[:, :],
                                     kt[pb:pb + 64, j, s * 128:(s + 1) * 128],
                                     qt[pb:pb + 64, j, :], start=True, stop=True)
                    nc.scalar.activation(eb[:, s, :], p_[:, :], AF.Exp)
                    if masked and s < 4:
                        nc.vector.tensor_mul(eb[:, s, :], eb[:, s, :], sm_sb[:, s, :])

            def emit_av(h):
                j, half = h // 2, h % 2
                pb = 64 * half
                eb = ebs.pop(h)
                av = ps()
                for s in range(8):
                    nc.tensor.matmul(av[0:65, :], vv[:, s, h, :], eb[:, s, :],
                                     start=(s == 0), stop=(s == 7))
                nc.vector.tensor_copy(avo[pb:pb + 64, j, :], av[0:64, :])
                rowb = pb_.tile([65, NC_], F32, tag="rowb", bufs=2, name="rowb")
                nc.vector.tensor_copy(rowb[64:65, :], av[64:65, :])
                rowbufs[h] = rowb

            def emit_gather(h):
                rowb = rowbufs.pop(h)
                nc.sync.dma_start(out=dn16[h:h + 1, :], in_=rowb[64:65, :])

            def qkav():
                emit_qk(0)
                for h in range(H):
                    if h + 1 < H:
                        emit_qk(h + 1)
                    emit_av(h)
                    if h > 0:
                        emit_gather(h - 1)
                emit_gather(H - 1)

            def norm_tail():
                nc.vector.reciprocal(dn16[:, :], dn16[:, :])
                rb = pb_.tile([16, NC_], BF16, tag="rb", bufs=1, name="rb")
                nc.vector.tensor_copy(rb[:, :], dn16[:, :])
                for j in range(8):
                    bc = ps()
                    nc.tensor.matmul(bc[:, :], sel_sb[:, j, :], rb[:, :],
                                     start=True, stop=True)
                    nc.vector.tensor_tensor(avo[:, j, :], avo[:, j, :],
                                            bc[:, :], OP.mult)
            return qkav, norm_tail

        def out_proj2(pb_, onorm, wo_d, bias_pp, per_m=None):
            for m in range(KO):
                wom = pb_.tile([128, KO, 128], BF16, tag="wom", bufs=2, name="wom")
                nc.sync.dma_start(
                    out=wom[:],
                    in_=wo_d[m].rearrange("p (ks e) -> p ks e", e=128))
                p_ = ps()
                for ks in range(KO):
                    nc.tensor.matmul(p_[:, :], wom[:, ks, :], onorm[:, ks, :],
                                     start=(ks == 0), stop=(ks == KO - 1))
                nc.vector.scalar_tensor_tensor(xTo_sb[:, m, :], p_[:, :],
                                               bias_pp[:, m:m + 1], xTo_sb[:, m, :],
                                               OP.add, OP.add)
                if per_m is not None:
                    per_m(m)

        with tc.tile_pool(name=f"B{ibody}", bufs=1) as pb_:
            # ---- self attention ----
            ln1 = pa.tile([128, KO, T], BF16, tag="ln1", name="ln1")
            ln_v2(0, xTb_sb, 0,
                  [[xTb_sb[:, ko, 0:NC_] for ko in range(KO)],
                   [xTb_sb[:, ko, NC_:T] for ko in range(KO)]],
                  ln1, 2)
            qt = pb_.tile([128, 8, NC_], BF16, tag="qt", bufs=1, name="qt")
            proj16(pb_, ln1, W["wq"], qt, NC_)
            kt = pb_.tile([128, 8, T], BF16, tag="kt", bufs=1, name="kt")
            proj16(pb_, ln1, W["wk"], kt, T)
            vv = pb_.tile([128, 8, H, 65], BF16, tag="vv", bufs=1, name="vv")
            proj_v(pb_, ln1, W["wv"], vv, use_vmask=True)
            avo = pb_.tile([128, 8, NC_], BF16, tag="avo", bufs=1, name="avo")
            qkav, norm_tail = attention2(pb_, qt, kt, vv, avo, masked=True)
            qkav()
            # cross K projection is independent -> fills the softmax-recip tail
            ktc = pb_.tile([128, 8, T], BF16, tag="kt", bufs=1, name="ktc")
            proj16(pb_, ca_sb, W["wkc"], ktc, T)
            norm_tail()
            ln2 = pa.tile([128, KO, NC_], BF16, tag="ln2", name="ln2")
            xb2 = pa.tile([128, KO, NC_], BF16, tag="xb2", bufs=1, name="xb2")
            out_proj2(pb_, avo, W["wo"], bopp,
                      per_m=lambda m: nc.scalar.copy(out=xb2[:, m, :],
                                                     in_=xTo_sb[:, m, :]))

            # ---- cross attention ----
            ln_v2(1, xb2, 0, [[xTo_sb[:, ko, :] for ko in range(KO)]], ln2, 1)
            vvc = pb_.tile([128, 8, H, 65], BF16, tag="vv", bufs=1, name="vvc")
            proj_v(pb_, ca_sb, W["wvc"], vvc, use_vmask=False)
            qtc = pb_.tile([128, 8, NC_], BF16, tag="qt", bufs=1, name="qtc")
            proj16(pb_, ln2, W["wqc"], qtc, NC_)
            avoc = pb_.tile([128, 8, NC_], BF16, tag="avo", bufs=1, name="avoc")
            qkavc, norm_tailc = attention2(pb_, qtc, ktc, vvc, avoc, masked=False)
            qkavc()
            norm_tailc()
            xb3 = pa.tile([128, KO, NC_], BF16, tag="xb2", bufs=1, name="xb3")
            out_proj2(pb_, avoc, W["woc"], bocpp,
                      per_m=lambda m: nc.scalar.copy(out=xb3[:, m, :],
                                                     in_=xTo_sb[:, m, :]))

        # ---- FFN ----
        with tc.tile_pool(name=f"C{ibody}", bufs=1) as pc:
            ln3 = pa.tile([128, KO, NC_], BF16, tag="ln2", name="ln3")
            ln_v2(2, xb3, 0, [[xTo_sb[:, ko, :] for ko in range(KO)]], ln3, 1)
            FH = 4 * E // 128
            ht = pc.tile([128, FH, NC_], BF16, tag="ht", name="ht")
            for m in range(FH):
                w1m = pc.tile([128, KO, 128], BF16, tag="w1m", bufs=6, name="w1m")
                nc.sync.dma_start(
                    out=w1m[:],
                    in_=W["w1"][m].rearrange("p (ko f) -> p ko f", f=128))
                p_ = ps()
                for ko in range(KO):
                    nc.tensor.matmul(p_[:, :], w1m[:, ko, :], ln3[:, ko, :],
                                     start=(ko == 0), stop=(ko == KO - 1))
                nc.scalar.activation(ht[:, m, :], p_[:, :], AF.Relu,
                                     bias=b1pp[:, m:m + 1])
            for m in range(KO):
                w2m = pc.tile([128, FH, 128], BF16, tag="w2m", bufs=2, name="w2m")
                nc.sync.dma_start(
                    out=w2m[:],
                    in_=W["w2"][m].rearrange("p (ks e) -> p ks e", e=128))
                p_ = ps()
                for ks in range(FH):
                    nc.tensor.matmul(p_[:, :], w2m[:, ks, :], ht[:, ks, :],
                                     start=(ks == 0), stop=(ks == FH - 1))
                nc.vector.scalar_tensor_tensor(xTo_sb[:, m, :], p_[:, :],
                                               b2pp[:, m:m + 1], xTo_sb[:, m, :],
                                               OP.add, OP.add)
                nc.sync.dma_start(out=out_xT[m * 128:(m + 1) * 128, :],
                                  in_=xTo_sb[:, m, :])


def _make_core_inputs2(c, inp):
    bf = ml_dtypes.bfloat16
    b, h = divmod(c, 2)
    sc = float(E) ** -0.5
    own = slice(512 * h, 512 * h + 512)
    oth = slice(512 * (1 - h), 512 * (1 - h) + 512)

    def stack_heads(w):  # [16, E, D] -> [E, 1024]
        return np.ascontiguousarray(np.transpose(w, (1, 0, 2)).reshape(E, E))

    def qkv_layout(w):  # [E, 1024] -> [2(half), 128, KO*512], contiguous DMA
        a = w.reshape(8, 128, 1024).transpose(1, 0, 2)      # [p, ko, m]
        return np.ascontiguousarray(np.stack(
            [a[:, :, :512].reshape(128, 8 * 512),
             a[:, :, 512:].reshape(128, 8 * 512)])).astype(bf)

    def mtile_layout(w, km, cm):  # [km*128, cm*128] -> [cm, 128, km*128]
        a = w.reshape(km, 128, cm, 128).transpose(2, 1, 0, 3)
        return np.ascontiguousarray(a.reshape(cm, 128, km * 128)).astype(bf)

    def pkt_layout(xT):  # [E, ncols] -> [128, KO*ncols]
        n = xT.shape[1]
        return np.ascontiguousarray(
            xT.reshape(8, 128, n).transpose(1, 0, 2).reshape(128, 8 * n))

    xt = np.asarray(inp["x"][b], np.float32)           # [T, E]
    xperm = np.concatenate([xt[own], xt[oth]], axis=0)  # keys permuted: own first

    # diagonal causal mask blocks: key local pos (128j + p) <= query local pos f
    p, f = np.arange(128)[:, None, None], np.arange(512)[None, None, :]
    jj = np.arange(4)[None, :, None]
    smask = (128 * jj + p <= f).astype(bf)

    # V-block mask: subs 0..3 = own half (visible, diag-masked); 4..7 = other
    # half: visible iff this core owns the second half (h == 1)
    vmsk = np.ones((128, 8), np.float32)
    vmsk[:, 4:] = 1.0 if h == 1 else 0.0

    # selector: sel[k, j, m] = 1 iff head (2j + m//64) == k  (softmax recip
    # broadcast: one K=16 matmul expands rows of rb to a [128,512] tile)
    kk = np.arange(16)[:, None, None]
    jj2 = np.arange(8)[None, :, None]
    mm = np.arange(128)[None, None, :]
    sel = (kk == 2 * jj2 + mm // 64).astype(bf)

    return {
        "xTb": np.ascontiguousarray(xperm.T).astype(bf),
        "xTo": pkt_layout(xt[own].T).astype(np.float32),
        "caT": pkt_layout(np.asarray(inp["ca"][b]).T).astype(bf),
        "wq": qkv_layout(stack_heads(inp["Wq_s"]) * sc),
        "wk": qkv_layout(stack_heads(inp["Wk_s"])),
        "wv": qkv_layout(stack_heads(inp["Wv_s"])),
        "wqc": qkv_layout(stack_heads(inp["Wq_c"]) * sc),
        "wkc": qkv_layout(stack_heads(inp["Wk_c"])),
        "wvc": qkv_layout(stack_heads(inp["Wv_c"])),
        "wo": mtile_layout(np.asarray(inp["Wo_s"], np.float32), 8, 8),
        "woc": mtile_layout(np.asarray(inp["Wo_c"], np.float32), 8, 8),
        "w1": mtile_layout(np.asarray(inp["W1"], np.float32), 8, 32),
        "w2": mtile_layout(np.asarray(inp["W2"], np.float32), 32, 8),
        "g1": np.asarray(inp["ln1_g"], np.float32),
        "g2": np.asarray(inp["ln2_g"], np.float32),
        "g3": np.asarray(inp["ln3_g"], np.float32),
        "be1": np.asarray(inp["ln1_b"], np.float32),
        "be2": np.asarray(inp["ln2_b"], np.float32),
        "be3": np.asarray(inp["ln3_b"], np.float32),
        "bo": np.asarray(inp["bo_s"], np.float32),
        "boc": np.asarray(inp["bo_c"], np.float32),
        "b2": np.asarray(inp["b2"], np.float32),
        "b1r": np.asarray(inp["b1"], np.float32),
        "smask": smask,
        "vmsk": vmsk,
        "seld": sel,
    }


# ------------------------------------------------------------------ host side

_CACHE = {}


COMM_FREE = True
VARIANT = "v2"   # "tp2" | "nocc" | "v2"


def _variant():
    builders = {"tp2": (build, _make_core_inputs),
                "nocc": (build_nocc, _make_core_inputs_nocc),
                "v2": (build2, _make_core_inputs2)}
    return builders[VARIANT]


def _get_runner(nbody=1):
    key = (nbody, VARIANT)
    if key in _CACHE:
        return _CACHE[key]
    import jax
    from jax.sharding import Mesh, PartitionSpec
    from jax.experimental.shard_map import shard_map
    from concourse.bass2jax import (_bass_exec_p, install_neuronx_cc_hook,
                                    partition_id_tensor)

    nc = _variant()[0](nbody)
    install_neuronx_cc_hook()
    pn = nc.partition_id_tensor.name if nc.partition_id_tensor else None
    in_names, out_names, out_avals = [], [], []
    for alloc in nc.m.functions[0].allocations:
        if not isinstance(alloc, mybir.MemoryLocationSet):
            continue
        name = alloc.memorylocations[0].name
        if alloc.kind == "ExternalInput":
            if name != pn:
                in_names.append(name)
        elif alloc.kind == "ExternalOutput":
            out_names.append(name)
            out_avals.append(jax.core.ShapedArray(
                tuple(alloc.tensor_shape), mybir.dt.np(alloc.dtype)))
    n_params = len(in_names)
    all_in = in_names + out_names + ([pn] if pn else [])

    def _jbody(*args):
        ops = list(args)
        if pn:
            ops.append(partition_id_tensor())
        return tuple(_bass_exec_p.bind(
            *ops, out_avals=tuple(out_avals), in_names=tuple(all_in),
            out_names=tuple(out_names), lowering_input_output_aliases=(),
            sim_require_finite=True, sim_require_nnan=True, nc=nc))

    devices = jax.devices()[:8]
    mesh = Mesh(np.asarray(devices), ("core",))
    spec = (PartitionSpec("core"),)
    fn = jax.jit(shard_map(_jbody, mesh=mesh,
                           in_specs=spec * (n_params + len(out_names)),
                           out_specs=spec * len(out_names), check_rep=False),
                 keep_unused=True)
    _CACHE[key] = (fn, in_names, out_names, out_avals)
    return _CACHE[key]


def _make_core_inputs(c, inp):
    bf = ml_dtypes.bfloat16
    b, r = divmod(c, 2)
    hs = slice(8 * r, 8 * r + 8)
    sc = float(E) ** -0.5

    def stack_heads(w):  # [8, E, D] -> [E, 512]
        return np.ascontiguousarray(np.transpose(w, (1, 0, 2)).reshape(E, 512))

    p, f = np.arange(128)[:, None, None], np.arange(512)[None, None, :]
    jj = np.arange(4)[None, :, None]
    cmask = (f >= 128 * jj + p).astype(bf)

    return {
        "xT": np.ascontiguousarray(inp["x"][b].T).astype(np.float32),
        "caT": np.ascontiguousarray(inp["ca"][b].T).astype(bf),
        "wq": (stack_heads(inp["Wq_s"][hs]) * sc).astype(bf),
        "wk": stack_heads(inp["Wk_s"][hs]).astype(bf),
        "wv": stack_heads(inp["Wv_s"][hs]).astype(bf),
        "wqc": (stack_heads(inp["Wq_c"][hs]) * sc).astype(bf),
        "wkc": stack_heads(inp["Wk_c"][hs]).astype(bf),
        "wvc": stack_heads(inp["Wv_c"][hs]).astype(bf),
        "wo": np.ascontiguousarray(inp["Wo_s"][512 * r:512 * (r + 1), :]).astype(bf),
        "woc": np.ascontiguousarray(inp["Wo_c"][512 * r:512 * (r + 1), :]).astype(bf),
        "w1": np.ascontiguousarray(inp["W1"][:, FF * r:FF * (r + 1)]).astype(bf),
        "w2": np.ascontiguousarray(inp["W2"][FF * r:FF * (r + 1), :]).astype(bf),
        "gb1": np.stack([inp["ln1_g"], inp["ln1_b"]]).astype(bf),
        "gb2": np.stack([inp["ln2_g"], inp["ln2_b"]]).astype(bf),
        "gb3": np.stack([inp["ln3_g"], inp["ln3_b"]]).astype(bf),
        "g1": np.asarray(inp["ln1_g"], np.float32),
        "g2": np.asarray(inp["ln2_g"], np.float32),
        "g3": np.asarray(inp["ln3_g"], np.float32),
        "bo2": np.asarray(inp["bo_s"], np.float32) * 0.5,
        "bo2c": np.asarray(inp["bo_c"], np.float32) * 0.5,
        "b22": np.asarray(inp["b2"], np.float32) * 0.5,
        "b1r": np.asarray(inp["b1"][FF * r:FF * (r + 1)], np.float32),
        "cmask": cmask,
    }


def _run(nbody, in_maps, dev_inputs=None, dev_zeros=None, download=True):
    import jax
    fn, in_names, out_names, out_avals = _get_runner(nbody)
    if dev_inputs is None:
        concat = [np.concatenate([np.asarray(in_maps[c][n]) for c in range(8)], axis=0)
                  for n in in_names]
        dev_inputs = [jax.device_put(a) for a in concat]
    if dev_zeros is None:
        dev_zeros = [jax.device_put(np.zeros((8 * a.shape[0], *a.shape[1:]), a.dtype))
                     for a in out_avals]
    outs = fn(*dev_inputs, *dev_zeros)
    for o in outs:
        o.block_until_ready()
    if not download:
        return None, (dev_inputs, dev_zeros)
    res = []
    for c in range(8):
        res.append({n: np.asarray(outs[i]).reshape(8, *out_avals[i].shape)[c]
                    for i, n in enumerate(out_names)})
    return res, (dev_inputs, dev_zeros)


def kernel(**inputs):
    inp = {k: np.asarray(v) for k, v in inputs.items()}
    mk = _variant()[1]
    in_maps = [mk(c, inp) for c in range(8)]
    res, _ = _run(1, in_maps)
    if VARIANT in ("nocc", "v2"):
        out = np.stack([
            np.concatenate([res[2 * b]["out_xT"], res[2 * b + 1]["out_xT"]],
                           axis=1).T
            for b in range(B)]).astype(np.float32)
    else:
        out = np.stack([res[2 * b]["out_xT"].T for b in range(B)]).astype(np.float32)
    return out


# ---------------------------------------------------------------- comm-free

def build_nocc(nbody=1):
    """Communication-free sharding: core = (batch b, T-half h).  Each core
    computes its 512 query tokens for ALL 16 heads and the full FFN, with
    K/V duplicated across the pair.  Self-attn keys are permuted so the own
    half always sits at key positions 0..511 (the per-core causal mask input
    encodes the permutation) — keeps the SPMD program identical on all cores.
    """
    nc = bacc.Bacc(num_devices=8)

    def P(name, shape, dt):
        return nc.declare_dram_parameter(name, shape, dt, isOutput=False)

    # all weight/activation layouts are host-pretransposed so every DMA is
    # contiguous per partition (descriptor-bound strided gathers killed ~180us
    # per DMA engine in the naive [E, .] layouts)
    xTb = P("xTb", [E, T], BF16)        # permuted x^T, bf16 (LN1 / self K,V)
    xTo = P("xTo", [128, KO * NC_], F32)   # own-half x^T  [p, ko*t]
    caT = P("caT", [128, KO * S], BF16)    # ca^T           [p, ko*t]
    wq, wk, wv = P("wq", [2, 128, KO * 512], BF16), P("wk", [2, 128, KO * 512], BF16), P("wv", [2, 128, KO * 512], BF16)
    wqc, wkc, wvc = P("wqc", [2, 128, KO * 512], BF16), P("wkc", [2, 128, KO * 512], BF16), P("wvc", [2, 128, KO * 512], BF16)
    wo, woc = P("wo", [KO, 128, E], BF16), P("woc", [KO, 128, E], BF16)
    w1 = P("w1", [4 * E // 128, 128, KO * 128], BF16)
    w2 = P("w2", [KO, 128, (4 * E // 128) * 128], BF16)
    gb = [P(f"gb{i}", [2, E], BF16) for i in (1, 2, 3)]
    gpp_d = [P(f"g{i}", [E], F32) for i in (1, 2, 3)]
    bo_, boc_, b2_ = P("bo", [E], F32), P("boc", [E], F32), P("b2", [E], F32)
    b1r = P("b1r", [4 * E], F32)
    smask = P("smask", [128, 8, NC_], BF16)
    out_xT = nc.declare_dram_parameter("out_xT", [E, NC_], F32, isOutput=True)

    with tile.TileContext(nc) as tc:
        with tc.tile_pool(name="persist", bufs=1) as pp:
            xTb_sb = pp.tile([128, KO, T], BF16, tag="xTb")
            for ko in range(KO):
                nc.sync.dma_start(out=xTb_sb[:, ko, 0:NC_],
                                  in_=xTb[ko * 128:(ko + 1) * 128, 0:NC_])
            for ko in range(KO):
                nc.sync.dma_start(out=xTb_sb[:, ko, NC_:T],
                                  in_=xTb[ko * 128:(ko + 1) * 128, NC_:T])
            xTo_sb = pp.tile([128, KO, NC_], F32, tag="xTo")
            nc.sync.dma_start(out=xTo_sb[:], in_=xTo.rearrange("(ko p) t -> p ko t", p=128))
            ca_sb = pp.tile([128, KO, S], BF16, tag="ca")
            nc.sync.dma_start(out=ca_sb[:], in_=caT.rearrange("(ko p) t -> p ko t", p=128))
            sm_sb = pp.tile([128, 8, NC_], BF16, tag="sm")
            nc.sync.dma_start(out=sm_sb[:], in_=smask[:])
            ones_bf = pp.tile([128, 512], BF16, tag="ones")
            nc.vector.memset(ones_bf[:], 1.0)
            gl_sb, bl_sb, gpp = [], [], []
            for i in range(3):
                ta = pp.tile([1, KO, 128], BF16, tag=f"gl{i}")
                nc.sync.dma_start(out=ta[:], in_=gb[i].rearrange("a (ko m) -> a ko m", m=128)[0:1])
                gl_sb.append(ta)
                tb = pp.tile([1, KO, 128], BF16, tag=f"bl{i}")
                nc.sync.dma_start(out=tb[:], in_=gb[i].rearrange("a (ko m) -> a ko m", m=128)[1:2])
                bl_sb.append(tb)
                t2 = pp.tile([128, KO], F32, tag=f"gpp{i}")
                with nc.allow_non_contiguous_dma(reason="tiny LN vector"):
                    nc.sync.dma_start(out=t2[:], in_=gpp_d[i].rearrange("(ko p) -> p ko", p=128))
                gpp.append(t2)
            bpp = []
            for nm, d in (("bo", bo_), ("boc", boc_), ("b2", b2_)):
                t_ = pp.tile([128, KO], F32, tag=nm)
                with nc.allow_non_contiguous_dma(reason="tiny bias vector"):
                    nc.scalar.dma_start(out=t_[:], in_=d.rearrange("(ko p) -> p ko", p=128))
                bpp.append(t_)
            eps_t = pp.tile([1, 1], F32, tag="eps")
            nc.vector.memset(eps_t[:], 1e-5)
            b1pp = pp.tile([128, 4 * E // 128], F32, tag="b1")
            with nc.allow_non_contiguous_dma(reason="tiny bias vector"):
                nc.scalar.dma_start(out=b1pp[:], in_=b1r.rearrange("(m p) -> p m", p=128))

            for ibody in range(nbody):
                _body_nocc(nc, tc, ibody, xTb_sb, xTo_sb, ca_sb, sm_sb, ones_bf,
                           (gl_sb, bl_sb), gpp, bpp, b1pp, eps_t,
                           dict(wq=wq, wk=wk, wv=wv, wqc=wqc, wkc=wkc, wvc=wvc,
                                wo=wo, woc=woc, w1=w1, w2=w2, xTo=xTo),
                           out_xT)
    nc.finalize()
    return nc


def _body_nocc(nc, tc, ibody, xTb_sb, xTo_sb, ca_sb, sm_sb, ones_bf, gbl, gpp,
               bpp, b1pp, eps_t, W, out_xT):
    gl_sb, bl_sb = gbl
    bopp, bocpp, b2pp = bpp

    if ibody > 0:
        nc.sync.dma_start(out=xTo_sb[:],
                          in_=W["xTo"].rearrange("p (ko t) -> p ko t", t=NC_))

    with tc.tile_pool(name=f"A{ibody}", bufs=1) as pa, \
         tc.tile_pool(name=f"ps{ibody}", bufs=8, space="PSUM") as pspool:

        pb2_ref = [None]

        def ps():
            return pspool.tile([128, NC_], F32, tag="ps", name="ps")

        def ln_rows(i, ps1, ps2, cs_out, ln, src, src_is_bf, gsl, ncols):
            m_ = pa.tile([1, NC_], F32, tag="row_m", bufs=1, name="m_")
            nc.vector.tensor_scalar_mul(m_[:, :ncols], ps1[0:1, :ncols], 1.0 / E)
            msq = pa.tile([1, NC_], F32, tag="row_q", bufs=1, name="msq")
            nc.vector.tensor_mul(msq[:, :ncols], m_[:, :ncols], m_[:, :ncols])
            var = pa.tile([1, NC_], F32, tag="row_v", bufs=1, name="var")
            nc.vector.scalar_tensor_tensor(var[:, :ncols], ps2[0:1, :ncols], 1.0 / E,
                                           msq[:, :ncols], OP.mult, OP.subtract)
            sqv = pa.tile([1, NC_], F32, tag="row_s", bufs=1, name="sqv")
            nc.scalar.activation(sqv[:, :ncols], var[:, :ncols], AF.Sqrt, bias=eps_t[:])
            rstd = pa.tile([1, NC_], F32, tag="row_r", bufs=1, name="rstd")
            nc.vector.reciprocal(rstd[:, :ncols], sqv[:, :ncols])
            rbf = pa.tile([1, NC_], BF16, tag="rowsb2", bufs=1, name="rbf")
            nc.vector.tensor_copy(rbf[:, :ncols], rstd[:, :ncols])
            nmr = pa.tile([1, NC_], BF16, tag="rowsb1", bufs=1, name="nmr")
            nc.vector.scalar_tensor_tensor(nmr[:, :ncols], m_[:, :ncols], -1.0,
                                           rstd[:, :ncols], OP.mult, OP.mult)
            rbc = ps()
            nc.tensor.matmul(rbc[:, :ncols], ones_bf[0:1, 0:128], rbf[:, :ncols],
                             start=True, stop=True)
            for ko in range(KO):
                bbc = ps()
                nc.tensor.matmul(bbc[:, :ncols], gl_sb[i][:, ko, :], nmr[:, :ncols],
                                 start=True, stop=False)
                nc.tensor.matmul(bbc[:, :ncols], bl_sb[i][:, ko, :],
                                 ones_bf[0:1, :ncols], start=False, stop=True)
                t0 = pa.tile([128, NC_], F32, tag="tmp", bufs=2, name="t0")
                nc.vector.scalar_tensor_tensor(t0[:, :ncols], src[ko],
                                               gpp[i][:, ko:ko + 1], rbc[:, :ncols],
                                               OP.mult, OP.mult)
                nc.vector.tensor_tensor(ln[:, ko, cs_out], t0[:, :ncols],
                                        bbc[:, :ncols], OP.add)

        def layer_norm1():
            """full-T LN over xTb (bf16 source)."""
            ln = pa.tile([128, KO, T], BF16, tag="lnf", name="lnf")
            for c in range(CC):
                cs = slice(c * NC_, (c + 1) * NC_)
                sq = pa.tile([128, KO, NC_], BF16, tag="stat", bufs=2, name="sq")
                nc.scalar.activation(sq[:], xTb_sb[:, :, cs], AF.Square)
                ps1, ps2 = ps(), ps()
                for ko in range(KO):
                    nc.tensor.matmul(ps1[0:1, :], ones_bf[:, 0:1], xTb_sb[:, ko, cs],
                                     start=(ko == 0), stop=(ko == KO - 1))
                for ko in range(KO):
                    nc.tensor.matmul(ps2[0:1, :], ones_bf[:, 0:1], sq[:, ko, :],
                                     start=(ko == 0), stop=(ko == KO - 1))
                ln_rows(0, ps1, ps2, cs, ln,
                        [xTb_sb[:, ko, cs] for ko in range(KO)], True, None, NC_)
            return ln

        def layer_norm_h(i):
            """own-half LN over xTo (f32 residual)."""
            ln = pa.tile([128, KO, NC_], BF16, tag="lnh", bufs=1, name="lnh")
            xb = pa.tile([128, KO, NC_], BF16, tag="stat", bufs=2, name="xb")
            for ko in range(KO):
                nc.scalar.copy(out=xb[:, ko, :], in_=xTo_sb[:, ko, :])
            sq = pa.tile([128, KO, NC_], BF16, tag="stat", bufs=2, name="sq")
            nc.scalar.activation(sq[:], xb[:], AF.Square)
            ps1, ps2 = ps(), ps()
            for ko in range(KO):
                nc.tensor.matmul(ps1[0:1, :], ones_bf[:, 0:1], xb[:, ko, :],
                                 start=(ko == 0), stop=(ko == KO - 1))
            for ko in range(KO):
                nc.tensor.matmul(ps2[0:1, :], ones_bf[:, 0:1], sq[:, ko, :],
                                 start=(ko == 0), stop=(ko == KO - 1))
            ln_rows(i, ps1, ps2, slice(0, NC_), ln,
                    [xTo_sb[:, ko, :] for ko in range(KO)], False, None, NC_)
            return ln

        def project_qt(lnsrc, w_d, cols):
            """Q^T for 16 heads over `cols` own tokens -> [128, 8, 512]."""
            qt = pb2_ref[0].tile([128, 8, NC_], BF16, tag="qon", bufs=2, name="qt")
            for jh in range(2):          # stream wq in halves of 512 cols
                w_sb = pb2_ref[0].tile([128, KO, 512], BF16, tag="wqkv", bufs=1, name="wsb")
                nc.sync.dma_start(
                    out=w_sb[:],
                    in_=w_d.rearrange("(ko p) m -> p ko m", p=128)[:, :, jh * 512:(jh + 1) * 512])
                for jj in range(4):
                    j = jh * 4 + jj
                    p_ = ps()
                    for ko in range(KO):
                        nc.tensor.matmul(p_[:, :], w_sb[:, ko, jj * 128:(jj + 1) * 128],
                                         lnsrc[ko], start=(ko == 0), stop=(ko == KO - 1))
                    nc.vector.tensor_copy(qt[:, j, :], p_[:, :])
            return qt

        def project_kt(src, w_d):
            """K^T for 16 heads over full S -> [128, 8, 1024]."""
            kt = pb2_ref[0].tile([128, 8, T], BF16, tag="kt", name="kt")
            for jh in range(2):
                w_sb = pb2_ref[0].tile([128, KO, 512], BF16, tag="wqkv", bufs=1, name="wsb")
                nc.sync.dma_start(
                    out=w_sb[:],
                    in_=w_d.rearrange("(ko p) m -> p ko m", p=128)[:, :, jh * 512:(jh + 1) * 512])
                for jj in range(4):
                    j = jh * 4 + jj
                    for c in range(CC):
                        p_ = ps()
                        for ko in range(KO):
                            nc.tensor.matmul(p_[:, :], w_sb[:, ko, jj * 128:(jj + 1) * 128],
                                             src[:, ko, c * NC_:(c + 1) * NC_],
                                             start=(ko == 0), stop=(ko == KO - 1))
                        nc.vector.tensor_copy(kt[:, j, c * NC_:(c + 1) * NC_], p_[:, :])
            return kt

        def project_v(src, w_d):
            """V for 16 heads -> [128, 8, 16, 65]."""
            vv = pb2_ref[0].tile([128, 8, H, 65], BF16, tag="vv", name="vv")
            for jh in range(2):
                w_sb = pb2_ref[0].tile([128, KO, 512], BF16, tag="wqkv", bufs=1, name="wsb")
                nc.sync.dma_start(
                    out=w_sb[:],
                    in_=w_d.rearrange("(ko p) m -> p ko m", p=128)[:, :, jh * 512:(jh + 1) * 512])
                for s in range(8):
                    p_ = ps()
                    for ko in range(KO):
                        nc.tensor.matmul(p_[:, :], src[:, ko, s * 128:(s + 1) * 128],
                                         w_sb[:, ko, :], start=(ko == 0), stop=(ko == KO - 1))
                    nc.scalar.copy(out=vv[:, s, jh * 8:(jh + 1) * 8, 0:64],
                                   in_=p_[:, :].rearrange("p (h d) -> p h d", d=64))
                    nc.vector.memset(vv[:, s, jh * 8:(jh + 1) * 8, 64:65], 1.0)
            return vv

        def attention(qt, kt, vv, onorm, masked):
            for h_ in range(H):
                j, half = h_ // 2, h_ % 2
                pb = 64 * half
                eb = pb2_ref[0].tile([128, 8, NC_], BF16, tag="expb", bufs=1, name="eb")
                for s_ in range(8):
                    p_ = ps()
                    nc.tensor.matmul(p_[:, :],
                                     kt[pb:pb + 64, j, s_ * 128:(s_ + 1) * 128],
                                     qt[pb:pb + 64, j, :], start=True, stop=True)
                    nc.scalar.activation(eb[:, s_, :], p_[:, :], AF.Exp)
                    if masked:
                        nc.vector.tensor_mul(eb[:, s_, :], eb[:, s_, :],
                                             sm_sb[:, s_, :])
                av = ps()
                for s_ in range(8):
                    nc.tensor.matmul(av[0:65, :], vv[:, s_, h_, :], eb[:, s_, :],
                                     start=(s_ == 0), stop=(s_ == 7))
                rr = pb2_ref[0].tile([65, NC_], F32, tag="row_rr", bufs=1, name="rr")
                nc.vector.reciprocal(rr[64:65, :], av[64:65, :])
                rb = pb2_ref[0].tile([65, NC_], BF16, tag="row_rrb", bufs=1, name="rb")
                nc.vector.tensor_copy(rb[64:65, :], rr[64:65, :])
                bc = ps()
                nc.tensor.matmul(bc[0:64, :], ones_bf[64:65, 0:64], rb[64:65, :],
                                 start=True, stop=True)
                bcs = pb2_ref[0].tile([64, NC_], BF16, tag="bcs", bufs=2, name="bcs")
                nc.vector.tensor_copy(bcs[:, :], bc[0:64, :])
                nc.vector.tensor_tensor(onorm[pb:pb + 64, j, :],
                                        av[0:64, :], bcs[:, :], OP.mult)

        def out_proj(onorm, wo_d, bias_pp):
            for m in range(KO):
                wom = pb2_ref[0].tile([128, KO, 128], BF16, tag="wom", bufs=4, name="wom")
                nc.sync.dma_start(
                    out=wom[:],
                    in_=wo_d.rearrange("(ks p) e -> p ks e", p=128)[:, :, m * 128:(m + 1) * 128])
                p_ = ps()
                for ks in range(KO):
                    nc.tensor.matmul(p_[:, :], wom[:, ks, :], onorm[:, ks, :],
                                     start=(ks == 0), stop=(ks == KO - 1))
                nc.vector.scalar_tensor_tensor(xTo_sb[:, m, :], p_[:, :],
                                               bias_pp[:, m:m + 1], xTo_sb[:, m, :],
                                               OP.add, OP.add)

        with tc.tile_pool(name=f"B{ibody}", bufs=1) as _pb2:
            pb2_ref[0] = _pb2
            # ---- self attention ----
            ln1 = layer_norm1()
            qt = project_qt([ln1[:, ko, 0:NC_] for ko in range(KO)], W["wq"], NC_)
            kt = project_kt(ln1, W["wk"])
            vv = project_v(ln1, W["wv"])
            on1 = _pb2.tile([128, 8, NC_], BF16, tag="qon", bufs=2, name="on1")
            attention(qt, kt, vv, on1, masked=True)
            out_proj(on1, W["wo"], bopp)

            # ---- cross attention ----
            ln2 = layer_norm_h(1)
            qtc = project_qt([ln2[:, ko, :] for ko in range(KO)], W["wqc"], NC_)
            ktc = project_kt(ca_sb, W["wkc"])
            vvc = project_v(ca_sb, W["wvc"])
            on2 = _pb2.tile([128, 8, NC_], BF16, tag="qon", bufs=2, name="on2")
            attention(qtc, ktc, vvc, on2, masked=False)
            out_proj(on2, W["woc"], bocpp)

        # ---- FFN ----
        ln3 = layer_norm_h(2)
        with tc.tile_pool(name=f"C{ibody}", bufs=1) as pc:
            FH = 4 * E // 128
            ht = pc.tile([128, FH, NC_], BF16, tag="ht", name="ht")
            for m in range(FH):
                w1m = pc.tile([128, KO, 128], BF16, tag="w1m", bufs=6, name="w1m")
                nc.sync.dma_start(
                    out=w1m[:],
                    in_=W["w1"][m].rearrange("p (ko f) -> p ko f", f=128))
                p_ = ps()
                for ko in range(KO):
                    nc.tensor.matmul(p_[:, :], w1m[:, ko, :], ln3[:, ko, :],
                                     start=(ko == 0), stop=(ko == KO - 1))
                nc.scalar.activation(ht[:, m, :], p_[:, :], AF.Relu,
                                     bias=b1pp[:, m:m + 1])
            for m in range(KO):
                w2m = pc.tile([128, FH, 128], BF16, tag="w2m", bufs=2, name="w2m")
                nc.sync.dma_start(
                    out=w2m[:],
                    in_=W["w2"][m].rearrange("p (ks e) -> p ks e", e=128))
                p_ = ps()
                for ks in range(FH):
                    nc.tensor.matmul(p_[:, :], w2m[:, ks, :], ht[:, ks, :],
                                     start=(ks == 0), stop=(ks == FH - 1))
                nc.vector.scalar_tensor_tensor(xTo_sb[:, m, :], p_[:, :],
                                               b2pp[:, m:m + 1], xTo_sb[:, m, :],
                                               OP.add, OP.add)
                nc.sync.dma_start(out=out_xT[m * 128:(m + 1) * 128, :],
                                  in_=xTo_sb[:, m, :])


def _make_core_inputs_nocc(c, inp):
    bf = ml_dtypes.bfloat16
    b, h = divmod(c, 2)
    sc = float(E) ** -0.5
    own = slice(512 * h, 512 * h + 512)
    oth = slice(512 * (1 - h), 512 * (1 - h) + 512)

    def stack_heads(w):  # [16, E, D] -> [E, 1024]
        return np.ascontiguousarray(np.transpose(w, (1, 0, 2)).reshape(E, E))

    def qkv_layout(w):  # [E, 1024] -> [2(half), 128, KO*512], contiguous DMA
        a = w.reshape(8, 128, 1024).transpose(1, 0, 2)      # [p, ko, m]
        return np.ascontiguousarray(np.stack(
            [a[:, :, :512].reshape(128, 8 * 512),
             a[:, :, 512:].reshape(128, 8 * 512)])).astype(bf)

    def mtile_layout(w, km, cm):  # [km*128, cm*128] -> [cm, 128, km*128]
        a = w.reshape(km, 128, cm, 128).transpose(2, 1, 0, 3)
        return np.ascontiguousarray(a.reshape(cm, 128, km * 128)).astype(bf)

    def pkt_layout(xT):  # [E, ncols] -> [128, KO*ncols]
        n = xT.shape[1]
        return np.ascontiguousarray(
            xT.reshape(8, 128, n).transpose(1, 0, 2).reshape(128, 8 * n))

    xt = np.asarray(inp["x"][b], np.float32)           # [T, E]
    xperm = np.concatenate([xt[own], xt[oth]], axis=0)  # keys permuted: own first
    # causal mask in permuted key order, own queries t = 512h + f
    s_perm = np.arange(1024)
    s_glob = np.where(s_perm < 512, s_perm + 512 * h, s_perm - 512 * h)
    f = np.arange(512)
    mask = (s_glob[:, None] <= (512 * h + f)[None, :])  # [1024, 512]
    smask = mask.reshape(8, 128, 512).transpose(1, 0, 2).astype(bf)

    return {
        "xTb": np.ascontiguousarray(xperm.T).astype(bf),
        "xTo": pkt_layout(xt[own].T).astype(np.float32),
        "caT": pkt_layout(np.asarray(inp["ca"][b]).T).astype(bf),
        "wq": qkv_layout(stack_heads(inp["Wq_s"]) * sc),
        "wk": qkv_layout(stack_heads(inp["Wk_s"])),
        "wv": qkv_layout(stack_heads(inp["Wv_s"])),
        "wqc": qkv_layout(stack_heads(inp["Wq_c"]) * sc),
        "wkc": qkv_layout(stack_heads(inp["Wk_c"])),
        "wvc": qkv_layout(stack_heads(inp["Wv_c"])),
        "wo": mtile_layout(np.asarray(inp["Wo_s"], np.float32), 8, 8),
        "woc": mtile_layout(np.asarray(inp["Wo_c"], np.float32), 8, 8),
        "w1": mtile_layout(np.asarray(inp["W1"], np.float32), 8, 32),
        "w2": mtile_layout(np.asarray(inp["W2"], np.float32), 32, 8),
        "gb1": np.stack([inp["ln1_g"], inp["ln1_b"]]).astype(bf),
        "gb2": np.stack([inp["ln2_g"], inp["ln2_b"]]).astype(bf),
        "gb3": np.stack([inp["ln3_g"], inp["ln3_b"]]).astype(bf),
        "g1": np.asarray(inp["ln1_g"], np.float32),
        "g2": np.asarray(inp["ln2_g"], np.float32),
        "g3": np.asarray(inp["ln3_g"], np.float32),
        "bo": np.asarray(inp["bo_s"], np.float32),
        "boc": np.asarray(inp["bo_c"], np.float32),
        "b2": np.asarray(inp["b2"], np.float32),
        "b1r": np.asarray(inp["b1"], np.float32),
        "smask": smask,
    }

